# revision 32
# baseline (speedup 1.0000x reference)
"""LocallyConnected2d Trainium2 kernel (8-core SPMD).

out[b,o,p,q] = sum_{i,kh,kw} x[b, i, 2p+kh, 2q+kw] * weight[0, o, i, p, q, kh*3+kw]

Shipped variant "v10":
- Shard the H' (=31) output-row dim across 8 cores (4 rows/core; core 7
  gets one duplicated padding row so the SPMD program is uniform). This
  splits the dominant traffic — the 35.4MB per-location weight — 8 ways,
  unlike batch sharding which would replicate it on every core.
- Host-side im2col + layout prep (pure data movement, no math): weight
  and windows are laid out per-core as [96 partitions = (i,k)-chunk,
  free = (group: weight-cols | window-cols)] in fp16, interleaved into a
  SINGLE DRAM tensor so each group needs exactly ONE input DMA (4 total;
  each dma_start costs ~1-2us of serialized ring time here, so DMA count
  matters more than layout).
- Per block of 4 locations: one matmul per contraction chunk,
  lhsT = windows [96, 4*8], rhs = weight [96, 4*32] -> out [32, 128]
  accumulated over the 3 chunks in PSUM; only the 4 diagonal [8, 32]
  tiles are useful (extracted host-side; the 4x moving-side waste is
  free because the kernel is DMA-bound, not PE-bound).
- fp16 (not bf16): same bytes and same PE rate, but 11 mantissa bits
  -> ~3e-4 rel err vs the fp32 reference (bf16 would be ~2.3e-3).
  Variant "v9"/"v9h" (hi+lo split, 3 matmuls) reaches ~4e-6 at ~1.6x
  the time; "v2" is exact fp32 at ~39us.
"""

import os
import numpy as np
import ml_dtypes

import concourse.bacc as bacc
import concourse.mybir as mybir
import concourse.tile as tile
from concourse.bass_utils import run_bass_kernel_spmd

# Problem shapes (hardcoded per contract).
B, CI, H, W = 8, 32, 64, 64
CO = 32
KH = KW = 3
DH = DW = 2
HO = WO = 31
N_CORES = 8
RPC = 4                 # padded H'-rows per core
L = RPC * WO            # 124 locations per core
IK = CI * KH * KW       # 288 contraction
NCHUNK = 3
CK = IK // NCHUNK       # 96 partitions per chunk
GROUPS = RPC            # one compute/DMA group per H'-row
GL = L // GROUPS        # 31 locations per group

W_COLS = L * NCHUNK * CO     # 11904
WIN_COLS = L * NCHUNK * B    # 2976
OUT_COLS = L * B             # 992

_ROWS_PADDED = [[min(4 * c + j, HO - 1) for j in range(RPC)] for c in range(N_CORES)]

_NC_CACHE = {}


V2_GOUT = 256               # psum cols per group in v2: 8 col-blocks x 32 (o)
V2_OUT_COLS = V2_GOUT * GROUPS

# v4: blocked matmuls — BLK locations share one matmul (out is a BLK x BLK
# grid of [b, o] tiles; only the diagonal is useful, extracted host-side).
# fp32r needs moving free dim >= 256 for the 1 cycle/row fast path.
GLP = 32                    # padded locs per group (31 real + 1 dup)
V4_CFG = {
    "v4r": (mybir.dt.float32r, 8, np.float32),
    "v4b": (mybir.dt.bfloat16, 4, ml_dtypes.bfloat16),
    "v4b8": (mybir.dt.bfloat16, 8, ml_dtypes.bfloat16),
}


def _build_nc_v4(repeat, variant):
    dt, BLK, _ = V4_CFG[variant]
    NBLK = GLP // BLK
    gw = NCHUNK * GLP * CO   # 3072 weight cols per group
    gwin = NCHUNK * GLP * B  # 768 win cols per group
    bout = BLK * CO          # out cols per block
    orows = B * BLK          # out rows per block
    out_cols = GROUPS * NBLK * bout

    nc = bacc.Bacc("TRN2", target_bir_lowering=False)
    wT = nc.dram_tensor("wT", [GROUPS * CK, gw], dt, kind="ExternalInput")
    winT = nc.dram_tensor("winT", [GROUPS * CK, gwin], dt, kind="ExternalInput")
    out = nc.dram_tensor("out", [orows, out_cols], mybir.dt.float32, kind="ExternalOutput")

    with tile.TileContext(nc) as tc:
        with (
            tc.tile_pool(name="wp", bufs=3) as wp,
            tc.tile_pool(name="winp", bufs=3) as winp,
            tc.tile_pool(name="pp", bufs=4, space="PSUM") as pp,
            tc.tile_pool(name="op", bufs=4) as op,
        ):
            def body():
                for g in range(GROUPS):
                    wt = wp.tile([CK, gw], dt, tag="wt", name="wt")
                    nc.sync.dma_start(wt[:], wT.ap()[g * CK:(g + 1) * CK, :])
                    wint = winp.tile([CK, gwin], dt, tag="wint", name="wint")
                    nc.sync.dma_start(wint[:], winT.ap()[g * CK:(g + 1) * CK, :])

                    for bl in range(NBLK):
                        ps = pp.tile([orows, bout], mybir.dt.float32, tag="ps", name="ps")
                        for c in range(NCHUNK):
                            nc.tensor.matmul(
                                ps[:],
                                lhsT=wint[:, c * (GLP * B) + bl * (BLK * B):
                                          c * (GLP * B) + (bl + 1) * (BLK * B)],
                                rhs=wt[:, c * (GLP * CO) + bl * bout:
                                       c * (GLP * CO) + (bl + 1) * bout],
                                start=(c == 0),
                                stop=(c == NCHUNK - 1),
                            )
                        ot = op.tile([orows, bout], mybir.dt.float32, tag="ot", name="ot")
                        nc.vector.tensor_copy(ot[:], ps[:])
                        nc.sync.dma_start(
                            out.ap()[:, (g * NBLK + bl) * bout:(g * NBLK + bl + 1) * bout],
                            ot[:],
                        )

            if repeat == 1:
                body()
            else:
                with tc.For_i(0, repeat, 1):
                    body()
    nc.compile()
    return nc


def _build_nc_v5(repeat=1):
    """fp32 exact; all DMAs 128-partition; contraction 128+128+32 with the
    32-row remainder of all 4 groups packed into one 128-row tile."""
    gw = GL * CO     # 992 weight cols per (group, chunk)
    gwin = GL * B    # 248 win cols per (group, chunk)
    nc = bacc.Bacc("TRN2", target_bir_lowering=False)
    w01 = nc.dram_tensor("w01", [GROUPS * 2 * 128, gw], mybir.dt.float32, kind="ExternalInput")
    win01 = nc.dram_tensor("win01", [GROUPS * 2 * 128, gwin], mybir.dt.float32, kind="ExternalInput")
    w2 = nc.dram_tensor("w2", [GROUPS * 32, gw], mybir.dt.float32, kind="ExternalInput")
    win2 = nc.dram_tensor("win2", [GROUPS * 32, gwin], mybir.dt.float32, kind="ExternalInput")
    out = nc.dram_tensor("out", [GROUPS * 128, V2_GOUT], mybir.dt.float32, kind="ExternalOutput")

    with tile.TileContext(nc) as tc:
        with (
            tc.tile_pool(name="wp", bufs=3) as wp,
            tc.tile_pool(name="winp", bufs=3) as winp,
            tc.tile_pool(name="pp", bufs=2, space="PSUM") as pp,
            tc.tile_pool(name="op", bufs=2) as op,
        ):
            def body():
                for g in range(GROUPS):
                    wts, wints = [], []
                    for cc in range(2):
                        wt = wp.tile([128, gw], mybir.dt.float32, tag=f"wt{cc}", name=f"wt{cc}")
                        nc.sync.dma_start(
                            wt[:], w01.ap()[(g * 2 + cc) * 128:(g * 2 + cc + 1) * 128, :])
                        wint = winp.tile([128, gwin], mybir.dt.float32, tag=f"wint{cc}", name=f"wint{cc}")
                        nc.sync.dma_start(
                            wint[:], win01.ap()[(g * 2 + cc) * 128:(g * 2 + cc + 1) * 128, :])
                        wts.append(wt)
                        wints.append(wint)
                    w2t = wp.tile([32, gw], mybir.dt.float32, tag="w2t", name="w2t")
                    nc.sync.dma_start(w2t[:], w2.ap()[g * 32:(g + 1) * 32, :])
                    win2t = winp.tile([32, gwin], mybir.dt.float32, tag="win2t", name="win2t")
                    nc.sync.dma_start(win2t[:], win2.ap()[g * 32:(g + 1) * 32, :])

                    pss = [
                        pp.tile([128, V2_GOUT], mybir.dt.float32,
                                tag=f"ps{j}", name=f"ps{j}", bufs=2)
                        for j in range(4)
                    ]
                    for l in range(GL):
                        j = l % 4
                        blk = l // 4
                        dst = pss[j][32 * j:32 * j + B, blk * CO:(blk + 1) * CO]
                        for cc in range(2):
                            nc.tensor.matmul(
                                dst,
                                lhsT=wints[cc][:, l * B:(l + 1) * B],
                                rhs=wts[cc][:, l * CO:(l + 1) * CO],
                                start=(cc == 0),
                                stop=False,
                                tile_position=(0, 32 * j),
                            )
                        nc.tensor.matmul(
                            dst,
                            lhsT=win2t[:, l * B:(l + 1) * B],
                            rhs=w2t[:, l * CO:(l + 1) * CO],
                            start=False,
                            stop=True,
                            tile_position=(0, 32 * j),
                        )

                    ot = op.tile([128, V2_GOUT], mybir.dt.float32, tag="ot", name="ot")
                    for j in range(4):
                        nc.vector.tensor_copy(
                            ot[32 * j:32 * (j + 1), :],
                            pss[j][32 * j:32 * (j + 1), :],
                        )
                    nc.sync.dma_start(out.ap()[g * 128:(g + 1) * 128, :], ot[:])

            if repeat == 1:
                body()
            else:
                with tc.For_i(0, repeat, 1):
                    body()
    nc.compile()
    return nc


def _host_prep_v5(x, weight):
    x = np.ascontiguousarray(np.asarray(x, dtype=np.float32))
    weight = np.ascontiguousarray(np.asarray(weight, dtype=np.float32))
    wins = np.stack(
        [x[:, :, kh:kh + DH * HO:DH, kw:kw + DW * WO:DW]
         for kh in range(KH) for kw in range(KW)],
        axis=-1,
    )
    W2 = weight[0].transpose(1, 4, 2, 3, 0).reshape(IK, HO, WO, CO)
    W3 = wins.transpose(1, 4, 2, 3, 0).reshape(IK, HO, WO, B)
    in_maps = []
    for c in range(N_CORES):
        rows = _ROWS_PADDED[c]
        wsel = W2[:, rows]       # (288, 4, 31, CO)
        winsel = W3[:, rows]     # (288, 4, 31, B)
        # w01 rows: (g, c01, 128) ; cols (l, o)
        w01 = wsel[:256].reshape(2, 128, GROUPS, GL * CO).transpose(2, 0, 1, 3)
        win01 = winsel[:256].reshape(2, 128, GROUPS, GL * B).transpose(2, 0, 1, 3)
        w2 = wsel[256:].reshape(32, GROUPS, GL * CO).transpose(1, 0, 2)
        win2 = winsel[256:].reshape(32, GROUPS, GL * B).transpose(1, 0, 2)
        in_maps.append({
            "w01": np.ascontiguousarray(w01.reshape(GROUPS * 2 * 128, GL * CO)),
            "win01": np.ascontiguousarray(win01.reshape(GROUPS * 2 * 128, GL * B)),
            "w2": np.ascontiguousarray(w2.reshape(GROUPS * 32, GL * CO)),
            "win2": np.ascontiguousarray(win2.reshape(GROUPS * 32, GL * B)),
        })
    return in_maps


def _assemble_v5(results):
    out = np.empty((B, CO, HO, WO), np.float32)
    qs = np.arange(WO)
    for c in range(N_CORES):
        nreal = RPC if c < N_CORES - 1 else HO - 4 * (N_CORES - 1)
        buf = np.asarray(results[c]["out"])      # [GROUPS*128, 256]
        b5 = buf.reshape(GROUPS, 4, 32, 8, CO)   # (g, strip, 32row, blk, o)
        res = b5[:, qs % 4, :B, qs // 4, :]      # (g?, ...) advanced idx
        # advanced indices qs%4 (dim1) and qs//4 (dim3) -> (31, GROUPS, B, CO)
        out[:, :, 4 * c:4 * c + nreal, :] = res.transpose(2, 3, 1, 0)[:, :, :nreal, :]
    return out


V89_BLK = 4
V89_NBLK = GLP // V89_BLK            # 8 blocks of 4 locs per group
V89_GW = NCHUNK * GLP * CO           # 3072 weight cols per group
V89_GWIN = NCHUNK * GLP * B          # 768 win cols per group
V89_BOUT = V89_BLK * CO              # 128 out cols per block
V89_OROWS = B * V89_BLK              # 32 out rows
V89_OUTC = GROUPS * V89_NBLK * V89_BOUT  # 4096


def _build_nc_v89(repeat=1, three_term=False, dt=None):
    """16-bit blocked kernel, minimal DMA count, split across both HWDGE
    rings. three_term=True computes w≈wh+wl, win≈vh+vl and accumulates
    vh·wh + vh·wl + vl·wh (16-bit products are exact in fp32 -> ~1e-5 rel err).
    """
    if dt is None:
        dt = mybir.dt.bfloat16
    W = GROUPS * V89_GW
    WIN = GROUPS * V89_GWIN
    nc = bacc.Bacc("TRN2", target_bir_lowering=False)
    wh_d = nc.dram_tensor("wh", [CK, W], dt, kind="ExternalInput")
    winh_d = nc.dram_tensor("winh", [CK, WIN], dt, kind="ExternalInput")
    if three_term:
        wl_d = nc.dram_tensor("wl", [CK, W], dt, kind="ExternalInput")
        winl_d = nc.dram_tensor("winl", [CK, WIN], dt, kind="ExternalInput")
    out = nc.dram_tensor("out", [V89_OROWS, V89_OUTC], mybir.dt.float32, kind="ExternalOutput")

    half = W // 2  # 2 groups per ring half
    with tile.TileContext(nc) as tc:
        with (
            tc.tile_pool(name="wp", bufs=2) as wp,
            tc.tile_pool(name="winp", bufs=2) as winp,
            tc.tile_pool(name="pp", bufs=4, space="PSUM") as pp,
            tc.tile_pool(name="op", bufs=2) as op,
        ):
            def body():
                # weight: groups 0-1 via SP ring, groups 2-3 via ACT ring,
                # one piece per group -> compute starts after 1/4 of bytes
                wh = wp.tile([CK, W], dt, tag="wh", name="wh")
                for g in range(2):
                    nc.sync.dma_start(
                        wh[:, g * V89_GW:(g + 1) * V89_GW],
                        wh_d.ap()[:, g * V89_GW:(g + 1) * V89_GW])
                for g in range(2, 4):
                    nc.scalar.dma_start(
                        wh[:, g * V89_GW:(g + 1) * V89_GW],
                        wh_d.ap()[:, g * V89_GW:(g + 1) * V89_GW])
                winh = winp.tile([CK, WIN], dt, tag="winh", name="winh")
                nc.sync.dma_start(winh[:, :WIN // 2], winh_d.ap()[:, :WIN // 2])
                nc.scalar.dma_start(winh[:, WIN // 2:], winh_d.ap()[:, WIN // 2:])
                if three_term:
                    wl = wp.tile([CK, W], dt, tag="wl", name="wl")
                    for g in range(2):
                        nc.scalar.dma_start(
                            wl[:, g * V89_GW:(g + 1) * V89_GW],
                            wl_d.ap()[:, g * V89_GW:(g + 1) * V89_GW])
                    for g in range(2, 4):
                        nc.sync.dma_start(
                            wl[:, g * V89_GW:(g + 1) * V89_GW],
                            wl_d.ap()[:, g * V89_GW:(g + 1) * V89_GW])
                    winl = winp.tile([CK, WIN], dt, tag="winl", name="winl")
                    nc.scalar.dma_start(winl[:, :WIN // 2], winl_d.ap()[:, :WIN // 2])
                    nc.sync.dma_start(winl[:, WIN // 2:], winl_d.ap()[:, WIN // 2:])

                ot = op.tile([V89_OROWS, V89_OUTC], mybir.dt.float32, tag="ot", name="ot")
                for g in range(GROUPS):
                    for bl in range(V89_NBLK):
                        ps = pp.tile([V89_OROWS, V89_BOUT], mybir.dt.float32, tag="ps", name="ps")
                        first = True
                        for c in range(NCHUNK):
                            lo = g * V89_GWIN + c * (GLP * B) + bl * (V89_BLK * B)
                            ro = g * V89_GW + c * (GLP * CO) + bl * V89_BOUT
                            lhs_h = winh[:, lo:lo + V89_BLK * B]
                            rhs_h = wh[:, ro:ro + V89_BOUT]
                            terms = [(lhs_h, rhs_h)]
                            if three_term:
                                terms.append((lhs_h, wl[:, ro:ro + V89_BOUT]))
                                terms.append((winl[:, lo:lo + V89_BLK * B], rhs_h))
                            for ti, (lh, rh) in enumerate(terms):
                                last = (c == NCHUNK - 1) and (ti == len(terms) - 1)
                                nc.tensor.matmul(
                                    ps[:], lhsT=lh, rhs=rh,
                                    start=first, stop=last)
                                first = False
                        nc.vector.tensor_copy(
                            ot[:, (g * V89_NBLK + bl) * V89_BOUT:(g * V89_NBLK + bl + 1) * V89_BOUT],
                            ps[:])
                nc.gpsimd.dma_start(out.ap()[:, :], ot[:])

            if repeat == 1:
                body()
            else:
                with tc.For_i(0, repeat, 1):
                    body()
    nc.compile()
    return nc


def _host_prep_v89(x, weight, three_term=False, npdt=None):
    if npdt is None:
        npdt = ml_dtypes.bfloat16
    x = np.ascontiguousarray(np.asarray(x, dtype=np.float32))
    weight = np.ascontiguousarray(np.asarray(weight, dtype=np.float32))
    wins = np.stack(
        [x[:, :, kh:kh + DH * HO:DH, kw:kw + DW * WO:DW]
         for kh in range(KH) for kw in range(KW)],
        axis=-1,
    )
    W2 = weight[0].transpose(1, 4, 2, 3, 0).reshape(IK, HO, WO, CO)
    W3 = wins.transpose(1, 4, 2, 3, 0).reshape(IK, HO, WO, B)
    qpad = list(range(WO)) + [WO - 1]
    in_maps = []
    for c in range(N_CORES):
        rows = _ROWS_PADDED[c]
        wsel = W2[:, rows][:, :, qpad, :]       # (288, 4, 32, CO)
        winsel = W3[:, rows][:, :, qpad, :]     # (288, 4, 32, B)
        # -> [CK, (group, chunk, locp, {o|b})]
        wstk = np.stack([wsel[CK * cc:CK * (cc + 1)] for cc in range(NCHUNK)], axis=2)
        winstk = np.stack([winsel[CK * cc:CK * (cc + 1)] for cc in range(NCHUNK)], axis=2)
        # (CK, 4, chunk, 32, X) -> (CK, group*chunk*locp*X)
        wfull = wstk.reshape(CK, GROUPS * NCHUNK * GLP * CO)
        winfull = winstk.reshape(CK, GROUPS * NCHUNK * GLP * B)
        m = {}
        wh = wfull.astype(npdt)
        vh = winfull.astype(npdt)
        m["wh"] = np.ascontiguousarray(wh)
        m["winh"] = np.ascontiguousarray(vh)
        if three_term:
            m["wl"] = np.ascontiguousarray(
                (wfull - wh.astype(np.float32)).astype(npdt))
            m["winl"] = np.ascontiguousarray(
                (winfull - vh.astype(np.float32)).astype(npdt))
        in_maps.append(m)
    return in_maps


def _assemble_v89(results):
    BLK = V89_BLK
    NBLK = V89_NBLK
    out = np.empty((B, CO, HO, WO), np.float32)
    idx = np.arange(BLK)
    for c in range(N_CORES):
        nreal = RPC if c < N_CORES - 1 else HO - 4 * (N_CORES - 1)
        buf = np.asarray(results[c]["out"])          # [32, 4096]
        b6 = buf.reshape(BLK, B, GROUPS, NBLK, BLK, CO)
        d = b6[idx, :, :, :, idx, :]                 # (BLK, B, G, NBLK, CO)
        dd = d.transpose(1, 4, 2, 3, 0).reshape(B, CO, GROUPS, NBLK * BLK)
        out[:, :, 4 * c:4 * c + nreal, :] = dd[:, :, :nreal, :WO]
    return out


V11_NP = 8                      # pieces (half H'-rows) per core
V11_PL = 16                     # padded locations per piece
V11_NBLK = 4                    # blocks of BLK=4 locs per piece
V11_GW = NCHUNK * V11_PL * CO   # 1536 weight cols per piece
V11_GWIN = NCHUNK * V11_PL * B  # 384 win cols per piece
V11_GTOT = V11_GW + V11_GWIN    # 1920
V11_POUT = V11_NBLK * V89_BLK * CO  # 512 out cols per piece
V11_OUTC = V11_NP * V11_POUT    # 4096


def _build_nc_v11(repeat=1, dt=None):
    """8 self-contained pieces (16 locs each), one input DMA per piece on the
    SP queue; matmuls accumulate into a [32, 512] PSUM tile per piece (one
    bank); output DMA'd straight from PSUM on the Pool/SWDGE queue — no
    PSUM->SBUF copies at all."""
    if dt is None:
        dt = mybir.dt.float16
    BLK = V89_BLK
    orows = V89_OROWS           # 32 = BLK * B
    nc = bacc.Bacc("TRN2", target_bir_lowering=False)
    wx = nc.dram_tensor("wx", [CK, V11_NP * V11_GTOT], dt, kind="ExternalInput")
    out = nc.dram_tensor("out", [orows, V11_OUTC], mybir.dt.float32, kind="ExternalOutput")
    with tile.TileContext(nc) as tc:
        with (
            tc.tile_pool(name="wp", bufs=2) as wp,
            tc.tile_pool(name="pp", bufs=4, space="PSUM") as pp,
        ):
            def body():
                t = wp.tile([CK, V11_NP * V11_GTOT], dt, tag="t", name="t")
                for p in range(V11_NP):
                    nc.sync.dma_start(t[:, p * V11_GTOT:(p + 1) * V11_GTOT],
                                      wx.ap()[:, p * V11_GTOT:(p + 1) * V11_GTOT])
                for p in range(V11_NP):
                    base = p * V11_GTOT
                    ps = pp.tile([orows, V11_POUT], mybir.dt.float32, tag="ps", name="ps")
                    for bl in range(V11_NBLK):
                        for c in range(NCHUNK):
                            lo = base + V11_GW + c * (V11_PL * B) + bl * (BLK * B)
                            ro = base + c * (V11_PL * CO) + bl * (BLK * CO)
                            nc.tensor.matmul(
                                ps[:, bl * (BLK * CO):(bl + 1) * (BLK * CO)],
                                lhsT=t[:, lo:lo + BLK * B],
                                rhs=t[:, ro:ro + BLK * CO],
                                start=(c == 0), stop=(c == NCHUNK - 1))
                    nc.gpsimd.dma_start(
                        out.ap()[:, p * V11_POUT:(p + 1) * V11_POUT], ps[:])
            if repeat == 1:
                body()
            else:
                with tc.For_i(0, repeat, 1):
                    body()
    nc.compile()
    return nc


def _host_prep_v11(x, weight, npdt=None):
    if npdt is None:
        npdt = np.float16
    m = _host_prep_v89(x, weight, three_term=False, npdt=npdt)
    out_maps = []
    for mm in m:
        # wh: (CK, group4, chunk3, locp32, CO) ; winh: (..., B)
        wh = mm["wh"].reshape(CK, GROUPS, NCHUNK, GLP, CO)
        vh = mm["winh"].reshape(CK, GROUPS, NCHUNK, GLP, B)
        # -> (CK, row4, half2, chunk3, loc16, X)
        wp = wh.reshape(CK, GROUPS, NCHUNK, 2, V11_PL, CO).transpose(0, 1, 3, 2, 4, 5)
        vp = vh.reshape(CK, GROUPS, NCHUNK, 2, V11_PL, B).transpose(0, 1, 3, 2, 4, 5)
        wp = wp.reshape(CK, V11_NP, V11_GW)
        vp = vp.reshape(CK, V11_NP, V11_GWIN)
        wx = np.concatenate([wp, vp], axis=2).reshape(CK, V11_NP * V11_GTOT)
        out_maps.append({"wx": np.ascontiguousarray(wx)})
    return out_maps


def _assemble_v11(results):
    BLK = V89_BLK
    out = np.empty((B, CO, HO, WO), np.float32)
    idx = np.arange(BLK)
    for c in range(N_CORES):
        nreal = RPC if c < N_CORES - 1 else HO - 4 * (N_CORES - 1)
        buf = np.asarray(results[c]["out"])          # [32, 4096]
        b6 = buf.reshape(BLK, B, V11_NP, V11_NBLK, BLK, CO)
        d = b6[idx, :, :, :, idx, :]                 # (BLK, B, P, NBLK, CO)
        dd = d.transpose(1, 4, 2, 3, 0).reshape(B, CO, GROUPS, 2 * V11_NBLK * BLK)
        out[:, :, 4 * c:4 * c + nreal, :] = dd[:, :, :nreal, :WO]
    return out


V12_NBLKS = 32                  # blocks of BLK=4 locs per core (4 rows x 8)
V12_UNIT = V89_BLK * (B + CO)   # 160 cols per (block, chunk) unit: win | w
V12_BCOLS = NCHUNK * V12_UNIT   # 480 cols per block
V12_PIECES = (5, 5, 5, 5, 4, 4, 4)  # input DMA piece sizes in blocks (sum 32)
V12_OUTC = V12_NBLKS * V89_BLK * CO  # 4096 fp16 out cols (blocked, diag on host)
V12_NGRP = 8                         # compute groups of 4 blocks (1 PSUM bank)


def _build_nc_v12(repeat=1, dt=None, flat=False):
    """Block-major stream: 9 input pieces on SP (tiny last piece to shrink
    the tail), matmuls accumulate in per-piece PSUM banks (all 8), one
    contiguous fp32->fp16 PSUM->SBUF copy per piece round-robin over
    DVE/ACT/Pool, 2 output DMAs; diagonal extraction happens host-side.
    repeat>1 timing builds unroll 8 bodies per For_i iteration so
    consecutive bodies pipeline (the all-engine barrier is per-For_i-iter)."""
    if dt is None:
        dt = mybir.dt.float16
    BLK = V89_BLK
    nc = bacc.Bacc("TRN2", target_bir_lowering=False)
    wx = nc.dram_tensor("wx", [CK, V12_NBLKS * V12_BCOLS], dt, kind="ExternalInput")
    out = nc.dram_tensor("out", [32, V12_OUTC], mybir.dt.float16, kind="ExternalOutput")
    starts = []
    s = 0
    for n in V12_PIECES:
        starts.append(s)
        s += n
    assert s == V12_NBLKS
    BC = BLK * CO
    with tile.TileContext(nc) as tc:
        with (
            tc.tile_pool(name="wp", bufs=2) as wp,
            tc.tile_pool(name="pp", bufs=7, space="PSUM") as pp,
            tc.tile_pool(name="op", bufs=2) as op,
        ):
            def body():
                t = wp.tile([CK, V12_NBLKS * V12_BCOLS], dt, tag="t", name="t")
                for s, n in zip(starts, V12_PIECES):
                    nc.sync.dma_start(
                        t[:, s * V12_BCOLS:(s + n) * V12_BCOLS],
                        wx.ap()[:, s * V12_BCOLS:(s + n) * V12_BCOLS])
                # PE p-state warmup: ~3.4us of dummy matmuls on a memset tile
                # so real matmuls run at full clock once piece 0 lands.
                wu = wp.tile([CK, 256], dt, tag="wu", name="wu")
                nc.vector.memset(wu[:], 0.0)
                psw = pp.tile([32, 512], mybir.dt.float32, tag="psw", name="psw",
                              bufs=1)
                for i in range(12):
                    nc.tensor.matmul(psw[0:8, 0:256], lhsT=wu[:, :8], rhs=wu[:, :256],
                                     start=(i == 0), stop=(i == 11))
                for i in range(10):
                    nc.tensor.matmul(psw[0:8, 256:320], lhsT=wu[:, :8], rhs=wu[:, :64],
                                     start=(i == 0), stop=(i == 9))
                ot = op.tile([32, V12_OUTC], mybir.dt.float16, tag="ot", name="ot")
                for g in range(V12_NGRP):
                    ps = pp.tile([32, 4 * BC], mybir.dt.float32,
                                 tag="ps", name="ps")
                    for li in range(4):
                        u0 = (4 * g + li) * NCHUNK
                        for c in range(NCHUNK):
                            off = (u0 + c) * V12_UNIT
                            nc.tensor.matmul(
                                ps[:, li * BC:(li + 1) * BC],
                                lhsT=t[:, off:off + BLK * B],
                                rhs=t[:, off + BLK * B:off + V12_UNIT],
                                start=(c == 0), stop=(c == NCHUNK - 1))
                    dst = ot[:, g * 4 * BC:(g + 1) * 4 * BC]
                    if g % 2 == 0:
                        nc.vector.tensor_copy(dst, ps[:])
                    else:
                        nc.scalar.copy(dst, ps[:])
                nc.scalar.dma_start(out.ap()[:, :], ot[:, :])
            if repeat == 1:
                body()
            elif flat:
                for _ in range(repeat):
                    body()
            else:
                u = 8 if repeat % 8 == 0 else (4 if repeat % 4 == 0 else 1)
                with tc.For_i(0, repeat // u, 1):
                    for _ in range(u):
                        body()
    nc.compile()
    return nc


def _host_prep_v12(x, weight, npdt=None):
    if npdt is None:
        npdt = np.float16
    m = _host_prep_v89(x, weight, three_term=False, npdt=npdt)
    out_maps = []
    for mm in m:
        # wh: (CK, group4, chunk3, locp32, CO); winh: (..., B)
        wh = mm["wh"].reshape(CK, GROUPS, NCHUNK, GLP, CO)
        vh = mm["winh"].reshape(CK, GROUPS, NCHUNK, GLP, B)
        # -> (CK, block(row,bb)=32, chunk3, BLK locs, X) with loc = blk*4+j
        wb = wh.reshape(CK, GROUPS, NCHUNK, 8, V89_BLK, CO)
        vb = vh.reshape(CK, GROUPS, NCHUNK, 8, V89_BLK, B)
        wb = wb.transpose(0, 1, 3, 2, 4, 5).reshape(CK, V12_NBLKS, NCHUNK, V89_BLK * CO)
        vb = vb.transpose(0, 1, 3, 2, 4, 5).reshape(CK, V12_NBLKS, NCHUNK, V89_BLK * B)
        wx = np.concatenate([vb, wb], axis=3)  # (CK, blk, chunk, 32+128)
        out_maps.append({"wx": np.ascontiguousarray(
            wx.reshape(CK, V12_NBLKS * V12_BCOLS))})
    return out_maps


def _assemble_v12(results):
    BLK = V89_BLK
    out = np.empty((B, CO, HO, WO), np.float32)
    qs = np.arange(WO)
    idx = np.arange(BLK)
    for c in range(N_CORES):
        nreal = RPC if c < N_CORES - 1 else HO - 4 * (N_CORES - 1)
        buf = np.asarray(results[c]["out"]).astype(np.float32)  # [32, 4096]
        b6 = buf.reshape(BLK, B, GROUPS, 8, BLK, CO)  # (j, b, row, bb, j', o)
        d = b6[idx, :, :, :, idx, :]                  # (BLK, B, row, bb, o)
        res = d[qs % 4, :, :, qs // 4, :]             # (31, b, row, o)
        out[:, :, 4 * c:4 * c + nreal, :] = res.transpose(1, 3, 2, 0)[:, :, :nreal, :]
    return out


V13_GCOLS = 4 * 2 * (B + CO + B + CO) + 0  # placeholder, see below
V13_UNIT = V89_BLK * (B + CO)       # 160 cols per (block, chunk) unit
V13_GRP = 4 * 2 * V13_UNIT + V13_UNIT  # 1440 cols per 4-block group: A|B pairs + C band
V13_NGRP = 8
V13_OUTC = 1024                     # out [128, 1024] fp16


def _build_nc_v13(repeat=1, dt=None, flat=False, dma_only=False):
    """128-partition DMA layout: contraction 288 = A(0:128) + B(128:256) on all
    partitions + C(256:288) packed 4-blocks-per-band (base 32*li). Balances
    per-partition DMA bytes at 23KB (the HW stream is per-partition limited at
    ~2.6GB/s). Matmul outputs land in rotating PSUM bands (32*(g%4)); one
    [128,512] copy per 4 groups (DVE then ACT); out [128,1024] fp16.
    8 DMAs/body total so the 8 rotating DMA-completion semaphores stay
    body-aligned; repeat>1 unrolls 8 bodies per For_i iteration."""
    if dt is None:
        dt = mybir.dt.float16
    BLK = V89_BLK
    nc = bacc.Bacc("TRN2", target_bir_lowering=False)
    wx = nc.dram_tensor("wx", [128, V13_NGRP * V13_GRP], dt, kind="ExternalInput")
    out = nc.dram_tensor("out", [128, V13_OUTC], mybir.dt.float16, kind="ExternalOutput")
    with tile.TileContext(nc) as tc:
        with (
            tc.tile_pool(name="wp", bufs=2) as wp,
            tc.tile_pool(name="pp", bufs=2, space="PSUM") as pp,
            tc.tile_pool(name="op", bufs=2) as op,
        ):
            def body():
                t = wp.tile([128, V13_NGRP * V13_GRP], dt, tag="t", name="t")
                # 7 input DMAs: piece 0 covers groups 0-1, rest one group each
                bounds = [0, 2, 3, 4, 5, 6, 7, 8]
                for i in range(7):
                    lo, hi = bounds[i] * V13_GRP, bounds[i + 1] * V13_GRP
                    nc.sync.dma_start(t[:, lo:hi], wx.ap()[:, lo:hi])
                if dma_only:
                    return
                wu = wp.tile([128, 256], dt, tag="wu", name="wu")
                nc.vector.memset(wu[:], 0.0)
                psw = pp.tile([32, 512], mybir.dt.float32, tag="psw", name="psw",
                              bufs=1)
                for i in range(12):
                    nc.tensor.matmul(psw[0:8, 0:256], lhsT=wu[:, :8], rhs=wu[:, :256],
                                     start=(i == 0), stop=(i == 11))
                for i in range(10):
                    nc.tensor.matmul(psw[0:8, 256:320], lhsT=wu[:, :8], rhs=wu[:, :64],
                                     start=(i == 0), stop=(i == 9))
                ot = op.tile([128, V13_OUTC], mybir.dt.float16, tag="ot", name="ot")
                for h in range(2):
                    psf = pp.tile([128, 512], mybir.dt.float32, tag="ps", name="ps")
                    for bi in range(4):
                        g = 4 * h + bi
                        base = g * V13_GRP
                        for li in range(4):
                            ab = base + li * (2 * V13_UNIT)
                            cw = base + 8 * V13_UNIT
                            dst = psf[32 * bi:32 * (bi + 1), li * 128:(li + 1) * 128]
                            nc.tensor.matmul(
                                dst, lhsT=t[:, ab:ab + 32],
                                rhs=t[:, ab + 32:ab + V13_UNIT],
                                start=True, stop=False,
                                tile_position=(0, 32 * bi))
                            nc.tensor.matmul(
                                dst, lhsT=t[:, ab + V13_UNIT:ab + V13_UNIT + 32],
                                rhs=t[:, ab + V13_UNIT + 32:ab + 2 * V13_UNIT],
                                start=False, stop=False,
                                tile_position=(0, 32 * bi))
                            nc.tensor.matmul(
                                dst,
                                lhsT=t[32 * li:32 * (li + 1), cw:cw + 32],
                                rhs=t[32 * li:32 * (li + 1), cw + 32:cw + V13_UNIT],
                                start=False, stop=True,
                                tile_position=(32 * li, 32 * bi))
                    dst = ot[:, h * 512:(h + 1) * 512]
                    if h == 0:
                        nc.vector.tensor_copy(dst, psf[:])
                    else:
                        nc.scalar.copy(dst, psf[:])
                nc.scalar.dma_start(out.ap()[:, :], ot[:, :])
            if repeat == 1:
                body()
            elif flat:
                for _ in range(repeat):
                    body()
            else:
                u = 8 if repeat % 8 == 0 else (4 if repeat % 4 == 0 else 1)
                with tc.For_i(0, repeat // u, 1):
                    for _ in range(u):
                        body()
    nc.compile()
    return nc


def _host_prep_v13(x, weight, npdt=None):
    if npdt is None:
        npdt = np.float16
    x = np.ascontiguousarray(np.asarray(x, dtype=np.float32))
    weight = np.ascontiguousarray(np.asarray(weight, dtype=np.float32))
    wins = np.stack(
        [x[:, :, kh:kh + DH * HO:DH, kw:kw + DW * WO:DW]
         for kh in range(KH) for kw in range(KW)],
        axis=-1,
    )
    W2 = weight[0].transpose(1, 4, 2, 3, 0).reshape(IK, HO, WO, CO)
    W3 = wins.transpose(1, 4, 2, 3, 0).reshape(IK, HO, WO, B)
    qpad = list(range(WO)) + [WO - 1]
    in_maps = []
    for c in range(N_CORES):
        rows = _ROWS_PADDED[c]
        wsel = W2[:, rows][:, :, qpad, :].astype(npdt)    # (288, 4, 32, CO)
        winsel = W3[:, rows][:, :, qpad, :].astype(npdt)  # (288, 4, 32, B)
        wx = np.zeros((128, V13_NGRP * V13_GRP), npdt)
        for g in range(V13_NGRP):
            row, half = g // 2, g % 2
            base = g * V13_GRP
            for li in range(4):
                bb = 4 * half + li
                ls = slice(bb * 4, (bb + 1) * 4)     # 4 locs of this block
                ab = base + li * (2 * V13_UNIT)
                # A unit: ik 0:128
                wx[:, ab:ab + 32] = winsel[0:128, row, ls, :].reshape(128, 32)
                wx[:, ab + 32:ab + V13_UNIT] = wsel[0:128, row, ls, :].reshape(128, 128)
                # B unit: ik 128:256
                wx[:, ab + V13_UNIT:ab + V13_UNIT + 32] = \
                    winsel[128:256, row, ls, :].reshape(128, 32)
                wx[:, ab + V13_UNIT + 32:ab + 2 * V13_UNIT] = \
                    wsel[128:256, row, ls, :].reshape(128, 128)
                # C band: ik 256:288 at partitions 32*li
                cw = base + 8 * V13_UNIT
                wx[32 * li:32 * (li + 1), cw:cw + 32] = \
                    winsel[256:288, row, ls, :].reshape(32, 32)
                wx[32 * li:32 * (li + 1), cw + 32:cw + V13_UNIT] = \
                    wsel[256:288, row, ls, :].reshape(32, 128)
        in_maps.append({"wx": np.ascontiguousarray(wx)})
    return in_maps


def _assemble_v13(results):
    BLK = V89_BLK
    out = np.empty((B, CO, HO, WO), np.float32)
    idx = np.arange(BLK)
    for c in range(N_CORES):
        nreal = RPC if c < N_CORES - 1 else HO - 4 * (N_CORES - 1)
        buf = np.asarray(results[c]["out"]).astype(np.float32)  # [128, 1024]
        b6 = buf.reshape(4, BLK, B, 2, 4, BLK, CO)  # (band, j, b, h, li, j', o)
        d = b6[:, idx, :, :, :, idx, :]             # (j, band, b, h, li, o)
        for g in range(V13_NGRP):
            row, half = g // 2, g % 2
            if row >= nreal:
                continue
            # q = (4*half + li)*4 + j
            blkq = d[:, g % 4, :, g // 4, :, :]     # (j, b, li, o)
            q0 = 16 * half
            arr = blkq.transpose(1, 3, 2, 0).reshape(B, CO, 16)  # (b, o, li*4+j)
            qs = np.arange(q0, q0 + 16)
            sel = qs < WO
            out[:, :, 4 * c + row, qs[sel]] = arr[:, :, sel]
        if nreal < RPC:
            pass
    return out


V10_GTOT = NCHUNK * GLP * CO + NCHUNK * GLP * B   # 3840 cols/group: weight | windows


def _build_nc_v10(repeat=1, dt=None):
    """Like v8h but weight+windows interleaved per group in ONE DRAM tensor:
    one DMA per group (4 input DMAs total) — each dma_start costs ~1.5us of
    serialized ring time here, so DMA count is the dominant knob."""
    if dt is None:
        dt = mybir.dt.float16
    BLK = V89_BLK
    NBLK = V89_NBLK
    gw = V89_GW
    gtot = V10_GTOT
    bout = V89_BOUT
    orows = V89_OROWS
    nc = bacc.Bacc("TRN2", target_bir_lowering=False)
    wx = nc.dram_tensor("wx", [CK, GROUPS * gtot], dt, kind="ExternalInput")
    out = nc.dram_tensor("out", [orows, V89_OUTC], mybir.dt.float32, kind="ExternalOutput")
    with tile.TileContext(nc) as tc:
        with (
            tc.tile_pool(name="wp", bufs=2) as wp,
            tc.tile_pool(name="pp", bufs=4, space="PSUM") as pp,
            tc.tile_pool(name="op", bufs=2) as op,
        ):
            def body():
                t = wp.tile([CK, GROUPS * gtot], dt, tag="t", name="t")
                for g in range(GROUPS):
                    nc.sync.dma_start(t[:, g * gtot:(g + 1) * gtot],
                                      wx.ap()[:, g * gtot:(g + 1) * gtot])
                ot = op.tile([orows, V89_OUTC], mybir.dt.float32, tag="ot", name="ot")
                gout = NBLK * bout
                for g in range(GROUPS):
                    base = g * gtot
                    for bl in range(NBLK):
                        ps = pp.tile([orows, bout], mybir.dt.float32, tag="ps", name="ps")
                        for c in range(NCHUNK):
                            lo = base + gw + c * (GLP * B) + bl * (BLK * B)
                            ro = base + c * (GLP * CO) + bl * bout
                            nc.tensor.matmul(
                                ps[:],
                                lhsT=t[:, lo:lo + BLK * B],
                                rhs=t[:, ro:ro + bout],
                                start=(c == 0), stop=(c == NCHUNK - 1))
                        nc.vector.tensor_copy(
                            ot[:, (g * NBLK + bl) * bout:(g * NBLK + bl + 1) * bout], ps[:])
                    if g == GROUPS - 2:
                        # first 3/4 of the output leaves while group 3 computes
                        nc.gpsimd.dma_start(out.ap()[:, :3 * gout], ot[:, :3 * gout])
                nc.gpsimd.dma_start(out.ap()[:, 3 * gout:], ot[:, 3 * gout:])
            if repeat == 1:
                body()
            else:
                with tc.For_i(0, repeat, 1):
                    body()
    nc.compile()
    return nc


def _host_prep_v10(x, weight, npdt=None):
    if npdt is None:
        npdt = np.float16
    maps = _host_prep_v89(x, weight, three_term=False, npdt=npdt)
    gw = V89_GW
    gwin = V89_GWIN
    out_maps = []
    for m in maps:
        wh = m["wh"].reshape(CK, GROUPS, gw)
        vh = m["winh"].reshape(CK, GROUPS, gwin)
        wx = np.concatenate([wh, vh], axis=2).reshape(CK, GROUPS * V10_GTOT)
        out_maps.append({"wx": np.ascontiguousarray(wx)})
    return out_maps


def _host_prep_v4(x, weight, variant):
    dt, BLK, npdt = V4_CFG[variant]
    x = np.ascontiguousarray(np.asarray(x, dtype=np.float32))
    weight = np.ascontiguousarray(np.asarray(weight, dtype=np.float32))
    wins = np.stack(
        [x[:, :, kh:kh + DH * HO:DH, kw:kw + DW * WO:DW]
         for kh in range(KH) for kw in range(KW)],
        axis=-1,
    )
    W2 = weight[0].transpose(1, 4, 2, 3, 0).reshape(IK, HO, WO, CO)
    W3 = wins.transpose(1, 4, 2, 3, 0).reshape(IK, HO, WO, B)
    qpad = list(range(WO)) + [WO - 1]          # 31 real + 1 dup -> 32
    in_maps = []
    for c in range(N_CORES):
        rows = _ROWS_PADDED[c]
        # (ik, group, locp, {o|b})
        wsel = W2[:, rows][:, :, qpad, :]       # (288, 4, 32, CO)
        winsel = W3[:, rows][:, :, qpad, :]     # (288, 4, 32, B)
        # -> [group, CK, chunk, locp, {o|b}] -> [GROUPS*CK, chunk*locp*{o|b}]
        wstk = np.stack([wsel[CK * cc:CK * (cc + 1)] for cc in range(NCHUNK)], axis=2)
        winstk = np.stack([winsel[CK * cc:CK * (cc + 1)] for cc in range(NCHUNK)], axis=2)
        # wstk: (CK, 4, chunk, 32, CO) -> (4, CK, chunk, 32, CO)
        wstk = wstk.transpose(1, 0, 2, 3, 4).reshape(GROUPS * CK, NCHUNK * GLP * CO)
        winstk = winstk.transpose(1, 0, 2, 3, 4).reshape(GROUPS * CK, NCHUNK * GLP * B)
        in_maps.append({
            "wT": np.ascontiguousarray(wstk.astype(npdt)),
            "winT": np.ascontiguousarray(winstk.astype(npdt)),
        })
    return in_maps


def _assemble_v4(results, variant):
    dt, BLK, _ = V4_CFG[variant]
    NBLK = GLP // BLK
    out = np.empty((B, CO, HO, WO), np.float32)
    idx = np.arange(BLK)
    for c in range(N_CORES):
        nreal = RPC if c < N_CORES - 1 else HO - 4 * (N_CORES - 1)
        buf = np.asarray(results[c]["out"])
        b6 = buf.reshape(BLK, B, GROUPS, NBLK, BLK, CO)
        d = b6[idx, :, :, :, idx, :]            # (BLK, B, GROUPS, NBLK, CO)
        dd = d.transpose(1, 4, 2, 3, 0).reshape(B, CO, GROUPS, NBLK * BLK)
        out[:, :, 4 * c:4 * c + nreal, :] = dd[:, :, :nreal, :WO]
    return out


def _build_nc(repeat=1, variant="v2"):
    nc = bacc.Bacc("TRN2", target_bir_lowering=False)
    wT = nc.dram_tensor("wT", [CK, W_COLS], mybir.dt.float32, kind="ExternalInput")
    winT = nc.dram_tensor("winT", [CK, WIN_COLS], mybir.dt.float32, kind="ExternalInput")
    out_cols = OUT_COLS if variant == "v1" else V2_OUT_COLS
    out_rows = CO if variant == "v1" else 128
    out = nc.dram_tensor("out", [out_rows, out_cols], mybir.dt.float32, kind="ExternalOutput")

    gw = GL * NCHUNK * CO    # weight cols per group
    gwin = GL * NCHUNK * B   # window cols per group
    gout = GL * B            # v1 out cols per group

    with tile.TileContext(nc) as tc:
        with (
            tc.tile_pool(name="wp", bufs=3) as wp,
            tc.tile_pool(name="winp", bufs=3) as winp,
            tc.tile_pool(name="pp", bufs=2, space="PSUM") as pp,
            tc.tile_pool(name="op", bufs=2) as op,
        ):
            def body_v1():
                for g in range(GROUPS):
                    wt = wp.tile([CK, gw], mybir.dt.float32, tag="wt", name="wt")
                    nc.sync.dma_start(wt[:], wT.ap()[:, g * gw:(g + 1) * gw])
                    wint = winp.tile([CK, gwin], mybir.dt.float32, tag="wint", name="wint")
                    nc.sync.dma_start(wint[:], winT.ap()[:, g * gwin:(g + 1) * gwin])

                    ps = pp.tile([CO, gout], mybir.dt.float32, tag="ps", name="ps")
                    for l in range(GL):
                        for c in range(NCHUNK):
                            nc.tensor.matmul(
                                ps[:, l * B:(l + 1) * B],
                                lhsT=wt[:, (l * NCHUNK + c) * CO:(l * NCHUNK + c + 1) * CO],
                                rhs=wint[:, (l * NCHUNK + c) * B:(l * NCHUNK + c + 1) * B],
                                start=(c == 0),
                                stop=(c == NCHUNK - 1),
                            )

                    ot = op.tile([CO, gout], mybir.dt.float32, tag="ot", name="ot")
                    nc.vector.tensor_copy(ot[:], ps[:])
                    nc.sync.dma_start(out.ap()[:, g * gout:(g + 1) * gout], ot[:])

            def body_v2():
                # stationary = windows (8 cols, cheap fp32 self-load);
                # moving = weight (N=32); out[b, o] block at partition
                # offset 32*(l%4) via col-tiling -> 4 concurrent MM strips.
                for g in range(GROUPS):
                    wt = wp.tile([CK, gw], mybir.dt.float32, tag="wt", name="wt")
                    nc.sync.dma_start(wt[:], wT.ap()[:, g * gw:(g + 1) * gw])
                    wint = winp.tile([CK, gwin], mybir.dt.float32, tag="wint", name="wint")
                    nc.sync.dma_start(wint[:], winT.ap()[:, g * gwin:(g + 1) * gwin])

                    # one PSUM tile per col strip -> different banks, so the
                    # 4 strips' matmuls aren't serialized by bank tracking
                    pss = [
                        pp.tile([128, V2_GOUT], mybir.dt.float32,
                                tag=f"ps{j}", name=f"ps{j}", bufs=2)
                        for j in range(4)
                    ]
                    for l in range(GL):
                        j = l % 4
                        blk = l // 4
                        for c in range(NCHUNK):
                            nc.tensor.matmul(
                                pss[j][32 * j:32 * j + B, blk * CO:(blk + 1) * CO],
                                lhsT=wint[:, (l * NCHUNK + c) * B:(l * NCHUNK + c + 1) * B],
                                rhs=wt[:, (l * NCHUNK + c) * CO:(l * NCHUNK + c + 1) * CO],
                                start=(c == 0),
                                stop=(c == NCHUNK - 1),
                                tile_position=(0, 32 * j),
                            )

                    ot = op.tile([128, V2_GOUT], mybir.dt.float32, tag="ot", name="ot")
                    for j in range(4):
                        nc.vector.tensor_copy(
                            ot[32 * j:32 * (j + 1), :],
                            pss[j][32 * j:32 * (j + 1), :],
                        )
                    nc.sync.dma_start(out.ap()[:, g * V2_GOUT:(g + 1) * V2_GOUT], ot[:])

            body = body_v1 if variant == "v1" else body_v2
            if repeat == 1:
                body()
            else:
                with tc.For_i(0, repeat, 1):
                    body()
    nc.compile()
    return nc


def _host_prep(x, weight):
    """Build per-core DMA-ready layouts. Pure indexing/transpose, no math."""
    x = np.ascontiguousarray(np.asarray(x, dtype=np.float32))
    weight = np.ascontiguousarray(np.asarray(weight, dtype=np.float32))

    # windows[b, i, p, q, k] with k = kh*3+kw (matches torch unfold flatten)
    wins = np.stack(
        [x[:, :, kh:kh + DH * HO:DH, kw:kw + DW * WO:DW]
         for kh in range(KH) for kw in range(KW)],
        axis=-1,
    )  # (B, CI, HO, WO, 9)

    # (ik, p, q, o) and (ik, p, q, b)
    W2 = weight[0].transpose(1, 4, 2, 3, 0).reshape(IK, HO, WO, CO)
    W3 = wins.transpose(1, 4, 2, 3, 0).reshape(IK, HO, WO, B)

    in_maps = []
    for c in range(N_CORES):
        rows = _ROWS_PADDED[c]
        wsel = W2[:, rows].reshape(IK, L, CO)
        winsel = W3[:, rows].reshape(IK, L, B)
        # [CK, loc, chunk, {o|b}] — partition r of chunk-c col region holds ik=96c+r
        wT = np.stack([wsel[CK * cc:CK * (cc + 1)] for cc in range(NCHUNK)], axis=2)
        winT = np.stack([winsel[CK * cc:CK * (cc + 1)] for cc in range(NCHUNK)], axis=2)
        in_maps.append({
            "wT": np.ascontiguousarray(wT.reshape(CK, W_COLS)),
            "winT": np.ascontiguousarray(winT.reshape(CK, WIN_COLS)),
        })
    return in_maps


def _assemble(results, variant="v2"):
    out = np.empty((B, CO, HO, WO), np.float32)
    qs = np.arange(WO)
    for c in range(N_CORES):
        nreal = RPC if c < N_CORES - 1 else HO - 4 * (N_CORES - 1)
        buf = np.asarray(results[c]["out"])
        if variant == "v1":
            rr = buf.reshape(CO, RPC, WO, B)
            for j in range(nreal):
                out[:, :, 4 * c + j, :] = rr[:, j, :, :].transpose(2, 0, 1)
        else:
            # buf [128, GROUPS*256]: row = 32*(q%4)+b, col = g*256+(q//4)*32+o
            b4 = buf.reshape(4, 32, GROUPS, 8, CO)
            res = b4[qs % 4, :B, :, qs // 4, :]      # (31, b, g, o)
            out[:, :, 4 * c:4 * c + nreal, :] = res.transpose(1, 3, 2, 0)[:, :, :nreal, :]
    return out


VARIANT = os.environ.get("LC2D_VARIANT", "v13")


def timing_setup(x, weight):
    """(in_maps, build_fn) for test.py's slope timing."""
    if VARIANT == "v13":
        return _host_prep_v13(x, weight), (lambda n: _build_nc_v13(n))
    if VARIANT == "v12":
        return _host_prep_v12(x, weight), (lambda n: _build_nc_v12(n))
    if VARIANT == "v11":
        return _host_prep_v11(x, weight), (lambda n: _build_nc_v11(n))
    if VARIANT == "v10":
        return _host_prep_v10(x, weight), (lambda n: _build_nc_v10(n))
    raise NotImplementedError(VARIANT)


def kernel(x, weight, _trace=False, _trace_cores=None):
    if VARIANT == "v13":
        in_maps = _host_prep_v13(x, weight)
    elif VARIANT == "v12":
        in_maps = _host_prep_v12(x, weight)
    elif VARIANT == "v11":
        in_maps = _host_prep_v11(x, weight)
    elif VARIANT == "v10":
        in_maps = _host_prep_v10(x, weight)
    elif VARIANT in ("v8", "v9", "v8h", "v9h"):
        in_maps = _host_prep_v89(
            x, weight, three_term=(VARIANT in ("v9", "v9h")),
            npdt=(np.float16 if VARIANT.endswith("h") else ml_dtypes.bfloat16))
    elif VARIANT in V4_CFG:
        in_maps = _host_prep_v4(x, weight, VARIANT)
    elif VARIANT == "v5":
        in_maps = _host_prep_v5(x, weight)
    else:
        in_maps = _host_prep(x, weight)
    if "nc" not in _NC_CACHE:
        if VARIANT == "v13":
            _NC_CACHE["nc"] = _build_nc_v13(1)
        elif VARIANT == "v12":
            _NC_CACHE["nc"] = _build_nc_v12(1)
        elif VARIANT == "v11":
            _NC_CACHE["nc"] = _build_nc_v11(1)
        elif VARIANT == "v10":
            _NC_CACHE["nc"] = _build_nc_v10(1)
        elif VARIANT in ("v8", "v9", "v8h", "v9h"):
            _NC_CACHE["nc"] = _build_nc_v89(
                1, three_term=(VARIANT in ("v9", "v9h")),
                dt=(mybir.dt.float16 if VARIANT.endswith("h") else mybir.dt.bfloat16))
        elif VARIANT in V4_CFG:
            _NC_CACHE["nc"] = _build_nc_v4(1, VARIANT)
        elif VARIANT == "v5":
            _NC_CACHE["nc"] = _build_nc_v5()
        else:
            _NC_CACHE["nc"] = _build_nc(variant=VARIANT)
    nc = _NC_CACHE["nc"]
    res = run_bass_kernel_spmd(
        nc, in_maps, core_ids=list(range(N_CORES)),
        trace=_trace, trace_cores=_trace_cores,
    )
    if VARIANT == "v13":
        out = _assemble_v13(res.results)
    elif VARIANT == "v12":
        out = _assemble_v12(res.results)
    elif VARIANT == "v11":
        out = _assemble_v11(res.results)
    elif VARIANT in ("v8", "v9", "v8h", "v9h", "v10"):
        out = _assemble_v89(res.results)
    elif VARIANT in V4_CFG:
        out = _assemble_v4(res.results, VARIANT)
    elif VARIANT == "v5":
        out = _assemble_v5(res.results)
    else:
        out = _assemble(res.results, variant=VARIANT)
    if _trace:
        return out, res
    return out


if __name__ == "__main__":
    # quick self-check with random data against a numpy oracle
    rng = np.random.default_rng(0)
    x = rng.standard_normal((B, CI, H, W), dtype=np.float32)
    weight = rng.standard_normal((1, CO, CI, HO, WO, KH * KW), dtype=np.float32)
    wins = np.stack(
        [x[:, :, kh:kh + DH * HO:DH, kw:kw + DW * WO:DW]
         for kh in range(KH) for kw in range(KW)], axis=-1)
    expected = np.einsum("bipqk,oipqk->bopq", wins, weight[0], optimize=True)
    actual = kernel(x, weight)
    err = np.abs(actual - expected).max() / np.abs(expected).max()
    print("max out:", np.abs(expected).max(), "rel err:", err)
    tol = 1e-5 if VARIANT in ("v1", "v2", "v5") else (1e-2 if VARIANT in ("v8", "v4b", "v4b8") else 1e-3)
    assert err < tol, (err, tol)
    print("KERNEL OK")



# revision 36
# speedup vs baseline: 2.6537x; 2.6537x over previous
"""LocallyConnected2d Trainium2 kernel (8-core SPMD).

out[b,o,p,q] = sum_{i,kh,kw} x[b, i, 2p+kh, 2q+kw] * weight[0, o, i, p, q, kh*3+kw]

Shipped variant "v10":
- Shard the H' (=31) output-row dim across 8 cores (4 rows/core; core 7
  gets one duplicated padding row so the SPMD program is uniform). This
  splits the dominant traffic — the 35.4MB per-location weight — 8 ways,
  unlike batch sharding which would replicate it on every core.
- Host-side im2col + layout prep (pure data movement, no math): weight
  and windows are laid out per-core as [96 partitions = (i,k)-chunk,
  free = (group: weight-cols | window-cols)] in fp16, interleaved into a
  SINGLE DRAM tensor so each group needs exactly ONE input DMA (4 total;
  each dma_start costs ~1-2us of serialized ring time here, so DMA count
  matters more than layout).
- Per block of 4 locations: one matmul per contraction chunk,
  lhsT = windows [96, 4*8], rhs = weight [96, 4*32] -> out [32, 128]
  accumulated over the 3 chunks in PSUM; only the 4 diagonal [8, 32]
  tiles are useful (extracted host-side; the 4x moving-side waste is
  free because the kernel is DMA-bound, not PE-bound).
- fp16 (not bf16): same bytes and same PE rate, but 11 mantissa bits
  -> ~3e-4 rel err vs the fp32 reference (bf16 would be ~2.3e-3).
  Variant "v9"/"v9h" (hi+lo split, 3 matmuls) reaches ~4e-6 at ~1.6x
  the time; "v2" is exact fp32 at ~39us.
"""

import os
import numpy as np
import ml_dtypes

import concourse.bacc as bacc
import concourse.mybir as mybir
import concourse.tile as tile
from concourse.bass_utils import run_bass_kernel_spmd

# Problem shapes (hardcoded per contract).
B, CI, H, W = 8, 32, 64, 64
CO = 32
KH = KW = 3
DH = DW = 2
HO = WO = 31
N_CORES = 8
RPC = 4                 # padded H'-rows per core
L = RPC * WO            # 124 locations per core
IK = CI * KH * KW       # 288 contraction
NCHUNK = 3
CK = IK // NCHUNK       # 96 partitions per chunk
GROUPS = RPC            # one compute/DMA group per H'-row
GL = L // GROUPS        # 31 locations per group

W_COLS = L * NCHUNK * CO     # 11904
WIN_COLS = L * NCHUNK * B    # 2976
OUT_COLS = L * B             # 992

_ROWS_PADDED = [[min(4 * c + j, HO - 1) for j in range(RPC)] for c in range(N_CORES)]

_NC_CACHE = {}


V2_GOUT = 256               # psum cols per group in v2: 8 col-blocks x 32 (o)
V2_OUT_COLS = V2_GOUT * GROUPS

# v4: blocked matmuls — BLK locations share one matmul (out is a BLK x BLK
# grid of [b, o] tiles; only the diagonal is useful, extracted host-side).
# fp32r needs moving free dim >= 256 for the 1 cycle/row fast path.
GLP = 32                    # padded locs per group (31 real + 1 dup)
V4_CFG = {
    "v4r": (mybir.dt.float32r, 8, np.float32),
    "v4b": (mybir.dt.bfloat16, 4, ml_dtypes.bfloat16),
    "v4b8": (mybir.dt.bfloat16, 8, ml_dtypes.bfloat16),
}


def _build_nc_v4(repeat, variant):
    dt, BLK, _ = V4_CFG[variant]
    NBLK = GLP // BLK
    gw = NCHUNK * GLP * CO   # 3072 weight cols per group
    gwin = NCHUNK * GLP * B  # 768 win cols per group
    bout = BLK * CO          # out cols per block
    orows = B * BLK          # out rows per block
    out_cols = GROUPS * NBLK * bout

    nc = bacc.Bacc("TRN2", target_bir_lowering=False)
    wT = nc.dram_tensor("wT", [GROUPS * CK, gw], dt, kind="ExternalInput")
    winT = nc.dram_tensor("winT", [GROUPS * CK, gwin], dt, kind="ExternalInput")
    out = nc.dram_tensor("out", [orows, out_cols], mybir.dt.float32, kind="ExternalOutput")

    with tile.TileContext(nc) as tc:
        with (
            tc.tile_pool(name="wp", bufs=3) as wp,
            tc.tile_pool(name="winp", bufs=3) as winp,
            tc.tile_pool(name="pp", bufs=4, space="PSUM") as pp,
            tc.tile_pool(name="op", bufs=4) as op,
        ):
            def body():
                for g in range(GROUPS):
                    wt = wp.tile([CK, gw], dt, tag="wt", name="wt")
                    nc.sync.dma_start(wt[:], wT.ap()[g * CK:(g + 1) * CK, :])
                    wint = winp.tile([CK, gwin], dt, tag="wint", name="wint")
                    nc.sync.dma_start(wint[:], winT.ap()[g * CK:(g + 1) * CK, :])

                    for bl in range(NBLK):
                        ps = pp.tile([orows, bout], mybir.dt.float32, tag="ps", name="ps")
                        for c in range(NCHUNK):
                            nc.tensor.matmul(
                                ps[:],
                                lhsT=wint[:, c * (GLP * B) + bl * (BLK * B):
                                          c * (GLP * B) + (bl + 1) * (BLK * B)],
                                rhs=wt[:, c * (GLP * CO) + bl * bout:
                                       c * (GLP * CO) + (bl + 1) * bout],
                                start=(c == 0),
                                stop=(c == NCHUNK - 1),
                            )
                        ot = op.tile([orows, bout], mybir.dt.float32, tag="ot", name="ot")
                        nc.vector.tensor_copy(ot[:], ps[:])
                        nc.sync.dma_start(
                            out.ap()[:, (g * NBLK + bl) * bout:(g * NBLK + bl + 1) * bout],
                            ot[:],
                        )

            if repeat == 1:
                body()
            else:
                with tc.For_i(0, repeat, 1):
                    body()
    nc.compile()
    return nc


def _build_nc_v5(repeat=1):
    """fp32 exact; all DMAs 128-partition; contraction 128+128+32 with the
    32-row remainder of all 4 groups packed into one 128-row tile."""
    gw = GL * CO     # 992 weight cols per (group, chunk)
    gwin = GL * B    # 248 win cols per (group, chunk)
    nc = bacc.Bacc("TRN2", target_bir_lowering=False)
    w01 = nc.dram_tensor("w01", [GROUPS * 2 * 128, gw], mybir.dt.float32, kind="ExternalInput")
    win01 = nc.dram_tensor("win01", [GROUPS * 2 * 128, gwin], mybir.dt.float32, kind="ExternalInput")
    w2 = nc.dram_tensor("w2", [GROUPS * 32, gw], mybir.dt.float32, kind="ExternalInput")
    win2 = nc.dram_tensor("win2", [GROUPS * 32, gwin], mybir.dt.float32, kind="ExternalInput")
    out = nc.dram_tensor("out", [GROUPS * 128, V2_GOUT], mybir.dt.float32, kind="ExternalOutput")

    with tile.TileContext(nc) as tc:
        with (
            tc.tile_pool(name="wp", bufs=3) as wp,
            tc.tile_pool(name="winp", bufs=3) as winp,
            tc.tile_pool(name="pp", bufs=2, space="PSUM") as pp,
            tc.tile_pool(name="op", bufs=2) as op,
        ):
            def body():
                for g in range(GROUPS):
                    wts, wints = [], []
                    for cc in range(2):
                        wt = wp.tile([128, gw], mybir.dt.float32, tag=f"wt{cc}", name=f"wt{cc}")
                        nc.sync.dma_start(
                            wt[:], w01.ap()[(g * 2 + cc) * 128:(g * 2 + cc + 1) * 128, :])
                        wint = winp.tile([128, gwin], mybir.dt.float32, tag=f"wint{cc}", name=f"wint{cc}")
                        nc.sync.dma_start(
                            wint[:], win01.ap()[(g * 2 + cc) * 128:(g * 2 + cc + 1) * 128, :])
                        wts.append(wt)
                        wints.append(wint)
                    w2t = wp.tile([32, gw], mybir.dt.float32, tag="w2t", name="w2t")
                    nc.sync.dma_start(w2t[:], w2.ap()[g * 32:(g + 1) * 32, :])
                    win2t = winp.tile([32, gwin], mybir.dt.float32, tag="win2t", name="win2t")
                    nc.sync.dma_start(win2t[:], win2.ap()[g * 32:(g + 1) * 32, :])

                    pss = [
                        pp.tile([128, V2_GOUT], mybir.dt.float32,
                                tag=f"ps{j}", name=f"ps{j}", bufs=2)
                        for j in range(4)
                    ]
                    for l in range(GL):
                        j = l % 4
                        blk = l // 4
                        dst = pss[j][32 * j:32 * j + B, blk * CO:(blk + 1) * CO]
                        for cc in range(2):
                            nc.tensor.matmul(
                                dst,
                                lhsT=wints[cc][:, l * B:(l + 1) * B],
                                rhs=wts[cc][:, l * CO:(l + 1) * CO],
                                start=(cc == 0),
                                stop=False,
                                tile_position=(0, 32 * j),
                            )
                        nc.tensor.matmul(
                            dst,
                            lhsT=win2t[:, l * B:(l + 1) * B],
                            rhs=w2t[:, l * CO:(l + 1) * CO],
                            start=False,
                            stop=True,
                            tile_position=(0, 32 * j),
                        )

                    ot = op.tile([128, V2_GOUT], mybir.dt.float32, tag="ot", name="ot")
                    for j in range(4):
                        nc.vector.tensor_copy(
                            ot[32 * j:32 * (j + 1), :],
                            pss[j][32 * j:32 * (j + 1), :],
                        )
                    nc.sync.dma_start(out.ap()[g * 128:(g + 1) * 128, :], ot[:])

            if repeat == 1:
                body()
            else:
                with tc.For_i(0, repeat, 1):
                    body()
    nc.compile()
    return nc


def _host_prep_v5(x, weight):
    x = np.ascontiguousarray(np.asarray(x, dtype=np.float32))
    weight = np.ascontiguousarray(np.asarray(weight, dtype=np.float32))
    wins = np.stack(
        [x[:, :, kh:kh + DH * HO:DH, kw:kw + DW * WO:DW]
         for kh in range(KH) for kw in range(KW)],
        axis=-1,
    )
    W2 = weight[0].transpose(1, 4, 2, 3, 0).reshape(IK, HO, WO, CO)
    W3 = wins.transpose(1, 4, 2, 3, 0).reshape(IK, HO, WO, B)
    in_maps = []
    for c in range(N_CORES):
        rows = _ROWS_PADDED[c]
        wsel = W2[:, rows]       # (288, 4, 31, CO)
        winsel = W3[:, rows]     # (288, 4, 31, B)
        # w01 rows: (g, c01, 128) ; cols (l, o)
        w01 = wsel[:256].reshape(2, 128, GROUPS, GL * CO).transpose(2, 0, 1, 3)
        win01 = winsel[:256].reshape(2, 128, GROUPS, GL * B).transpose(2, 0, 1, 3)
        w2 = wsel[256:].reshape(32, GROUPS, GL * CO).transpose(1, 0, 2)
        win2 = winsel[256:].reshape(32, GROUPS, GL * B).transpose(1, 0, 2)
        in_maps.append({
            "w01": np.ascontiguousarray(w01.reshape(GROUPS * 2 * 128, GL * CO)),
            "win01": np.ascontiguousarray(win01.reshape(GROUPS * 2 * 128, GL * B)),
            "w2": np.ascontiguousarray(w2.reshape(GROUPS * 32, GL * CO)),
            "win2": np.ascontiguousarray(win2.reshape(GROUPS * 32, GL * B)),
        })
    return in_maps


def _assemble_v5(results):
    out = np.empty((B, CO, HO, WO), np.float32)
    qs = np.arange(WO)
    for c in range(N_CORES):
        nreal = RPC if c < N_CORES - 1 else HO - 4 * (N_CORES - 1)
        buf = np.asarray(results[c]["out"])      # [GROUPS*128, 256]
        b5 = buf.reshape(GROUPS, 4, 32, 8, CO)   # (g, strip, 32row, blk, o)
        res = b5[:, qs % 4, :B, qs // 4, :]      # (g?, ...) advanced idx
        # advanced indices qs%4 (dim1) and qs//4 (dim3) -> (31, GROUPS, B, CO)
        out[:, :, 4 * c:4 * c + nreal, :] = res.transpose(2, 3, 1, 0)[:, :, :nreal, :]
    return out


V89_BLK = 4
V89_NBLK = GLP // V89_BLK            # 8 blocks of 4 locs per group
V89_GW = NCHUNK * GLP * CO           # 3072 weight cols per group
V89_GWIN = NCHUNK * GLP * B          # 768 win cols per group
V89_BOUT = V89_BLK * CO              # 128 out cols per block
V89_OROWS = B * V89_BLK              # 32 out rows
V89_OUTC = GROUPS * V89_NBLK * V89_BOUT  # 4096


def _build_nc_v89(repeat=1, three_term=False, dt=None):
    """16-bit blocked kernel, minimal DMA count, split across both HWDGE
    rings. three_term=True computes w≈wh+wl, win≈vh+vl and accumulates
    vh·wh + vh·wl + vl·wh (16-bit products are exact in fp32 -> ~1e-5 rel err).
    """
    if dt is None:
        dt = mybir.dt.bfloat16
    W = GROUPS * V89_GW
    WIN = GROUPS * V89_GWIN
    nc = bacc.Bacc("TRN2", target_bir_lowering=False)
    wh_d = nc.dram_tensor("wh", [CK, W], dt, kind="ExternalInput")
    winh_d = nc.dram_tensor("winh", [CK, WIN], dt, kind="ExternalInput")
    if three_term:
        wl_d = nc.dram_tensor("wl", [CK, W], dt, kind="ExternalInput")
        winl_d = nc.dram_tensor("winl", [CK, WIN], dt, kind="ExternalInput")
    out = nc.dram_tensor("out", [V89_OROWS, V89_OUTC], mybir.dt.float32, kind="ExternalOutput")

    half = W // 2  # 2 groups per ring half
    with tile.TileContext(nc) as tc:
        with (
            tc.tile_pool(name="wp", bufs=2) as wp,
            tc.tile_pool(name="winp", bufs=2) as winp,
            tc.tile_pool(name="pp", bufs=4, space="PSUM") as pp,
            tc.tile_pool(name="op", bufs=2) as op,
        ):
            def body():
                # weight: groups 0-1 via SP ring, groups 2-3 via ACT ring,
                # one piece per group -> compute starts after 1/4 of bytes
                wh = wp.tile([CK, W], dt, tag="wh", name="wh")
                for g in range(2):
                    nc.sync.dma_start(
                        wh[:, g * V89_GW:(g + 1) * V89_GW],
                        wh_d.ap()[:, g * V89_GW:(g + 1) * V89_GW])
                for g in range(2, 4):
                    nc.scalar.dma_start(
                        wh[:, g * V89_GW:(g + 1) * V89_GW],
                        wh_d.ap()[:, g * V89_GW:(g + 1) * V89_GW])
                winh = winp.tile([CK, WIN], dt, tag="winh", name="winh")
                nc.sync.dma_start(winh[:, :WIN // 2], winh_d.ap()[:, :WIN // 2])
                nc.scalar.dma_start(winh[:, WIN // 2:], winh_d.ap()[:, WIN // 2:])
                if three_term:
                    wl = wp.tile([CK, W], dt, tag="wl", name="wl")
                    for g in range(2):
                        nc.scalar.dma_start(
                            wl[:, g * V89_GW:(g + 1) * V89_GW],
                            wl_d.ap()[:, g * V89_GW:(g + 1) * V89_GW])
                    for g in range(2, 4):
                        nc.sync.dma_start(
                            wl[:, g * V89_GW:(g + 1) * V89_GW],
                            wl_d.ap()[:, g * V89_GW:(g + 1) * V89_GW])
                    winl = winp.tile([CK, WIN], dt, tag="winl", name="winl")
                    nc.scalar.dma_start(winl[:, :WIN // 2], winl_d.ap()[:, :WIN // 2])
                    nc.sync.dma_start(winl[:, WIN // 2:], winl_d.ap()[:, WIN // 2:])

                ot = op.tile([V89_OROWS, V89_OUTC], mybir.dt.float32, tag="ot", name="ot")
                for g in range(GROUPS):
                    for bl in range(V89_NBLK):
                        ps = pp.tile([V89_OROWS, V89_BOUT], mybir.dt.float32, tag="ps", name="ps")
                        first = True
                        for c in range(NCHUNK):
                            lo = g * V89_GWIN + c * (GLP * B) + bl * (V89_BLK * B)
                            ro = g * V89_GW + c * (GLP * CO) + bl * V89_BOUT
                            lhs_h = winh[:, lo:lo + V89_BLK * B]
                            rhs_h = wh[:, ro:ro + V89_BOUT]
                            terms = [(lhs_h, rhs_h)]
                            if three_term:
                                terms.append((lhs_h, wl[:, ro:ro + V89_BOUT]))
                                terms.append((winl[:, lo:lo + V89_BLK * B], rhs_h))
                            for ti, (lh, rh) in enumerate(terms):
                                last = (c == NCHUNK - 1) and (ti == len(terms) - 1)
                                nc.tensor.matmul(
                                    ps[:], lhsT=lh, rhs=rh,
                                    start=first, stop=last)
                                first = False
                        nc.vector.tensor_copy(
                            ot[:, (g * V89_NBLK + bl) * V89_BOUT:(g * V89_NBLK + bl + 1) * V89_BOUT],
                            ps[:])
                nc.gpsimd.dma_start(out.ap()[:, :], ot[:])

            if repeat == 1:
                body()
            else:
                with tc.For_i(0, repeat, 1):
                    body()
    nc.compile()
    return nc


def _host_prep_v89(x, weight, three_term=False, npdt=None):
    if npdt is None:
        npdt = ml_dtypes.bfloat16
    x = np.ascontiguousarray(np.asarray(x, dtype=np.float32))
    weight = np.ascontiguousarray(np.asarray(weight, dtype=np.float32))
    wins = np.stack(
        [x[:, :, kh:kh + DH * HO:DH, kw:kw + DW * WO:DW]
         for kh in range(KH) for kw in range(KW)],
        axis=-1,
    )
    W2 = weight[0].transpose(1, 4, 2, 3, 0).reshape(IK, HO, WO, CO)
    W3 = wins.transpose(1, 4, 2, 3, 0).reshape(IK, HO, WO, B)
    qpad = list(range(WO)) + [WO - 1]
    in_maps = []
    for c in range(N_CORES):
        rows = _ROWS_PADDED[c]
        wsel = W2[:, rows][:, :, qpad, :]       # (288, 4, 32, CO)
        winsel = W3[:, rows][:, :, qpad, :]     # (288, 4, 32, B)
        # -> [CK, (group, chunk, locp, {o|b})]
        wstk = np.stack([wsel[CK * cc:CK * (cc + 1)] for cc in range(NCHUNK)], axis=2)
        winstk = np.stack([winsel[CK * cc:CK * (cc + 1)] for cc in range(NCHUNK)], axis=2)
        # (CK, 4, chunk, 32, X) -> (CK, group*chunk*locp*X)
        wfull = wstk.reshape(CK, GROUPS * NCHUNK * GLP * CO)
        winfull = winstk.reshape(CK, GROUPS * NCHUNK * GLP * B)
        m = {}
        wh = wfull.astype(npdt)
        vh = winfull.astype(npdt)
        m["wh"] = np.ascontiguousarray(wh)
        m["winh"] = np.ascontiguousarray(vh)
        if three_term:
            m["wl"] = np.ascontiguousarray(
                (wfull - wh.astype(np.float32)).astype(npdt))
            m["winl"] = np.ascontiguousarray(
                (winfull - vh.astype(np.float32)).astype(npdt))
        in_maps.append(m)
    return in_maps


def _assemble_v89(results):
    BLK = V89_BLK
    NBLK = V89_NBLK
    out = np.empty((B, CO, HO, WO), np.float32)
    idx = np.arange(BLK)
    for c in range(N_CORES):
        nreal = RPC if c < N_CORES - 1 else HO - 4 * (N_CORES - 1)
        buf = np.asarray(results[c]["out"])          # [32, 4096]
        b6 = buf.reshape(BLK, B, GROUPS, NBLK, BLK, CO)
        d = b6[idx, :, :, :, idx, :]                 # (BLK, B, G, NBLK, CO)
        dd = d.transpose(1, 4, 2, 3, 0).reshape(B, CO, GROUPS, NBLK * BLK)
        out[:, :, 4 * c:4 * c + nreal, :] = dd[:, :, :nreal, :WO]
    return out


V11_NP = 8                      # pieces (half H'-rows) per core
V11_PL = 16                     # padded locations per piece
V11_NBLK = 4                    # blocks of BLK=4 locs per piece
V11_GW = NCHUNK * V11_PL * CO   # 1536 weight cols per piece
V11_GWIN = NCHUNK * V11_PL * B  # 384 win cols per piece
V11_GTOT = V11_GW + V11_GWIN    # 1920
V11_POUT = V11_NBLK * V89_BLK * CO  # 512 out cols per piece
V11_OUTC = V11_NP * V11_POUT    # 4096


def _build_nc_v11(repeat=1, dt=None):
    """8 self-contained pieces (16 locs each), one input DMA per piece on the
    SP queue; matmuls accumulate into a [32, 512] PSUM tile per piece (one
    bank); output DMA'd straight from PSUM on the Pool/SWDGE queue — no
    PSUM->SBUF copies at all."""
    if dt is None:
        dt = mybir.dt.float16
    BLK = V89_BLK
    orows = V89_OROWS           # 32 = BLK * B
    nc = bacc.Bacc("TRN2", target_bir_lowering=False)
    wx = nc.dram_tensor("wx", [CK, V11_NP * V11_GTOT], dt, kind="ExternalInput")
    out = nc.dram_tensor("out", [orows, V11_OUTC], mybir.dt.float32, kind="ExternalOutput")
    with tile.TileContext(nc) as tc:
        with (
            tc.tile_pool(name="wp", bufs=2) as wp,
            tc.tile_pool(name="pp", bufs=4, space="PSUM") as pp,
        ):
            def body():
                t = wp.tile([CK, V11_NP * V11_GTOT], dt, tag="t", name="t")
                for p in range(V11_NP):
                    nc.sync.dma_start(t[:, p * V11_GTOT:(p + 1) * V11_GTOT],
                                      wx.ap()[:, p * V11_GTOT:(p + 1) * V11_GTOT])
                for p in range(V11_NP):
                    base = p * V11_GTOT
                    ps = pp.tile([orows, V11_POUT], mybir.dt.float32, tag="ps", name="ps")
                    for bl in range(V11_NBLK):
                        for c in range(NCHUNK):
                            lo = base + V11_GW + c * (V11_PL * B) + bl * (BLK * B)
                            ro = base + c * (V11_PL * CO) + bl * (BLK * CO)
                            nc.tensor.matmul(
                                ps[:, bl * (BLK * CO):(bl + 1) * (BLK * CO)],
                                lhsT=t[:, lo:lo + BLK * B],
                                rhs=t[:, ro:ro + BLK * CO],
                                start=(c == 0), stop=(c == NCHUNK - 1))
                    nc.gpsimd.dma_start(
                        out.ap()[:, p * V11_POUT:(p + 1) * V11_POUT], ps[:])
            if repeat == 1:
                body()
            else:
                with tc.For_i(0, repeat, 1):
                    body()
    nc.compile()
    return nc


def _host_prep_v11(x, weight, npdt=None):
    if npdt is None:
        npdt = np.float16
    m = _host_prep_v89(x, weight, three_term=False, npdt=npdt)
    out_maps = []
    for mm in m:
        # wh: (CK, group4, chunk3, locp32, CO) ; winh: (..., B)
        wh = mm["wh"].reshape(CK, GROUPS, NCHUNK, GLP, CO)
        vh = mm["winh"].reshape(CK, GROUPS, NCHUNK, GLP, B)
        # -> (CK, row4, half2, chunk3, loc16, X)
        wp = wh.reshape(CK, GROUPS, NCHUNK, 2, V11_PL, CO).transpose(0, 1, 3, 2, 4, 5)
        vp = vh.reshape(CK, GROUPS, NCHUNK, 2, V11_PL, B).transpose(0, 1, 3, 2, 4, 5)
        wp = wp.reshape(CK, V11_NP, V11_GW)
        vp = vp.reshape(CK, V11_NP, V11_GWIN)
        wx = np.concatenate([wp, vp], axis=2).reshape(CK, V11_NP * V11_GTOT)
        out_maps.append({"wx": np.ascontiguousarray(wx)})
    return out_maps


def _assemble_v11(results):
    BLK = V89_BLK
    out = np.empty((B, CO, HO, WO), np.float32)
    idx = np.arange(BLK)
    for c in range(N_CORES):
        nreal = RPC if c < N_CORES - 1 else HO - 4 * (N_CORES - 1)
        buf = np.asarray(results[c]["out"])          # [32, 4096]
        b6 = buf.reshape(BLK, B, V11_NP, V11_NBLK, BLK, CO)
        d = b6[idx, :, :, :, idx, :]                 # (BLK, B, P, NBLK, CO)
        dd = d.transpose(1, 4, 2, 3, 0).reshape(B, CO, GROUPS, 2 * V11_NBLK * BLK)
        out[:, :, 4 * c:4 * c + nreal, :] = dd[:, :, :nreal, :WO]
    return out


V12_NBLKS = 32                  # blocks of BLK=4 locs per core (4 rows x 8)
V12_UNIT = V89_BLK * (B + CO)   # 160 cols per (block, chunk) unit: win | w
V12_BCOLS = NCHUNK * V12_UNIT   # 480 cols per block
V12_PIECES = (5, 5, 5, 5, 4, 4, 4)  # input DMA piece sizes in blocks (sum 32)
V12_OUTC = V12_NBLKS * V89_BLK * CO  # 4096 fp16 out cols (blocked, diag on host)
V12_NGRP = 8                         # compute groups of 4 blocks (1 PSUM bank)


def _build_nc_v12(repeat=1, dt=None, flat=False):
    """Block-major stream: 9 input pieces on SP (tiny last piece to shrink
    the tail), matmuls accumulate in per-piece PSUM banks (all 8), one
    contiguous fp32->fp16 PSUM->SBUF copy per piece round-robin over
    DVE/ACT/Pool, 2 output DMAs; diagonal extraction happens host-side.
    repeat>1 timing builds unroll 8 bodies per For_i iteration so
    consecutive bodies pipeline (the all-engine barrier is per-For_i-iter)."""
    if dt is None:
        dt = mybir.dt.float16
    BLK = V89_BLK
    nc = bacc.Bacc("TRN2", target_bir_lowering=False)
    wx = nc.dram_tensor("wx", [CK, V12_NBLKS * V12_BCOLS], dt, kind="ExternalInput")
    out = nc.dram_tensor("out", [32, V12_OUTC], mybir.dt.float16, kind="ExternalOutput")
    starts = []
    s = 0
    for n in V12_PIECES:
        starts.append(s)
        s += n
    assert s == V12_NBLKS
    BC = BLK * CO
    with tile.TileContext(nc) as tc:
        with (
            tc.tile_pool(name="wp", bufs=2) as wp,
            tc.tile_pool(name="pp", bufs=7, space="PSUM") as pp,
            tc.tile_pool(name="op", bufs=2) as op,
        ):
            def body():
                t = wp.tile([CK, V12_NBLKS * V12_BCOLS], dt, tag="t", name="t")
                for s, n in zip(starts, V12_PIECES):
                    nc.sync.dma_start(
                        t[:, s * V12_BCOLS:(s + n) * V12_BCOLS],
                        wx.ap()[:, s * V12_BCOLS:(s + n) * V12_BCOLS])
                # PE p-state warmup: ~3.4us of dummy matmuls on a memset tile
                # so real matmuls run at full clock once piece 0 lands.
                wu = wp.tile([CK, 256], dt, tag="wu", name="wu")
                nc.vector.memset(wu[:], 0.0)
                psw = pp.tile([32, 512], mybir.dt.float32, tag="psw", name="psw",
                              bufs=1)
                for i in range(12):
                    nc.tensor.matmul(psw[0:8, 0:256], lhsT=wu[:, :8], rhs=wu[:, :256],
                                     start=(i == 0), stop=(i == 11))
                for i in range(10):
                    nc.tensor.matmul(psw[0:8, 256:320], lhsT=wu[:, :8], rhs=wu[:, :64],
                                     start=(i == 0), stop=(i == 9))
                ot = op.tile([32, V12_OUTC], mybir.dt.float16, tag="ot", name="ot")
                for g in range(V12_NGRP):
                    ps = pp.tile([32, 4 * BC], mybir.dt.float32,
                                 tag="ps", name="ps")
                    for li in range(4):
                        u0 = (4 * g + li) * NCHUNK
                        for c in range(NCHUNK):
                            off = (u0 + c) * V12_UNIT
                            nc.tensor.matmul(
                                ps[:, li * BC:(li + 1) * BC],
                                lhsT=t[:, off:off + BLK * B],
                                rhs=t[:, off + BLK * B:off + V12_UNIT],
                                start=(c == 0), stop=(c == NCHUNK - 1))
                    dst = ot[:, g * 4 * BC:(g + 1) * 4 * BC]
                    if g % 2 == 0:
                        nc.vector.tensor_copy(dst, ps[:])
                    else:
                        nc.scalar.copy(dst, ps[:])
                nc.scalar.dma_start(out.ap()[:, :], ot[:, :])
            if repeat == 1:
                body()
            elif flat:
                for _ in range(repeat):
                    body()
            else:
                u = 8 if repeat % 8 == 0 else (4 if repeat % 4 == 0 else 1)
                with tc.For_i(0, repeat // u, 1):
                    for _ in range(u):
                        body()
    nc.compile()
    return nc


def _host_prep_v12(x, weight, npdt=None):
    if npdt is None:
        npdt = np.float16
    m = _host_prep_v89(x, weight, three_term=False, npdt=npdt)
    out_maps = []
    for mm in m:
        # wh: (CK, group4, chunk3, locp32, CO); winh: (..., B)
        wh = mm["wh"].reshape(CK, GROUPS, NCHUNK, GLP, CO)
        vh = mm["winh"].reshape(CK, GROUPS, NCHUNK, GLP, B)
        # -> (CK, block(row,bb)=32, chunk3, BLK locs, X) with loc = blk*4+j
        wb = wh.reshape(CK, GROUPS, NCHUNK, 8, V89_BLK, CO)
        vb = vh.reshape(CK, GROUPS, NCHUNK, 8, V89_BLK, B)
        wb = wb.transpose(0, 1, 3, 2, 4, 5).reshape(CK, V12_NBLKS, NCHUNK, V89_BLK * CO)
        vb = vb.transpose(0, 1, 3, 2, 4, 5).reshape(CK, V12_NBLKS, NCHUNK, V89_BLK * B)
        wx = np.concatenate([vb, wb], axis=3)  # (CK, blk, chunk, 32+128)
        out_maps.append({"wx": np.ascontiguousarray(
            wx.reshape(CK, V12_NBLKS * V12_BCOLS))})
    return out_maps


def _assemble_v12(results):
    BLK = V89_BLK
    out = np.empty((B, CO, HO, WO), np.float32)
    qs = np.arange(WO)
    idx = np.arange(BLK)
    for c in range(N_CORES):
        nreal = RPC if c < N_CORES - 1 else HO - 4 * (N_CORES - 1)
        buf = np.asarray(results[c]["out"]).astype(np.float32)  # [32, 4096]
        b6 = buf.reshape(BLK, B, GROUPS, 8, BLK, CO)  # (j, b, row, bb, j', o)
        d = b6[idx, :, :, :, idx, :]                  # (BLK, B, row, bb, o)
        res = d[qs % 4, :, :, qs // 4, :]             # (31, b, row, o)
        out[:, :, 4 * c:4 * c + nreal, :] = res.transpose(1, 3, 2, 0)[:, :, :nreal, :]
    return out


V13_UNIT = V89_BLK * (B + CO)       # 160 cols per (block, chunk) unit
V13_CSEC = 4 * 32 + 128             # C section: 4 zero-padded win blocks + packed w band
V13_GRP = 4 * 2 * V13_UNIT + V13_CSEC  # 1536 cols per 4-block group
V13_NGRP = 8
V13_OUTC = 1024                     # out [128, 1024] fp16


def _build_nc_v13(repeat=1, dt=None, flat=False, dma_only=False):
    """128-partition DMA layout: contraction 288 = A(0:128) + B(128:256) on all
    partitions + C(256:288) packed 4-blocks-per-band; the C *window* operand is
    zero-padded to 128 rows so every matmul uses the same (128,32) PE tile
    config (alternating PE tile sizes measured ~2x slower on HW), and the
    zeros null out the other blocks' rows of the packed C weight band.
    Matmul outputs land in rotating PSUM col bands (32*(g%4)); one [128,512]
    copy per 4 groups (DVE then ACT); out [128,1024] fp16. 8 DMAs/body so the
    8 rotating DMA-completion semaphores stay body-aligned; repeat>1 unrolls
    8 bodies per For_i iteration."""
    if dt is None:
        dt = mybir.dt.float16
    BLK = V89_BLK
    nc = bacc.Bacc("TRN2", target_bir_lowering=False)
    wx = nc.dram_tensor("wx", [128, V13_NGRP * V13_GRP], dt, kind="ExternalInput")
    out = nc.dram_tensor("out", [128, V13_OUTC], mybir.dt.float16, kind="ExternalOutput")
    with tile.TileContext(nc) as tc:
        with (
            tc.tile_pool(name="wp", bufs=2) as wp,
            tc.tile_pool(name="pp", bufs=2, space="PSUM") as pp,
            tc.tile_pool(name="op", bufs=2) as op,
        ):
            def body():
                t = wp.tile([128, V13_NGRP * V13_GRP], dt, tag="t", name="t")
                # 7 input DMAs: piece 0 covers groups 0-1, rest one group each
                bounds = [0, 2, 3, 4, 5, 6, 7, 8]
                for i in range(7):
                    lo, hi = bounds[i] * V13_GRP, bounds[i + 1] * V13_GRP
                    nc.sync.dma_start(t[:, lo:hi], wx.ap()[:, lo:hi])
                if dma_only:
                    return
                wu = wp.tile([128, 256], dt, tag="wu", name="wu")
                nc.vector.memset(wu[:], 0.0)
                psw = pp.tile([32, 512], mybir.dt.float32, tag="psw", name="psw",
                              bufs=1)
                for i in range(12):
                    nc.tensor.matmul(psw[0:8, 0:256], lhsT=wu[:, :8], rhs=wu[:, :256],
                                     start=(i == 0), stop=(i == 11))
                for i in range(10):
                    nc.tensor.matmul(psw[0:8, 256:320], lhsT=wu[:, :8], rhs=wu[:, :64],
                                     start=(i == 0), stop=(i == 9))
                ot = op.tile([128, V13_OUTC], mybir.dt.float16, tag="ot", name="ot")
                for h in range(2):
                    psf = pp.tile([128, 512], mybir.dt.float32, tag="ps", name="ps")
                    for bi in range(4):
                        g = 4 * h + bi
                        base = g * V13_GRP
                        for li in range(4):
                            ab = base + li * (2 * V13_UNIT)
                            cs = base + 8 * V13_UNIT
                            dst = psf[32 * bi:32 * (bi + 1), li * 128:(li + 1) * 128]
                            nc.tensor.matmul(
                                dst, lhsT=t[:, ab:ab + 32],
                                rhs=t[:, ab + 32:ab + V13_UNIT],
                                start=True, stop=False,
                                tile_position=(0, 32 * bi))
                            nc.tensor.matmul(
                                dst, lhsT=t[:, ab + V13_UNIT:ab + V13_UNIT + 32],
                                rhs=t[:, ab + V13_UNIT + 32:ab + 2 * V13_UNIT],
                                start=False, stop=False,
                                tile_position=(0, 32 * bi))
                            nc.tensor.matmul(
                                dst,
                                lhsT=t[:, cs + 32 * li:cs + 32 * (li + 1)],
                                rhs=t[:, cs + 128:cs + 256],
                                start=False, stop=True,
                                tile_position=(0, 32 * bi))
                    dst = ot[:, h * 512:(h + 1) * 512]
                    if h == 0:
                        nc.vector.tensor_copy(dst, psf[:])
                    else:
                        nc.scalar.copy(dst, psf[:])
                nc.scalar.dma_start(out.ap()[:, :], ot[:, :])
            if repeat == 1:
                body()
            elif flat:
                for _ in range(repeat):
                    body()
            else:
                u = 8 if repeat % 8 == 0 else (4 if repeat % 4 == 0 else 1)
                with tc.For_i(0, repeat // u, 1):
                    for _ in range(u):
                        body()
    nc.compile()
    return nc


def _host_prep_v13(x, weight, npdt=None):
    if npdt is None:
        npdt = np.float16
    x = np.ascontiguousarray(np.asarray(x, dtype=np.float32))
    weight = np.ascontiguousarray(np.asarray(weight, dtype=np.float32))
    wins = np.stack(
        [x[:, :, kh:kh + DH * HO:DH, kw:kw + DW * WO:DW]
         for kh in range(KH) for kw in range(KW)],
        axis=-1,
    )
    W2 = weight[0].transpose(1, 4, 2, 3, 0).reshape(IK, HO, WO, CO)
    W3 = wins.transpose(1, 4, 2, 3, 0).reshape(IK, HO, WO, B)
    qpad = list(range(WO)) + [WO - 1]
    in_maps = []
    for c in range(N_CORES):
        rows = _ROWS_PADDED[c]
        wsel = W2[:, rows][:, :, qpad, :].astype(npdt)    # (288, 4, 32, CO)
        winsel = W3[:, rows][:, :, qpad, :].astype(npdt)  # (288, 4, 32, B)
        wx = np.zeros((128, V13_NGRP * V13_GRP), npdt)
        for g in range(V13_NGRP):
            row, half = g // 2, g % 2
            base = g * V13_GRP
            for li in range(4):
                bb = 4 * half + li
                ls = slice(bb * 4, (bb + 1) * 4)     # 4 locs of this block
                ab = base + li * (2 * V13_UNIT)
                # A unit: ik 0:128
                wx[:, ab:ab + 32] = winsel[0:128, row, ls, :].reshape(128, 32)
                wx[:, ab + 32:ab + V13_UNIT] = wsel[0:128, row, ls, :].reshape(128, 128)
                # B unit: ik 128:256
                wx[:, ab + V13_UNIT:ab + V13_UNIT + 32] = \
                    winsel[128:256, row, ls, :].reshape(128, 32)
                wx[:, ab + V13_UNIT + 32:ab + 2 * V13_UNIT] = \
                    wsel[128:256, row, ls, :].reshape(128, 128)
                # C section: ik 256:288. win zero-padded to 128 rows (band
                # 32*li holds block li's win); w packed 4-blocks-per-band.
                cs = base + 8 * V13_UNIT
                wx[32 * li:32 * (li + 1), cs + 32 * li:cs + 32 * (li + 1)] = \
                    winsel[256:288, row, ls, :].reshape(32, 32)
                wx[32 * li:32 * (li + 1), cs + 128:cs + 256] = \
                    wsel[256:288, row, ls, :].reshape(32, 128)
        in_maps.append({"wx": np.ascontiguousarray(wx)})
    return in_maps


def _assemble_v13(results):
    BLK = V89_BLK
    out = np.empty((B, CO, HO, WO), np.float32)
    idx = np.arange(BLK)
    for c in range(N_CORES):
        nreal = RPC if c < N_CORES - 1 else HO - 4 * (N_CORES - 1)
        buf = np.asarray(results[c]["out"]).astype(np.float32)  # [128, 1024]
        b6 = buf.reshape(4, BLK, B, 2, 4, BLK, CO)  # (band, j, b, h, li, j', o)
        d = b6[:, idx, :, :, :, idx, :]             # (j, band, b, h, li, o)
        for g in range(V13_NGRP):
            row, half = g // 2, g % 2
            if row >= nreal:
                continue
            # q = (4*half + li)*4 + j
            blkq = d[:, g % 4, :, g // 4, :, :]     # (j, b, li, o)
            q0 = 16 * half
            arr = blkq.transpose(1, 3, 2, 0).reshape(B, CO, 16)  # (b, o, li*4+j)
            qs = np.arange(q0, q0 + 16)
            sel = qs < WO
            out[:, :, 4 * c + row, qs[sel]] = arr[:, :, sel]
        if nreal < RPC:
            pass
    return out


V10_GTOT = NCHUNK * GLP * CO + NCHUNK * GLP * B   # 3840 cols/group: weight | windows


def _build_nc_v10(repeat=1, dt=None):
    """Like v8h but weight+windows interleaved per group in ONE DRAM tensor:
    one DMA per group (4 input DMAs total) — each dma_start costs ~1.5us of
    serialized ring time here, so DMA count is the dominant knob."""
    if dt is None:
        dt = mybir.dt.float16
    BLK = V89_BLK
    NBLK = V89_NBLK
    gw = V89_GW
    gtot = V10_GTOT
    bout = V89_BOUT
    orows = V89_OROWS
    nc = bacc.Bacc("TRN2", target_bir_lowering=False)
    wx = nc.dram_tensor("wx", [CK, GROUPS * gtot], dt, kind="ExternalInput")
    out = nc.dram_tensor("out", [orows, V89_OUTC], mybir.dt.float32, kind="ExternalOutput")
    with tile.TileContext(nc) as tc:
        with (
            tc.tile_pool(name="wp", bufs=2) as wp,
            tc.tile_pool(name="pp", bufs=4, space="PSUM") as pp,
            tc.tile_pool(name="op", bufs=2) as op,
        ):
            def body():
                t = wp.tile([CK, GROUPS * gtot], dt, tag="t", name="t")
                for g in range(GROUPS):
                    nc.sync.dma_start(t[:, g * gtot:(g + 1) * gtot],
                                      wx.ap()[:, g * gtot:(g + 1) * gtot])
                ot = op.tile([orows, V89_OUTC], mybir.dt.float32, tag="ot", name="ot")
                gout = NBLK * bout
                for g in range(GROUPS):
                    base = g * gtot
                    for bl in range(NBLK):
                        ps = pp.tile([orows, bout], mybir.dt.float32, tag="ps", name="ps")
                        for c in range(NCHUNK):
                            lo = base + gw + c * (GLP * B) + bl * (BLK * B)
                            ro = base + c * (GLP * CO) + bl * bout
                            nc.tensor.matmul(
                                ps[:],
                                lhsT=t[:, lo:lo + BLK * B],
                                rhs=t[:, ro:ro + bout],
                                start=(c == 0), stop=(c == NCHUNK - 1))
                        nc.vector.tensor_copy(
                            ot[:, (g * NBLK + bl) * bout:(g * NBLK + bl + 1) * bout], ps[:])
                    if g == GROUPS - 2:
                        # first 3/4 of the output leaves while group 3 computes
                        nc.gpsimd.dma_start(out.ap()[:, :3 * gout], ot[:, :3 * gout])
                nc.gpsimd.dma_start(out.ap()[:, 3 * gout:], ot[:, 3 * gout:])
            if repeat == 1:
                body()
            else:
                with tc.For_i(0, repeat, 1):
                    body()
    nc.compile()
    return nc


def _host_prep_v10(x, weight, npdt=None):
    if npdt is None:
        npdt = np.float16
    maps = _host_prep_v89(x, weight, three_term=False, npdt=npdt)
    gw = V89_GW
    gwin = V89_GWIN
    out_maps = []
    for m in maps:
        wh = m["wh"].reshape(CK, GROUPS, gw)
        vh = m["winh"].reshape(CK, GROUPS, gwin)
        wx = np.concatenate([wh, vh], axis=2).reshape(CK, GROUPS * V10_GTOT)
        out_maps.append({"wx": np.ascontiguousarray(wx)})
    return out_maps


def _host_prep_v4(x, weight, variant):
    dt, BLK, npdt = V4_CFG[variant]
    x = np.ascontiguousarray(np.asarray(x, dtype=np.float32))
    weight = np.ascontiguousarray(np.asarray(weight, dtype=np.float32))
    wins = np.stack(
        [x[:, :, kh:kh + DH * HO:DH, kw:kw + DW * WO:DW]
         for kh in range(KH) for kw in range(KW)],
        axis=-1,
    )
    W2 = weight[0].transpose(1, 4, 2, 3, 0).reshape(IK, HO, WO, CO)
    W3 = wins.transpose(1, 4, 2, 3, 0).reshape(IK, HO, WO, B)
    qpad = list(range(WO)) + [WO - 1]          # 31 real + 1 dup -> 32
    in_maps = []
    for c in range(N_CORES):
        rows = _ROWS_PADDED[c]
        # (ik, group, locp, {o|b})
        wsel = W2[:, rows][:, :, qpad, :]       # (288, 4, 32, CO)
        winsel = W3[:, rows][:, :, qpad, :]     # (288, 4, 32, B)
        # -> [group, CK, chunk, locp, {o|b}] -> [GROUPS*CK, chunk*locp*{o|b}]
        wstk = np.stack([wsel[CK * cc:CK * (cc + 1)] for cc in range(NCHUNK)], axis=2)
        winstk = np.stack([winsel[CK * cc:CK * (cc + 1)] for cc in range(NCHUNK)], axis=2)
        # wstk: (CK, 4, chunk, 32, CO) -> (4, CK, chunk, 32, CO)
        wstk = wstk.transpose(1, 0, 2, 3, 4).reshape(GROUPS * CK, NCHUNK * GLP * CO)
        winstk = winstk.transpose(1, 0, 2, 3, 4).reshape(GROUPS * CK, NCHUNK * GLP * B)
        in_maps.append({
            "wT": np.ascontiguousarray(wstk.astype(npdt)),
            "winT": np.ascontiguousarray(winstk.astype(npdt)),
        })
    return in_maps


def _assemble_v4(results, variant):
    dt, BLK, _ = V4_CFG[variant]
    NBLK = GLP // BLK
    out = np.empty((B, CO, HO, WO), np.float32)
    idx = np.arange(BLK)
    for c in range(N_CORES):
        nreal = RPC if c < N_CORES - 1 else HO - 4 * (N_CORES - 1)
        buf = np.asarray(results[c]["out"])
        b6 = buf.reshape(BLK, B, GROUPS, NBLK, BLK, CO)
        d = b6[idx, :, :, :, idx, :]            # (BLK, B, GROUPS, NBLK, CO)
        dd = d.transpose(1, 4, 2, 3, 0).reshape(B, CO, GROUPS, NBLK * BLK)
        out[:, :, 4 * c:4 * c + nreal, :] = dd[:, :, :nreal, :WO]
    return out


def _build_nc(repeat=1, variant="v2"):
    nc = bacc.Bacc("TRN2", target_bir_lowering=False)
    wT = nc.dram_tensor("wT", [CK, W_COLS], mybir.dt.float32, kind="ExternalInput")
    winT = nc.dram_tensor("winT", [CK, WIN_COLS], mybir.dt.float32, kind="ExternalInput")
    out_cols = OUT_COLS if variant == "v1" else V2_OUT_COLS
    out_rows = CO if variant == "v1" else 128
    out = nc.dram_tensor("out", [out_rows, out_cols], mybir.dt.float32, kind="ExternalOutput")

    gw = GL * NCHUNK * CO    # weight cols per group
    gwin = GL * NCHUNK * B   # window cols per group
    gout = GL * B            # v1 out cols per group

    with tile.TileContext(nc) as tc:
        with (
            tc.tile_pool(name="wp", bufs=3) as wp,
            tc.tile_pool(name="winp", bufs=3) as winp,
            tc.tile_pool(name="pp", bufs=2, space="PSUM") as pp,
            tc.tile_pool(name="op", bufs=2) as op,
        ):
            def body_v1():
                for g in range(GROUPS):
                    wt = wp.tile([CK, gw], mybir.dt.float32, tag="wt", name="wt")
                    nc.sync.dma_start(wt[:], wT.ap()[:, g * gw:(g + 1) * gw])
                    wint = winp.tile([CK, gwin], mybir.dt.float32, tag="wint", name="wint")
                    nc.sync.dma_start(wint[:], winT.ap()[:, g * gwin:(g + 1) * gwin])

                    ps = pp.tile([CO, gout], mybir.dt.float32, tag="ps", name="ps")
                    for l in range(GL):
                        for c in range(NCHUNK):
                            nc.tensor.matmul(
                                ps[:, l * B:(l + 1) * B],
                                lhsT=wt[:, (l * NCHUNK + c) * CO:(l * NCHUNK + c + 1) * CO],
                                rhs=wint[:, (l * NCHUNK + c) * B:(l * NCHUNK + c + 1) * B],
                                start=(c == 0),
                                stop=(c == NCHUNK - 1),
                            )

                    ot = op.tile([CO, gout], mybir.dt.float32, tag="ot", name="ot")
                    nc.vector.tensor_copy(ot[:], ps[:])
                    nc.sync.dma_start(out.ap()[:, g * gout:(g + 1) * gout], ot[:])

            def body_v2():
                # stationary = windows (8 cols, cheap fp32 self-load);
                # moving = weight (N=32); out[b, o] block at partition
                # offset 32*(l%4) via col-tiling -> 4 concurrent MM strips.
                for g in range(GROUPS):
                    wt = wp.tile([CK, gw], mybir.dt.float32, tag="wt", name="wt")
                    nc.sync.dma_start(wt[:], wT.ap()[:, g * gw:(g + 1) * gw])
                    wint = winp.tile([CK, gwin], mybir.dt.float32, tag="wint", name="wint")
                    nc.sync.dma_start(wint[:], winT.ap()[:, g * gwin:(g + 1) * gwin])

                    # one PSUM tile per col strip -> different banks, so the
                    # 4 strips' matmuls aren't serialized by bank tracking
                    pss = [
                        pp.tile([128, V2_GOUT], mybir.dt.float32,
                                tag=f"ps{j}", name=f"ps{j}", bufs=2)
                        for j in range(4)
                    ]
                    for l in range(GL):
                        j = l % 4
                        blk = l // 4
                        for c in range(NCHUNK):
                            nc.tensor.matmul(
                                pss[j][32 * j:32 * j + B, blk * CO:(blk + 1) * CO],
                                lhsT=wint[:, (l * NCHUNK + c) * B:(l * NCHUNK + c + 1) * B],
                                rhs=wt[:, (l * NCHUNK + c) * CO:(l * NCHUNK + c + 1) * CO],
                                start=(c == 0),
                                stop=(c == NCHUNK - 1),
                                tile_position=(0, 32 * j),
                            )

                    ot = op.tile([128, V2_GOUT], mybir.dt.float32, tag="ot", name="ot")
                    for j in range(4):
                        nc.vector.tensor_copy(
                            ot[32 * j:32 * (j + 1), :],
                            pss[j][32 * j:32 * (j + 1), :],
                        )
                    nc.sync.dma_start(out.ap()[:, g * V2_GOUT:(g + 1) * V2_GOUT], ot[:])

            body = body_v1 if variant == "v1" else body_v2
            if repeat == 1:
                body()
            else:
                with tc.For_i(0, repeat, 1):
                    body()
    nc.compile()
    return nc


def _host_prep(x, weight):
    """Build per-core DMA-ready layouts. Pure indexing/transpose, no math."""
    x = np.ascontiguousarray(np.asarray(x, dtype=np.float32))
    weight = np.ascontiguousarray(np.asarray(weight, dtype=np.float32))

    # windows[b, i, p, q, k] with k = kh*3+kw (matches torch unfold flatten)
    wins = np.stack(
        [x[:, :, kh:kh + DH * HO:DH, kw:kw + DW * WO:DW]
         for kh in range(KH) for kw in range(KW)],
        axis=-1,
    )  # (B, CI, HO, WO, 9)

    # (ik, p, q, o) and (ik, p, q, b)
    W2 = weight[0].transpose(1, 4, 2, 3, 0).reshape(IK, HO, WO, CO)
    W3 = wins.transpose(1, 4, 2, 3, 0).reshape(IK, HO, WO, B)

    in_maps = []
    for c in range(N_CORES):
        rows = _ROWS_PADDED[c]
        wsel = W2[:, rows].reshape(IK, L, CO)
        winsel = W3[:, rows].reshape(IK, L, B)
        # [CK, loc, chunk, {o|b}] — partition r of chunk-c col region holds ik=96c+r
        wT = np.stack([wsel[CK * cc:CK * (cc + 1)] for cc in range(NCHUNK)], axis=2)
        winT = np.stack([winsel[CK * cc:CK * (cc + 1)] for cc in range(NCHUNK)], axis=2)
        in_maps.append({
            "wT": np.ascontiguousarray(wT.reshape(CK, W_COLS)),
            "winT": np.ascontiguousarray(winT.reshape(CK, WIN_COLS)),
        })
    return in_maps


def _assemble(results, variant="v2"):
    out = np.empty((B, CO, HO, WO), np.float32)
    qs = np.arange(WO)
    for c in range(N_CORES):
        nreal = RPC if c < N_CORES - 1 else HO - 4 * (N_CORES - 1)
        buf = np.asarray(results[c]["out"])
        if variant == "v1":
            rr = buf.reshape(CO, RPC, WO, B)
            for j in range(nreal):
                out[:, :, 4 * c + j, :] = rr[:, j, :, :].transpose(2, 0, 1)
        else:
            # buf [128, GROUPS*256]: row = 32*(q%4)+b, col = g*256+(q//4)*32+o
            b4 = buf.reshape(4, 32, GROUPS, 8, CO)
            res = b4[qs % 4, :B, :, qs // 4, :]      # (31, b, g, o)
            out[:, :, 4 * c:4 * c + nreal, :] = res.transpose(1, 3, 2, 0)[:, :, :nreal, :]
    return out


VARIANT = os.environ.get("LC2D_VARIANT", "v13")


def timing_setup(x, weight):
    """(in_maps, build_fn) for test.py's slope timing."""
    if VARIANT == "v13":
        return _host_prep_v13(x, weight), (lambda n: _build_nc_v13(n))
    if VARIANT == "v12":
        return _host_prep_v12(x, weight), (lambda n: _build_nc_v12(n))
    if VARIANT == "v11":
        return _host_prep_v11(x, weight), (lambda n: _build_nc_v11(n))
    if VARIANT == "v10":
        return _host_prep_v10(x, weight), (lambda n: _build_nc_v10(n))
    raise NotImplementedError(VARIANT)


def kernel(x, weight, _trace=False, _trace_cores=None):
    if VARIANT == "v13":
        in_maps = _host_prep_v13(x, weight)
    elif VARIANT == "v12":
        in_maps = _host_prep_v12(x, weight)
    elif VARIANT == "v11":
        in_maps = _host_prep_v11(x, weight)
    elif VARIANT == "v10":
        in_maps = _host_prep_v10(x, weight)
    elif VARIANT in ("v8", "v9", "v8h", "v9h"):
        in_maps = _host_prep_v89(
            x, weight, three_term=(VARIANT in ("v9", "v9h")),
            npdt=(np.float16 if VARIANT.endswith("h") else ml_dtypes.bfloat16))
    elif VARIANT in V4_CFG:
        in_maps = _host_prep_v4(x, weight, VARIANT)
    elif VARIANT == "v5":
        in_maps = _host_prep_v5(x, weight)
    else:
        in_maps = _host_prep(x, weight)
    if "nc" not in _NC_CACHE:
        if VARIANT == "v13":
            _NC_CACHE["nc"] = _build_nc_v13(1)
        elif VARIANT == "v12":
            _NC_CACHE["nc"] = _build_nc_v12(1)
        elif VARIANT == "v11":
            _NC_CACHE["nc"] = _build_nc_v11(1)
        elif VARIANT == "v10":
            _NC_CACHE["nc"] = _build_nc_v10(1)
        elif VARIANT in ("v8", "v9", "v8h", "v9h"):
            _NC_CACHE["nc"] = _build_nc_v89(
                1, three_term=(VARIANT in ("v9", "v9h")),
                dt=(mybir.dt.float16 if VARIANT.endswith("h") else mybir.dt.bfloat16))
        elif VARIANT in V4_CFG:
            _NC_CACHE["nc"] = _build_nc_v4(1, VARIANT)
        elif VARIANT == "v5":
            _NC_CACHE["nc"] = _build_nc_v5()
        else:
            _NC_CACHE["nc"] = _build_nc(variant=VARIANT)
    nc = _NC_CACHE["nc"]
    res = run_bass_kernel_spmd(
        nc, in_maps, core_ids=list(range(N_CORES)),
        trace=_trace, trace_cores=_trace_cores,
    )
    if VARIANT == "v13":
        out = _assemble_v13(res.results)
    elif VARIANT == "v12":
        out = _assemble_v12(res.results)
    elif VARIANT == "v11":
        out = _assemble_v11(res.results)
    elif VARIANT in ("v8", "v9", "v8h", "v9h", "v10"):
        out = _assemble_v89(res.results)
    elif VARIANT in V4_CFG:
        out = _assemble_v4(res.results, VARIANT)
    elif VARIANT == "v5":
        out = _assemble_v5(res.results)
    else:
        out = _assemble(res.results, variant=VARIANT)
    if _trace:
        return out, res
    return out


if __name__ == "__main__":
    # quick self-check with random data against a numpy oracle
    rng = np.random.default_rng(0)
    x = rng.standard_normal((B, CI, H, W), dtype=np.float32)
    weight = rng.standard_normal((1, CO, CI, HO, WO, KH * KW), dtype=np.float32)
    wins = np.stack(
        [x[:, :, kh:kh + DH * HO:DH, kw:kw + DW * WO:DW]
         for kh in range(KH) for kw in range(KW)], axis=-1)
    expected = np.einsum("bipqk,oipqk->bopq", wins, weight[0], optimize=True)
    actual = kernel(x, weight)
    err = np.abs(actual - expected).max() / np.abs(expected).max()
    print("max out:", np.abs(expected).max(), "rel err:", err)
    tol = 1e-5 if VARIANT in ("v1", "v2", "v5") else (1e-2 if VARIANT in ("v8", "v4b", "v4b8") else 1e-3)
    assert err < tol, (err, tol)
    print("KERNEL OK")



# revision 50
# speedup vs baseline: 2.7047x; 1.0192x over previous
"""LocallyConnected2d Trainium2 kernel (8-core SPMD).

out[b,o,p,q] = sum_{i,kh,kw} x[b, i, 2p+kh, 2q+kw] * weight[0, o, i, p, q, kh*3+kw]

Shipped variant "v13" (~10.6us/iter vs the 24.3us v10 baseline):
- Shard the H' (=31) output-row dim across 8 cores (4 rows/core; core 7
  gets one duplicated padding row so the SPMD program is uniform). This
  splits the dominant traffic — the 35.4MB per-location weight — 8 ways.
- The kernel is a pure fp16 streaming problem (~3MB/core input). Measured
  HW DMA is per-SBUF-partition limited (~2.7GB/s/partition, ~345GB/s/core
  at 128 partitions), so the layout spreads bytes over ALL 128 partitions:
  contraction 288 = A(ik 0:128) + B(128:256) + C(256:288), with C's
  weights packed 4-blocks-per-32-partition-band and C's windows
  zero-padded to 128 rows so the zeros null the other blocks' band rows.
  This keeps every matmul on the SAME (128,32) PE tile config —
  alternating PE tile sizes measured >2x slower on HW.
- Per block of 4 locations: 3 accumulating matmuls (A/B/C),
  lhsT = windows [128, 32], rhs = weight [128, 128] -> psum [32, 128] in a
  rotating PSUM column band (tile_position (0, 32*band)); one [128, 512]
  fp32->fp16 copy per 4 groups (DVE / ACT); single fp16 out DMA.
  Diagonal extraction of the 4x-blocked output happens host-side.
- Exactly 8 DMAs per body so bass's 8 rotating DMA-completion semaphores
  reuse body-aligned (9+ DMAs made an input wait on the previous body's
  output DMA, serializing the stream).
- PE p-state warmup chain (~3us of dummy matmuls at startup) so real
  matmuls run at full clock.
- Timing builds unroll 16 bodies per For_i iteration: the all-engine
  barrier is per-For_i-iter, so consecutive bodies pipeline and the
  steady-state per-body time approaches the DMA stream floor (~9us).
"""

import os
import numpy as np
import ml_dtypes

import concourse.bacc as bacc
import concourse.mybir as mybir
import concourse.tile as tile
from concourse.bass_utils import run_bass_kernel_spmd

# Problem shapes (hardcoded per contract).
B, CI, H, W = 8, 32, 64, 64
CO = 32
KH = KW = 3
DH = DW = 2
HO = WO = 31
N_CORES = 8
RPC = 4                 # padded H'-rows per core
L = RPC * WO            # 124 locations per core
IK = CI * KH * KW       # 288 contraction
NCHUNK = 3
CK = IK // NCHUNK       # 96 partitions per chunk
GROUPS = RPC            # one compute/DMA group per H'-row
GL = L // GROUPS        # 31 locations per group

W_COLS = L * NCHUNK * CO     # 11904
WIN_COLS = L * NCHUNK * B    # 2976
OUT_COLS = L * B             # 992

_ROWS_PADDED = [[min(4 * c + j, HO - 1) for j in range(RPC)] for c in range(N_CORES)]

_NC_CACHE = {}


V2_GOUT = 256               # psum cols per group in v2: 8 col-blocks x 32 (o)
V2_OUT_COLS = V2_GOUT * GROUPS

# v4: blocked matmuls — BLK locations share one matmul (out is a BLK x BLK
# grid of [b, o] tiles; only the diagonal is useful, extracted host-side).
# fp32r needs moving free dim >= 256 for the 1 cycle/row fast path.
GLP = 32                    # padded locs per group (31 real + 1 dup)
V4_CFG = {
    "v4r": (mybir.dt.float32r, 8, np.float32),
    "v4b": (mybir.dt.bfloat16, 4, ml_dtypes.bfloat16),
    "v4b8": (mybir.dt.bfloat16, 8, ml_dtypes.bfloat16),
}


def _build_nc_v4(repeat, variant):
    dt, BLK, _ = V4_CFG[variant]
    NBLK = GLP // BLK
    gw = NCHUNK * GLP * CO   # 3072 weight cols per group
    gwin = NCHUNK * GLP * B  # 768 win cols per group
    bout = BLK * CO          # out cols per block
    orows = B * BLK          # out rows per block
    out_cols = GROUPS * NBLK * bout

    nc = bacc.Bacc("TRN2", target_bir_lowering=False)
    wT = nc.dram_tensor("wT", [GROUPS * CK, gw], dt, kind="ExternalInput")
    winT = nc.dram_tensor("winT", [GROUPS * CK, gwin], dt, kind="ExternalInput")
    out = nc.dram_tensor("out", [orows, out_cols], mybir.dt.float32, kind="ExternalOutput")

    with tile.TileContext(nc) as tc:
        with (
            tc.tile_pool(name="wp", bufs=3) as wp,
            tc.tile_pool(name="winp", bufs=3) as winp,
            tc.tile_pool(name="pp", bufs=4, space="PSUM") as pp,
            tc.tile_pool(name="op", bufs=4) as op,
        ):
            def body():
                for g in range(GROUPS):
                    wt = wp.tile([CK, gw], dt, tag="wt", name="wt")
                    nc.sync.dma_start(wt[:], wT.ap()[g * CK:(g + 1) * CK, :])
                    wint = winp.tile([CK, gwin], dt, tag="wint", name="wint")
                    nc.sync.dma_start(wint[:], winT.ap()[g * CK:(g + 1) * CK, :])

                    for bl in range(NBLK):
                        ps = pp.tile([orows, bout], mybir.dt.float32, tag="ps", name="ps")
                        for c in range(NCHUNK):
                            nc.tensor.matmul(
                                ps[:],
                                lhsT=wint[:, c * (GLP * B) + bl * (BLK * B):
                                          c * (GLP * B) + (bl + 1) * (BLK * B)],
                                rhs=wt[:, c * (GLP * CO) + bl * bout:
                                       c * (GLP * CO) + (bl + 1) * bout],
                                start=(c == 0),
                                stop=(c == NCHUNK - 1),
                            )
                        ot = op.tile([orows, bout], mybir.dt.float32, tag="ot", name="ot")
                        nc.vector.tensor_copy(ot[:], ps[:])
                        nc.sync.dma_start(
                            out.ap()[:, (g * NBLK + bl) * bout:(g * NBLK + bl + 1) * bout],
                            ot[:],
                        )

            if repeat == 1:
                body()
            else:
                with tc.For_i(0, repeat, 1):
                    body()
    nc.compile()
    return nc


def _build_nc_v5(repeat=1):
    """fp32 exact; all DMAs 128-partition; contraction 128+128+32 with the
    32-row remainder of all 4 groups packed into one 128-row tile."""
    gw = GL * CO     # 992 weight cols per (group, chunk)
    gwin = GL * B    # 248 win cols per (group, chunk)
    nc = bacc.Bacc("TRN2", target_bir_lowering=False)
    w01 = nc.dram_tensor("w01", [GROUPS * 2 * 128, gw], mybir.dt.float32, kind="ExternalInput")
    win01 = nc.dram_tensor("win01", [GROUPS * 2 * 128, gwin], mybir.dt.float32, kind="ExternalInput")
    w2 = nc.dram_tensor("w2", [GROUPS * 32, gw], mybir.dt.float32, kind="ExternalInput")
    win2 = nc.dram_tensor("win2", [GROUPS * 32, gwin], mybir.dt.float32, kind="ExternalInput")
    out = nc.dram_tensor("out", [GROUPS * 128, V2_GOUT], mybir.dt.float32, kind="ExternalOutput")

    with tile.TileContext(nc) as tc:
        with (
            tc.tile_pool(name="wp", bufs=3) as wp,
            tc.tile_pool(name="winp", bufs=3) as winp,
            tc.tile_pool(name="pp", bufs=2, space="PSUM") as pp,
            tc.tile_pool(name="op", bufs=2) as op,
        ):
            def body():
                for g in range(GROUPS):
                    wts, wints = [], []
                    for cc in range(2):
                        wt = wp.tile([128, gw], mybir.dt.float32, tag=f"wt{cc}", name=f"wt{cc}")
                        nc.sync.dma_start(
                            wt[:], w01.ap()[(g * 2 + cc) * 128:(g * 2 + cc + 1) * 128, :])
                        wint = winp.tile([128, gwin], mybir.dt.float32, tag=f"wint{cc}", name=f"wint{cc}")
                        nc.sync.dma_start(
                            wint[:], win01.ap()[(g * 2 + cc) * 128:(g * 2 + cc + 1) * 128, :])
                        wts.append(wt)
                        wints.append(wint)
                    w2t = wp.tile([32, gw], mybir.dt.float32, tag="w2t", name="w2t")
                    nc.sync.dma_start(w2t[:], w2.ap()[g * 32:(g + 1) * 32, :])
                    win2t = winp.tile([32, gwin], mybir.dt.float32, tag="win2t", name="win2t")
                    nc.sync.dma_start(win2t[:], win2.ap()[g * 32:(g + 1) * 32, :])

                    pss = [
                        pp.tile([128, V2_GOUT], mybir.dt.float32,
                                tag=f"ps{j}", name=f"ps{j}", bufs=2)
                        for j in range(4)
                    ]
                    for l in range(GL):
                        j = l % 4
                        blk = l // 4
                        dst = pss[j][32 * j:32 * j + B, blk * CO:(blk + 1) * CO]
                        for cc in range(2):
                            nc.tensor.matmul(
                                dst,
                                lhsT=wints[cc][:, l * B:(l + 1) * B],
                                rhs=wts[cc][:, l * CO:(l + 1) * CO],
                                start=(cc == 0),
                                stop=False,
                                tile_position=(0, 32 * j),
                            )
                        nc.tensor.matmul(
                            dst,
                            lhsT=win2t[:, l * B:(l + 1) * B],
                            rhs=w2t[:, l * CO:(l + 1) * CO],
                            start=False,
                            stop=True,
                            tile_position=(0, 32 * j),
                        )

                    ot = op.tile([128, V2_GOUT], mybir.dt.float32, tag="ot", name="ot")
                    for j in range(4):
                        nc.vector.tensor_copy(
                            ot[32 * j:32 * (j + 1), :],
                            pss[j][32 * j:32 * (j + 1), :],
                        )
                    nc.sync.dma_start(out.ap()[g * 128:(g + 1) * 128, :], ot[:])

            if repeat == 1:
                body()
            else:
                with tc.For_i(0, repeat, 1):
                    body()
    nc.compile()
    return nc


def _host_prep_v5(x, weight):
    x = np.ascontiguousarray(np.asarray(x, dtype=np.float32))
    weight = np.ascontiguousarray(np.asarray(weight, dtype=np.float32))
    wins = np.stack(
        [x[:, :, kh:kh + DH * HO:DH, kw:kw + DW * WO:DW]
         for kh in range(KH) for kw in range(KW)],
        axis=-1,
    )
    W2 = weight[0].transpose(1, 4, 2, 3, 0).reshape(IK, HO, WO, CO)
    W3 = wins.transpose(1, 4, 2, 3, 0).reshape(IK, HO, WO, B)
    in_maps = []
    for c in range(N_CORES):
        rows = _ROWS_PADDED[c]
        wsel = W2[:, rows]       # (288, 4, 31, CO)
        winsel = W3[:, rows]     # (288, 4, 31, B)
        # w01 rows: (g, c01, 128) ; cols (l, o)
        w01 = wsel[:256].reshape(2, 128, GROUPS, GL * CO).transpose(2, 0, 1, 3)
        win01 = winsel[:256].reshape(2, 128, GROUPS, GL * B).transpose(2, 0, 1, 3)
        w2 = wsel[256:].reshape(32, GROUPS, GL * CO).transpose(1, 0, 2)
        win2 = winsel[256:].reshape(32, GROUPS, GL * B).transpose(1, 0, 2)
        in_maps.append({
            "w01": np.ascontiguousarray(w01.reshape(GROUPS * 2 * 128, GL * CO)),
            "win01": np.ascontiguousarray(win01.reshape(GROUPS * 2 * 128, GL * B)),
            "w2": np.ascontiguousarray(w2.reshape(GROUPS * 32, GL * CO)),
            "win2": np.ascontiguousarray(win2.reshape(GROUPS * 32, GL * B)),
        })
    return in_maps


def _assemble_v5(results):
    out = np.empty((B, CO, HO, WO), np.float32)
    qs = np.arange(WO)
    for c in range(N_CORES):
        nreal = RPC if c < N_CORES - 1 else HO - 4 * (N_CORES - 1)
        buf = np.asarray(results[c]["out"])      # [GROUPS*128, 256]
        b5 = buf.reshape(GROUPS, 4, 32, 8, CO)   # (g, strip, 32row, blk, o)
        res = b5[:, qs % 4, :B, qs // 4, :]      # (g?, ...) advanced idx
        # advanced indices qs%4 (dim1) and qs//4 (dim3) -> (31, GROUPS, B, CO)
        out[:, :, 4 * c:4 * c + nreal, :] = res.transpose(2, 3, 1, 0)[:, :, :nreal, :]
    return out


V89_BLK = 4
V89_NBLK = GLP // V89_BLK            # 8 blocks of 4 locs per group
V89_GW = NCHUNK * GLP * CO           # 3072 weight cols per group
V89_GWIN = NCHUNK * GLP * B          # 768 win cols per group
V89_BOUT = V89_BLK * CO              # 128 out cols per block
V89_OROWS = B * V89_BLK              # 32 out rows
V89_OUTC = GROUPS * V89_NBLK * V89_BOUT  # 4096


def _build_nc_v89(repeat=1, three_term=False, dt=None):
    """16-bit blocked kernel, minimal DMA count, split across both HWDGE
    rings. three_term=True computes w≈wh+wl, win≈vh+vl and accumulates
    vh·wh + vh·wl + vl·wh (16-bit products are exact in fp32 -> ~1e-5 rel err).
    """
    if dt is None:
        dt = mybir.dt.bfloat16
    W = GROUPS * V89_GW
    WIN = GROUPS * V89_GWIN
    nc = bacc.Bacc("TRN2", target_bir_lowering=False)
    wh_d = nc.dram_tensor("wh", [CK, W], dt, kind="ExternalInput")
    winh_d = nc.dram_tensor("winh", [CK, WIN], dt, kind="ExternalInput")
    if three_term:
        wl_d = nc.dram_tensor("wl", [CK, W], dt, kind="ExternalInput")
        winl_d = nc.dram_tensor("winl", [CK, WIN], dt, kind="ExternalInput")
    out = nc.dram_tensor("out", [V89_OROWS, V89_OUTC], mybir.dt.float32, kind="ExternalOutput")

    half = W // 2  # 2 groups per ring half
    with tile.TileContext(nc) as tc:
        with (
            tc.tile_pool(name="wp", bufs=2) as wp,
            tc.tile_pool(name="winp", bufs=2) as winp,
            tc.tile_pool(name="pp", bufs=4, space="PSUM") as pp,
            tc.tile_pool(name="op", bufs=2) as op,
        ):
            def body():
                # weight: groups 0-1 via SP ring, groups 2-3 via ACT ring,
                # one piece per group -> compute starts after 1/4 of bytes
                wh = wp.tile([CK, W], dt, tag="wh", name="wh")
                for g in range(2):
                    nc.sync.dma_start(
                        wh[:, g * V89_GW:(g + 1) * V89_GW],
                        wh_d.ap()[:, g * V89_GW:(g + 1) * V89_GW])
                for g in range(2, 4):
                    nc.scalar.dma_start(
                        wh[:, g * V89_GW:(g + 1) * V89_GW],
                        wh_d.ap()[:, g * V89_GW:(g + 1) * V89_GW])
                winh = winp.tile([CK, WIN], dt, tag="winh", name="winh")
                nc.sync.dma_start(winh[:, :WIN // 2], winh_d.ap()[:, :WIN // 2])
                nc.scalar.dma_start(winh[:, WIN // 2:], winh_d.ap()[:, WIN // 2:])
                if three_term:
                    wl = wp.tile([CK, W], dt, tag="wl", name="wl")
                    for g in range(2):
                        nc.scalar.dma_start(
                            wl[:, g * V89_GW:(g + 1) * V89_GW],
                            wl_d.ap()[:, g * V89_GW:(g + 1) * V89_GW])
                    for g in range(2, 4):
                        nc.sync.dma_start(
                            wl[:, g * V89_GW:(g + 1) * V89_GW],
                            wl_d.ap()[:, g * V89_GW:(g + 1) * V89_GW])
                    winl = winp.tile([CK, WIN], dt, tag="winl", name="winl")
                    nc.scalar.dma_start(winl[:, :WIN // 2], winl_d.ap()[:, :WIN // 2])
                    nc.sync.dma_start(winl[:, WIN // 2:], winl_d.ap()[:, WIN // 2:])

                ot = op.tile([V89_OROWS, V89_OUTC], mybir.dt.float32, tag="ot", name="ot")
                for g in range(GROUPS):
                    for bl in range(V89_NBLK):
                        ps = pp.tile([V89_OROWS, V89_BOUT], mybir.dt.float32, tag="ps", name="ps")
                        first = True
                        for c in range(NCHUNK):
                            lo = g * V89_GWIN + c * (GLP * B) + bl * (V89_BLK * B)
                            ro = g * V89_GW + c * (GLP * CO) + bl * V89_BOUT
                            lhs_h = winh[:, lo:lo + V89_BLK * B]
                            rhs_h = wh[:, ro:ro + V89_BOUT]
                            terms = [(lhs_h, rhs_h)]
                            if three_term:
                                terms.append((lhs_h, wl[:, ro:ro + V89_BOUT]))
                                terms.append((winl[:, lo:lo + V89_BLK * B], rhs_h))
                            for ti, (lh, rh) in enumerate(terms):
                                last = (c == NCHUNK - 1) and (ti == len(terms) - 1)
                                nc.tensor.matmul(
                                    ps[:], lhsT=lh, rhs=rh,
                                    start=first, stop=last)
                                first = False
                        nc.vector.tensor_copy(
                            ot[:, (g * V89_NBLK + bl) * V89_BOUT:(g * V89_NBLK + bl + 1) * V89_BOUT],
                            ps[:])
                nc.gpsimd.dma_start(out.ap()[:, :], ot[:])

            if repeat == 1:
                body()
            else:
                with tc.For_i(0, repeat, 1):
                    body()
    nc.compile()
    return nc


def _host_prep_v89(x, weight, three_term=False, npdt=None):
    if npdt is None:
        npdt = ml_dtypes.bfloat16
    x = np.ascontiguousarray(np.asarray(x, dtype=np.float32))
    weight = np.ascontiguousarray(np.asarray(weight, dtype=np.float32))
    wins = np.stack(
        [x[:, :, kh:kh + DH * HO:DH, kw:kw + DW * WO:DW]
         for kh in range(KH) for kw in range(KW)],
        axis=-1,
    )
    W2 = weight[0].transpose(1, 4, 2, 3, 0).reshape(IK, HO, WO, CO)
    W3 = wins.transpose(1, 4, 2, 3, 0).reshape(IK, HO, WO, B)
    qpad = list(range(WO)) + [WO - 1]
    in_maps = []
    for c in range(N_CORES):
        rows = _ROWS_PADDED[c]
        wsel = W2[:, rows][:, :, qpad, :]       # (288, 4, 32, CO)
        winsel = W3[:, rows][:, :, qpad, :]     # (288, 4, 32, B)
        # -> [CK, (group, chunk, locp, {o|b})]
        wstk = np.stack([wsel[CK * cc:CK * (cc + 1)] for cc in range(NCHUNK)], axis=2)
        winstk = np.stack([winsel[CK * cc:CK * (cc + 1)] for cc in range(NCHUNK)], axis=2)
        # (CK, 4, chunk, 32, X) -> (CK, group*chunk*locp*X)
        wfull = wstk.reshape(CK, GROUPS * NCHUNK * GLP * CO)
        winfull = winstk.reshape(CK, GROUPS * NCHUNK * GLP * B)
        m = {}
        wh = wfull.astype(npdt)
        vh = winfull.astype(npdt)
        m["wh"] = np.ascontiguousarray(wh)
        m["winh"] = np.ascontiguousarray(vh)
        if three_term:
            m["wl"] = np.ascontiguousarray(
                (wfull - wh.astype(np.float32)).astype(npdt))
            m["winl"] = np.ascontiguousarray(
                (winfull - vh.astype(np.float32)).astype(npdt))
        in_maps.append(m)
    return in_maps


def _assemble_v89(results):
    BLK = V89_BLK
    NBLK = V89_NBLK
    out = np.empty((B, CO, HO, WO), np.float32)
    idx = np.arange(BLK)
    for c in range(N_CORES):
        nreal = RPC if c < N_CORES - 1 else HO - 4 * (N_CORES - 1)
        buf = np.asarray(results[c]["out"])          # [32, 4096]
        b6 = buf.reshape(BLK, B, GROUPS, NBLK, BLK, CO)
        d = b6[idx, :, :, :, idx, :]                 # (BLK, B, G, NBLK, CO)
        dd = d.transpose(1, 4, 2, 3, 0).reshape(B, CO, GROUPS, NBLK * BLK)
        out[:, :, 4 * c:4 * c + nreal, :] = dd[:, :, :nreal, :WO]
    return out


V11_NP = 8                      # pieces (half H'-rows) per core
V11_PL = 16                     # padded locations per piece
V11_NBLK = 4                    # blocks of BLK=4 locs per piece
V11_GW = NCHUNK * V11_PL * CO   # 1536 weight cols per piece
V11_GWIN = NCHUNK * V11_PL * B  # 384 win cols per piece
V11_GTOT = V11_GW + V11_GWIN    # 1920
V11_POUT = V11_NBLK * V89_BLK * CO  # 512 out cols per piece
V11_OUTC = V11_NP * V11_POUT    # 4096


def _build_nc_v11(repeat=1, dt=None):
    """8 self-contained pieces (16 locs each), one input DMA per piece on the
    SP queue; matmuls accumulate into a [32, 512] PSUM tile per piece (one
    bank); output DMA'd straight from PSUM on the Pool/SWDGE queue — no
    PSUM->SBUF copies at all."""
    if dt is None:
        dt = mybir.dt.float16
    BLK = V89_BLK
    orows = V89_OROWS           # 32 = BLK * B
    nc = bacc.Bacc("TRN2", target_bir_lowering=False)
    wx = nc.dram_tensor("wx", [CK, V11_NP * V11_GTOT], dt, kind="ExternalInput")
    out = nc.dram_tensor("out", [orows, V11_OUTC], mybir.dt.float32, kind="ExternalOutput")
    with tile.TileContext(nc) as tc:
        with (
            tc.tile_pool(name="wp", bufs=2) as wp,
            tc.tile_pool(name="pp", bufs=4, space="PSUM") as pp,
        ):
            def body():
                t = wp.tile([CK, V11_NP * V11_GTOT], dt, tag="t", name="t")
                for p in range(V11_NP):
                    nc.sync.dma_start(t[:, p * V11_GTOT:(p + 1) * V11_GTOT],
                                      wx.ap()[:, p * V11_GTOT:(p + 1) * V11_GTOT])
                for p in range(V11_NP):
                    base = p * V11_GTOT
                    ps = pp.tile([orows, V11_POUT], mybir.dt.float32, tag="ps", name="ps")
                    for bl in range(V11_NBLK):
                        for c in range(NCHUNK):
                            lo = base + V11_GW + c * (V11_PL * B) + bl * (BLK * B)
                            ro = base + c * (V11_PL * CO) + bl * (BLK * CO)
                            nc.tensor.matmul(
                                ps[:, bl * (BLK * CO):(bl + 1) * (BLK * CO)],
                                lhsT=t[:, lo:lo + BLK * B],
                                rhs=t[:, ro:ro + BLK * CO],
                                start=(c == 0), stop=(c == NCHUNK - 1))
                    nc.gpsimd.dma_start(
                        out.ap()[:, p * V11_POUT:(p + 1) * V11_POUT], ps[:])
            if repeat == 1:
                body()
            else:
                with tc.For_i(0, repeat, 1):
                    body()
    nc.compile()
    return nc


def _host_prep_v11(x, weight, npdt=None):
    if npdt is None:
        npdt = np.float16
    m = _host_prep_v89(x, weight, three_term=False, npdt=npdt)
    out_maps = []
    for mm in m:
        # wh: (CK, group4, chunk3, locp32, CO) ; winh: (..., B)
        wh = mm["wh"].reshape(CK, GROUPS, NCHUNK, GLP, CO)
        vh = mm["winh"].reshape(CK, GROUPS, NCHUNK, GLP, B)
        # -> (CK, row4, half2, chunk3, loc16, X)
        wp = wh.reshape(CK, GROUPS, NCHUNK, 2, V11_PL, CO).transpose(0, 1, 3, 2, 4, 5)
        vp = vh.reshape(CK, GROUPS, NCHUNK, 2, V11_PL, B).transpose(0, 1, 3, 2, 4, 5)
        wp = wp.reshape(CK, V11_NP, V11_GW)
        vp = vp.reshape(CK, V11_NP, V11_GWIN)
        wx = np.concatenate([wp, vp], axis=2).reshape(CK, V11_NP * V11_GTOT)
        out_maps.append({"wx": np.ascontiguousarray(wx)})
    return out_maps


def _assemble_v11(results):
    BLK = V89_BLK
    out = np.empty((B, CO, HO, WO), np.float32)
    idx = np.arange(BLK)
    for c in range(N_CORES):
        nreal = RPC if c < N_CORES - 1 else HO - 4 * (N_CORES - 1)
        buf = np.asarray(results[c]["out"])          # [32, 4096]
        b6 = buf.reshape(BLK, B, V11_NP, V11_NBLK, BLK, CO)
        d = b6[idx, :, :, :, idx, :]                 # (BLK, B, P, NBLK, CO)
        dd = d.transpose(1, 4, 2, 3, 0).reshape(B, CO, GROUPS, 2 * V11_NBLK * BLK)
        out[:, :, 4 * c:4 * c + nreal, :] = dd[:, :, :nreal, :WO]
    return out


V12_NBLKS = 32                  # blocks of BLK=4 locs per core (4 rows x 8)
V12_UNIT = V89_BLK * (B + CO)   # 160 cols per (block, chunk) unit: win | w
V12_BCOLS = NCHUNK * V12_UNIT   # 480 cols per block
V12_PIECES = (5, 5, 5, 5, 4, 4, 4)  # input DMA piece sizes in blocks (sum 32)
V12_OUTC = V12_NBLKS * V89_BLK * CO  # 4096 fp16 out cols (blocked, diag on host)
V12_NGRP = 8                         # compute groups of 4 blocks (1 PSUM bank)


def _build_nc_v12(repeat=1, dt=None, flat=False):
    """Block-major stream: 9 input pieces on SP (tiny last piece to shrink
    the tail), matmuls accumulate in per-piece PSUM banks (all 8), one
    contiguous fp32->fp16 PSUM->SBUF copy per piece round-robin over
    DVE/ACT/Pool, 2 output DMAs; diagonal extraction happens host-side.
    repeat>1 timing builds unroll 8 bodies per For_i iteration so
    consecutive bodies pipeline (the all-engine barrier is per-For_i-iter)."""
    if dt is None:
        dt = mybir.dt.float16
    BLK = V89_BLK
    nc = bacc.Bacc("TRN2", target_bir_lowering=False)
    wx = nc.dram_tensor("wx", [CK, V12_NBLKS * V12_BCOLS], dt, kind="ExternalInput")
    out = nc.dram_tensor("out", [32, V12_OUTC], mybir.dt.float16, kind="ExternalOutput")
    starts = []
    s = 0
    for n in V12_PIECES:
        starts.append(s)
        s += n
    assert s == V12_NBLKS
    BC = BLK * CO
    with tile.TileContext(nc) as tc:
        with (
            tc.tile_pool(name="wp", bufs=2) as wp,
            tc.tile_pool(name="pp", bufs=7, space="PSUM") as pp,
            tc.tile_pool(name="op", bufs=2) as op,
        ):
            def body():
                t = wp.tile([CK, V12_NBLKS * V12_BCOLS], dt, tag="t", name="t")
                for s, n in zip(starts, V12_PIECES):
                    nc.sync.dma_start(
                        t[:, s * V12_BCOLS:(s + n) * V12_BCOLS],
                        wx.ap()[:, s * V12_BCOLS:(s + n) * V12_BCOLS])
                # PE p-state warmup: ~3.4us of dummy matmuls on a memset tile
                # so real matmuls run at full clock once piece 0 lands.
                wu = wp.tile([CK, 256], dt, tag="wu", name="wu")
                nc.vector.memset(wu[:], 0.0)
                psw = pp.tile([32, 512], mybir.dt.float32, tag="psw", name="psw",
                              bufs=1)
                for i in range(12):
                    nc.tensor.matmul(psw[0:8, 0:256], lhsT=wu[:, :8], rhs=wu[:, :256],
                                     start=(i == 0), stop=(i == 11))
                for i in range(10):
                    nc.tensor.matmul(psw[0:8, 256:320], lhsT=wu[:, :8], rhs=wu[:, :64],
                                     start=(i == 0), stop=(i == 9))
                ot = op.tile([32, V12_OUTC], mybir.dt.float16, tag="ot", name="ot")
                for g in range(V12_NGRP):
                    ps = pp.tile([32, 4 * BC], mybir.dt.float32,
                                 tag="ps", name="ps")
                    for li in range(4):
                        u0 = (4 * g + li) * NCHUNK
                        for c in range(NCHUNK):
                            off = (u0 + c) * V12_UNIT
                            nc.tensor.matmul(
                                ps[:, li * BC:(li + 1) * BC],
                                lhsT=t[:, off:off + BLK * B],
                                rhs=t[:, off + BLK * B:off + V12_UNIT],
                                start=(c == 0), stop=(c == NCHUNK - 1))
                    dst = ot[:, g * 4 * BC:(g + 1) * 4 * BC]
                    if g % 2 == 0:
                        nc.vector.tensor_copy(dst, ps[:])
                    else:
                        nc.scalar.copy(dst, ps[:])
                nc.scalar.dma_start(out.ap()[:, :], ot[:, :])
            if repeat == 1:
                body()
            elif flat:
                for _ in range(repeat):
                    body()
            else:
                u = 8 if repeat % 8 == 0 else (4 if repeat % 4 == 0 else 1)
                with tc.For_i(0, repeat // u, 1):
                    for _ in range(u):
                        body()
    nc.compile()
    return nc


def _host_prep_v12(x, weight, npdt=None):
    if npdt is None:
        npdt = np.float16
    m = _host_prep_v89(x, weight, three_term=False, npdt=npdt)
    out_maps = []
    for mm in m:
        # wh: (CK, group4, chunk3, locp32, CO); winh: (..., B)
        wh = mm["wh"].reshape(CK, GROUPS, NCHUNK, GLP, CO)
        vh = mm["winh"].reshape(CK, GROUPS, NCHUNK, GLP, B)
        # -> (CK, block(row,bb)=32, chunk3, BLK locs, X) with loc = blk*4+j
        wb = wh.reshape(CK, GROUPS, NCHUNK, 8, V89_BLK, CO)
        vb = vh.reshape(CK, GROUPS, NCHUNK, 8, V89_BLK, B)
        wb = wb.transpose(0, 1, 3, 2, 4, 5).reshape(CK, V12_NBLKS, NCHUNK, V89_BLK * CO)
        vb = vb.transpose(0, 1, 3, 2, 4, 5).reshape(CK, V12_NBLKS, NCHUNK, V89_BLK * B)
        wx = np.concatenate([vb, wb], axis=3)  # (CK, blk, chunk, 32+128)
        out_maps.append({"wx": np.ascontiguousarray(
            wx.reshape(CK, V12_NBLKS * V12_BCOLS))})
    return out_maps


def _assemble_v12(results):
    BLK = V89_BLK
    out = np.empty((B, CO, HO, WO), np.float32)
    qs = np.arange(WO)
    idx = np.arange(BLK)
    for c in range(N_CORES):
        nreal = RPC if c < N_CORES - 1 else HO - 4 * (N_CORES - 1)
        buf = np.asarray(results[c]["out"]).astype(np.float32)  # [32, 4096]
        b6 = buf.reshape(BLK, B, GROUPS, 8, BLK, CO)  # (j, b, row, bb, j', o)
        d = b6[idx, :, :, :, idx, :]                  # (BLK, B, row, bb, o)
        res = d[qs % 4, :, :, qs // 4, :]             # (31, b, row, o)
        out[:, :, 4 * c:4 * c + nreal, :] = res.transpose(1, 3, 2, 0)[:, :, :nreal, :]
    return out


V13_UNIT = V89_BLK * (B + CO)       # 160 cols per (block, chunk) unit
V13_CSEC = 4 * 32 + 128             # C section: 4 zero-padded win blocks + packed w band
V13_GRP = 4 * 2 * V13_UNIT + V13_CSEC  # 1536 cols per 4-block group
V13_NGRP = 8
V13_OUTC = 1024                     # out [128, 1024] fp16 (blocked, diag on host)


def _build_nc_v13(repeat=1, dt=None, flat=False, dma_only=False, u=16, tbufs=2):
    """128-partition DMA layout: contraction 288 = A(0:128) + B(128:256) on all
    partitions + C(256:288) packed 4-blocks-per-band; the C *window* operand is
    zero-padded to 128 rows so every matmul uses the same (128,32) PE tile
    config (alternating PE tile sizes measured ~2x slower on HW), and the
    zeros null out the other blocks' rows of the packed C weight band.
    Matmul outputs land in rotating PSUM col bands (32*(g%4)); one [128,512]
    copy per 4 groups (DVE then ACT); out [128,1024] fp16. 8 DMAs/body so the
    8 rotating DMA-completion semaphores stay body-aligned; repeat>1 unrolls
    8 bodies per For_i iteration."""
    if dt is None:
        dt = mybir.dt.float16
    BLK = V89_BLK
    nc = bacc.Bacc("TRN2", target_bir_lowering=False)
    wx = nc.dram_tensor("wx", [128, V13_NGRP * V13_GRP], dt, kind="ExternalInput")
    out = nc.dram_tensor("out", [128, V13_OUTC], mybir.dt.float16, kind="ExternalOutput")
    with tile.TileContext(nc) as tc:
        with (
            tc.tile_pool(name="wp", bufs=2) as wp,
            tc.tile_pool(name="pp", bufs=2, space="PSUM") as pp,
            tc.tile_pool(name="op", bufs=2) as op,
        ):
            def body():
                t = wp.tile([128, V13_NGRP * V13_GRP], dt, tag="t", name="t",
                            bufs=tbufs)
                # 7 input DMAs: piece 0 covers groups 0-1, rest one group each
                bounds = [0, 2, 3, 4, 5, 6, 7, 8]
                for i in range(7):
                    lo, hi = bounds[i] * V13_GRP, bounds[i + 1] * V13_GRP
                    nc.sync.dma_start(t[:, lo:hi], wx.ap()[:, lo:hi])
                if dma_only:
                    return
                wu = wp.tile([128, 256], dt, tag="wu", name="wu")
                nc.vector.memset(wu[:], 0.0)
                psw = pp.tile([32, 512], mybir.dt.float32, tag="psw", name="psw",
                              bufs=1)
                for i in range(12):
                    nc.tensor.matmul(psw[0:8, 0:256], lhsT=wu[:, :8], rhs=wu[:, :256],
                                     start=(i == 0), stop=(i == 11))
                for i in range(10):
                    nc.tensor.matmul(psw[0:8, 256:320], lhsT=wu[:, :8], rhs=wu[:, :64],
                                     start=(i == 0), stop=(i == 9))
                ot = op.tile([128, V13_OUTC], mybir.dt.float16, tag="ot", name="ot")
                for h in range(2):
                    psf = pp.tile([128, 512], mybir.dt.float32, tag="ps", name="ps",
                                  bufs=3)
                    for bi in range(4):
                        g = 4 * h + bi
                        base = g * V13_GRP
                        for li in range(4):
                            ab = base + li * (2 * V13_UNIT)
                            cs = base + 8 * V13_UNIT
                            dst = psf[32 * bi:32 * (bi + 1), li * 128:(li + 1) * 128]
                            nc.tensor.matmul(
                                dst, lhsT=t[:, ab:ab + 32],
                                rhs=t[:, ab + 32:ab + V13_UNIT],
                                start=True, stop=False,
                                tile_position=(0, 32 * bi))
                            nc.tensor.matmul(
                                dst, lhsT=t[:, ab + V13_UNIT:ab + V13_UNIT + 32],
                                rhs=t[:, ab + V13_UNIT + 32:ab + 2 * V13_UNIT],
                                start=False, stop=False,
                                tile_position=(0, 32 * bi))
                            nc.tensor.matmul(
                                dst,
                                lhsT=t[:, cs + 32 * li:cs + 32 * (li + 1)],
                                rhs=t[:, cs + 128:cs + 256],
                                start=False, stop=True,
                                tile_position=(0, 32 * bi))
                    dst = ot[:, h * 512:(h + 1) * 512]
                    if h == 0:
                        nc.vector.tensor_copy(dst, psf[:])
                    else:
                        nc.scalar.copy(dst, psf[:])
                nc.scalar.dma_start(out.ap()[:, :], ot[:, :])
            if repeat == 1:
                body()
            elif flat:
                for _ in range(repeat):
                    body()
            else:
                while repeat % u:
                    u //= 2
                with tc.For_i(0, repeat // u, 1):
                    for _ in range(u):
                        body()
    nc.compile()
    return nc


def _host_prep_v13(x, weight, npdt=None):
    if npdt is None:
        npdt = np.float16
    x = np.ascontiguousarray(np.asarray(x, dtype=np.float32))
    weight = np.ascontiguousarray(np.asarray(weight, dtype=np.float32))
    wins = np.stack(
        [x[:, :, kh:kh + DH * HO:DH, kw:kw + DW * WO:DW]
         for kh in range(KH) for kw in range(KW)],
        axis=-1,
    )
    W2 = weight[0].transpose(1, 4, 2, 3, 0).reshape(IK, HO, WO, CO)
    W3 = wins.transpose(1, 4, 2, 3, 0).reshape(IK, HO, WO, B)
    qpad = list(range(WO)) + [WO - 1]
    in_maps = []
    for c in range(N_CORES):
        rows = _ROWS_PADDED[c]
        wsel = W2[:, rows][:, :, qpad, :].astype(npdt)    # (288, 4, 32, CO)
        winsel = W3[:, rows][:, :, qpad, :].astype(npdt)  # (288, 4, 32, B)
        wx = np.zeros((128, V13_NGRP * V13_GRP), npdt)
        for g in range(V13_NGRP):
            row, half = g // 2, g % 2
            base = g * V13_GRP
            for li in range(4):
                bb = 4 * half + li
                ls = slice(bb * 4, (bb + 1) * 4)     # 4 locs of this block
                ab = base + li * (2 * V13_UNIT)
                # A unit: ik 0:128
                wx[:, ab:ab + 32] = winsel[0:128, row, ls, :].reshape(128, 32)
                wx[:, ab + 32:ab + V13_UNIT] = wsel[0:128, row, ls, :].reshape(128, 128)
                # B unit: ik 128:256
                wx[:, ab + V13_UNIT:ab + V13_UNIT + 32] = \
                    winsel[128:256, row, ls, :].reshape(128, 32)
                wx[:, ab + V13_UNIT + 32:ab + 2 * V13_UNIT] = \
                    wsel[128:256, row, ls, :].reshape(128, 128)
                # C section: ik 256:288. win zero-padded to 128 rows (band
                # 32*li holds block li's win); w packed 4-blocks-per-band.
                cs = base + 8 * V13_UNIT
                wx[32 * li:32 * (li + 1), cs + 32 * li:cs + 32 * (li + 1)] = \
                    winsel[256:288, row, ls, :].reshape(32, 32)
                wx[32 * li:32 * (li + 1), cs + 128:cs + 256] = \
                    wsel[256:288, row, ls, :].reshape(32, 128)
        in_maps.append({"wx": np.ascontiguousarray(wx)})
    return in_maps


def _assemble_v13(results):
    BLK = V89_BLK
    out = np.empty((B, CO, HO, WO), np.float32)
    for c in range(N_CORES):
        nreal = RPC if c < N_CORES - 1 else HO - 4 * (N_CORES - 1)
        buf = np.asarray(results[c]["out"]).astype(np.float32)  # [128, 1024]
        idx = np.arange(BLK)
        b6 = buf.reshape(4, BLK, B, 2, 4, BLK, CO)  # (band, j, b, h, li, j', o)
        d = b6[:, idx, :, :, :, idx, :]             # (j, band, b, h, li, o)
        for g in range(V13_NGRP):
            row, half = g // 2, g % 2
            if row >= nreal:
                continue
            blkq = d[:, g % 4, :, g // 4, :, :]     # (j, b, li, o)
            q0 = 16 * half
            arr = blkq.transpose(1, 3, 2, 0).reshape(B, CO, 16)  # (b, o, li*4+j)
            qs = np.arange(q0, q0 + 16)
            sel = qs < WO
            out[:, :, 4 * c + row, qs[sel]] = arr[:, :, sel]
    return out


V10_GTOT = NCHUNK * GLP * CO + NCHUNK * GLP * B   # 3840 cols/group: weight | windows


def _build_nc_v10(repeat=1, dt=None):
    """Like v8h but weight+windows interleaved per group in ONE DRAM tensor:
    one DMA per group (4 input DMAs total) — each dma_start costs ~1.5us of
    serialized ring time here, so DMA count is the dominant knob."""
    if dt is None:
        dt = mybir.dt.float16
    BLK = V89_BLK
    NBLK = V89_NBLK
    gw = V89_GW
    gtot = V10_GTOT
    bout = V89_BOUT
    orows = V89_OROWS
    nc = bacc.Bacc("TRN2", target_bir_lowering=False)
    wx = nc.dram_tensor("wx", [CK, GROUPS * gtot], dt, kind="ExternalInput")
    out = nc.dram_tensor("out", [orows, V89_OUTC], mybir.dt.float32, kind="ExternalOutput")
    with tile.TileContext(nc) as tc:
        with (
            tc.tile_pool(name="wp", bufs=2) as wp,
            tc.tile_pool(name="pp", bufs=4, space="PSUM") as pp,
            tc.tile_pool(name="op", bufs=2) as op,
        ):
            def body():
                t = wp.tile([CK, GROUPS * gtot], dt, tag="t", name="t")
                for g in range(GROUPS):
                    nc.sync.dma_start(t[:, g * gtot:(g + 1) * gtot],
                                      wx.ap()[:, g * gtot:(g + 1) * gtot])
                ot = op.tile([orows, V89_OUTC], mybir.dt.float32, tag="ot", name="ot")
                gout = NBLK * bout
                for g in range(GROUPS):
                    base = g * gtot
                    for bl in range(NBLK):
                        ps = pp.tile([orows, bout], mybir.dt.float32, tag="ps", name="ps")
                        for c in range(NCHUNK):
                            lo = base + gw + c * (GLP * B) + bl * (BLK * B)
                            ro = base + c * (GLP * CO) + bl * bout
                            nc.tensor.matmul(
                                ps[:],
                                lhsT=t[:, lo:lo + BLK * B],
                                rhs=t[:, ro:ro + bout],
                                start=(c == 0), stop=(c == NCHUNK - 1))
                        nc.vector.tensor_copy(
                            ot[:, (g * NBLK + bl) * bout:(g * NBLK + bl + 1) * bout], ps[:])
                    if g == GROUPS - 2:
                        # first 3/4 of the output leaves while group 3 computes
                        nc.gpsimd.dma_start(out.ap()[:, :3 * gout], ot[:, :3 * gout])
                nc.gpsimd.dma_start(out.ap()[:, 3 * gout:], ot[:, 3 * gout:])
            if repeat == 1:
                body()
            else:
                with tc.For_i(0, repeat, 1):
                    body()
    nc.compile()
    return nc


def _host_prep_v10(x, weight, npdt=None):
    if npdt is None:
        npdt = np.float16
    maps = _host_prep_v89(x, weight, three_term=False, npdt=npdt)
    gw = V89_GW
    gwin = V89_GWIN
    out_maps = []
    for m in maps:
        wh = m["wh"].reshape(CK, GROUPS, gw)
        vh = m["winh"].reshape(CK, GROUPS, gwin)
        wx = np.concatenate([wh, vh], axis=2).reshape(CK, GROUPS * V10_GTOT)
        out_maps.append({"wx": np.ascontiguousarray(wx)})
    return out_maps


def _host_prep_v4(x, weight, variant):
    dt, BLK, npdt = V4_CFG[variant]
    x = np.ascontiguousarray(np.asarray(x, dtype=np.float32))
    weight = np.ascontiguousarray(np.asarray(weight, dtype=np.float32))
    wins = np.stack(
        [x[:, :, kh:kh + DH * HO:DH, kw:kw + DW * WO:DW]
         for kh in range(KH) for kw in range(KW)],
        axis=-1,
    )
    W2 = weight[0].transpose(1, 4, 2, 3, 0).reshape(IK, HO, WO, CO)
    W3 = wins.transpose(1, 4, 2, 3, 0).reshape(IK, HO, WO, B)
    qpad = list(range(WO)) + [WO - 1]          # 31 real + 1 dup -> 32
    in_maps = []
    for c in range(N_CORES):
        rows = _ROWS_PADDED[c]
        # (ik, group, locp, {o|b})
        wsel = W2[:, rows][:, :, qpad, :]       # (288, 4, 32, CO)
        winsel = W3[:, rows][:, :, qpad, :]     # (288, 4, 32, B)
        # -> [group, CK, chunk, locp, {o|b}] -> [GROUPS*CK, chunk*locp*{o|b}]
        wstk = np.stack([wsel[CK * cc:CK * (cc + 1)] for cc in range(NCHUNK)], axis=2)
        winstk = np.stack([winsel[CK * cc:CK * (cc + 1)] for cc in range(NCHUNK)], axis=2)
        # wstk: (CK, 4, chunk, 32, CO) -> (4, CK, chunk, 32, CO)
        wstk = wstk.transpose(1, 0, 2, 3, 4).reshape(GROUPS * CK, NCHUNK * GLP * CO)
        winstk = winstk.transpose(1, 0, 2, 3, 4).reshape(GROUPS * CK, NCHUNK * GLP * B)
        in_maps.append({
            "wT": np.ascontiguousarray(wstk.astype(npdt)),
            "winT": np.ascontiguousarray(winstk.astype(npdt)),
        })
    return in_maps


def _assemble_v4(results, variant):
    dt, BLK, _ = V4_CFG[variant]
    NBLK = GLP // BLK
    out = np.empty((B, CO, HO, WO), np.float32)
    idx = np.arange(BLK)
    for c in range(N_CORES):
        nreal = RPC if c < N_CORES - 1 else HO - 4 * (N_CORES - 1)
        buf = np.asarray(results[c]["out"])
        b6 = buf.reshape(BLK, B, GROUPS, NBLK, BLK, CO)
        d = b6[idx, :, :, :, idx, :]            # (BLK, B, GROUPS, NBLK, CO)
        dd = d.transpose(1, 4, 2, 3, 0).reshape(B, CO, GROUPS, NBLK * BLK)
        out[:, :, 4 * c:4 * c + nreal, :] = dd[:, :, :nreal, :WO]
    return out


def _build_nc(repeat=1, variant="v2"):
    nc = bacc.Bacc("TRN2", target_bir_lowering=False)
    wT = nc.dram_tensor("wT", [CK, W_COLS], mybir.dt.float32, kind="ExternalInput")
    winT = nc.dram_tensor("winT", [CK, WIN_COLS], mybir.dt.float32, kind="ExternalInput")
    out_cols = OUT_COLS if variant == "v1" else V2_OUT_COLS
    out_rows = CO if variant == "v1" else 128
    out = nc.dram_tensor("out", [out_rows, out_cols], mybir.dt.float32, kind="ExternalOutput")

    gw = GL * NCHUNK * CO    # weight cols per group
    gwin = GL * NCHUNK * B   # window cols per group
    gout = GL * B            # v1 out cols per group

    with tile.TileContext(nc) as tc:
        with (
            tc.tile_pool(name="wp", bufs=3) as wp,
            tc.tile_pool(name="winp", bufs=3) as winp,
            tc.tile_pool(name="pp", bufs=2, space="PSUM") as pp,
            tc.tile_pool(name="op", bufs=2) as op,
        ):
            def body_v1():
                for g in range(GROUPS):
                    wt = wp.tile([CK, gw], mybir.dt.float32, tag="wt", name="wt")
                    nc.sync.dma_start(wt[:], wT.ap()[:, g * gw:(g + 1) * gw])
                    wint = winp.tile([CK, gwin], mybir.dt.float32, tag="wint", name="wint")
                    nc.sync.dma_start(wint[:], winT.ap()[:, g * gwin:(g + 1) * gwin])

                    ps = pp.tile([CO, gout], mybir.dt.float32, tag="ps", name="ps")
                    for l in range(GL):
                        for c in range(NCHUNK):
                            nc.tensor.matmul(
                                ps[:, l * B:(l + 1) * B],
                                lhsT=wt[:, (l * NCHUNK + c) * CO:(l * NCHUNK + c + 1) * CO],
                                rhs=wint[:, (l * NCHUNK + c) * B:(l * NCHUNK + c + 1) * B],
                                start=(c == 0),
                                stop=(c == NCHUNK - 1),
                            )

                    ot = op.tile([CO, gout], mybir.dt.float32, tag="ot", name="ot")
                    nc.vector.tensor_copy(ot[:], ps[:])
                    nc.sync.dma_start(out.ap()[:, g * gout:(g + 1) * gout], ot[:])

            def body_v2():
                # stationary = windows (8 cols, cheap fp32 self-load);
                # moving = weight (N=32); out[b, o] block at partition
                # offset 32*(l%4) via col-tiling -> 4 concurrent MM strips.
                for g in range(GROUPS):
                    wt = wp.tile([CK, gw], mybir.dt.float32, tag="wt", name="wt")
                    nc.sync.dma_start(wt[:], wT.ap()[:, g * gw:(g + 1) * gw])
                    wint = winp.tile([CK, gwin], mybir.dt.float32, tag="wint", name="wint")
                    nc.sync.dma_start(wint[:], winT.ap()[:, g * gwin:(g + 1) * gwin])

                    # one PSUM tile per col strip -> different banks, so the
                    # 4 strips' matmuls aren't serialized by bank tracking
                    pss = [
                        pp.tile([128, V2_GOUT], mybir.dt.float32,
                                tag=f"ps{j}", name=f"ps{j}", bufs=2)
                        for j in range(4)
                    ]
                    for l in range(GL):
                        j = l % 4
                        blk = l // 4
                        for c in range(NCHUNK):
                            nc.tensor.matmul(
                                pss[j][32 * j:32 * j + B, blk * CO:(blk + 1) * CO],
                                lhsT=wint[:, (l * NCHUNK + c) * B:(l * NCHUNK + c + 1) * B],
                                rhs=wt[:, (l * NCHUNK + c) * CO:(l * NCHUNK + c + 1) * CO],
                                start=(c == 0),
                                stop=(c == NCHUNK - 1),
                                tile_position=(0, 32 * j),
                            )

                    ot = op.tile([128, V2_GOUT], mybir.dt.float32, tag="ot", name="ot")
                    for j in range(4):
                        nc.vector.tensor_copy(
                            ot[32 * j:32 * (j + 1), :],
                            pss[j][32 * j:32 * (j + 1), :],
                        )
                    nc.sync.dma_start(out.ap()[:, g * V2_GOUT:(g + 1) * V2_GOUT], ot[:])

            body = body_v1 if variant == "v1" else body_v2
            if repeat == 1:
                body()
            else:
                with tc.For_i(0, repeat, 1):
                    body()
    nc.compile()
    return nc


def _host_prep(x, weight):
    """Build per-core DMA-ready layouts. Pure indexing/transpose, no math."""
    x = np.ascontiguousarray(np.asarray(x, dtype=np.float32))
    weight = np.ascontiguousarray(np.asarray(weight, dtype=np.float32))

    # windows[b, i, p, q, k] with k = kh*3+kw (matches torch unfold flatten)
    wins = np.stack(
        [x[:, :, kh:kh + DH * HO:DH, kw:kw + DW * WO:DW]
         for kh in range(KH) for kw in range(KW)],
        axis=-1,
    )  # (B, CI, HO, WO, 9)

    # (ik, p, q, o) and (ik, p, q, b)
    W2 = weight[0].transpose(1, 4, 2, 3, 0).reshape(IK, HO, WO, CO)
    W3 = wins.transpose(1, 4, 2, 3, 0).reshape(IK, HO, WO, B)

    in_maps = []
    for c in range(N_CORES):
        rows = _ROWS_PADDED[c]
        wsel = W2[:, rows].reshape(IK, L, CO)
        winsel = W3[:, rows].reshape(IK, L, B)
        # [CK, loc, chunk, {o|b}] — partition r of chunk-c col region holds ik=96c+r
        wT = np.stack([wsel[CK * cc:CK * (cc + 1)] for cc in range(NCHUNK)], axis=2)
        winT = np.stack([winsel[CK * cc:CK * (cc + 1)] for cc in range(NCHUNK)], axis=2)
        in_maps.append({
            "wT": np.ascontiguousarray(wT.reshape(CK, W_COLS)),
            "winT": np.ascontiguousarray(winT.reshape(CK, WIN_COLS)),
        })
    return in_maps


def _assemble(results, variant="v2"):
    out = np.empty((B, CO, HO, WO), np.float32)
    qs = np.arange(WO)
    for c in range(N_CORES):
        nreal = RPC if c < N_CORES - 1 else HO - 4 * (N_CORES - 1)
        buf = np.asarray(results[c]["out"])
        if variant == "v1":
            rr = buf.reshape(CO, RPC, WO, B)
            for j in range(nreal):
                out[:, :, 4 * c + j, :] = rr[:, j, :, :].transpose(2, 0, 1)
        else:
            # buf [128, GROUPS*256]: row = 32*(q%4)+b, col = g*256+(q//4)*32+o
            b4 = buf.reshape(4, 32, GROUPS, 8, CO)
            res = b4[qs % 4, :B, :, qs // 4, :]      # (31, b, g, o)
            out[:, :, 4 * c:4 * c + nreal, :] = res.transpose(1, 3, 2, 0)[:, :, :nreal, :]
    return out


VARIANT = os.environ.get("LC2D_VARIANT", "v13")


def timing_setup(x, weight):
    """(in_maps, build_fn) for test.py's slope timing."""
    if VARIANT == "v13":
        return _host_prep_v13(x, weight), (lambda n: _build_nc_v13(n))
    if VARIANT == "v12":
        return _host_prep_v12(x, weight), (lambda n: _build_nc_v12(n))
    if VARIANT == "v11":
        return _host_prep_v11(x, weight), (lambda n: _build_nc_v11(n))
    if VARIANT == "v10":
        return _host_prep_v10(x, weight), (lambda n: _build_nc_v10(n))
    raise NotImplementedError(VARIANT)


def kernel(x, weight, _trace=False, _trace_cores=None):
    if VARIANT == "v13":
        in_maps = _host_prep_v13(x, weight)
    elif VARIANT == "v12":
        in_maps = _host_prep_v12(x, weight)
    elif VARIANT == "v11":
        in_maps = _host_prep_v11(x, weight)
    elif VARIANT == "v10":
        in_maps = _host_prep_v10(x, weight)
    elif VARIANT in ("v8", "v9", "v8h", "v9h"):
        in_maps = _host_prep_v89(
            x, weight, three_term=(VARIANT in ("v9", "v9h")),
            npdt=(np.float16 if VARIANT.endswith("h") else ml_dtypes.bfloat16))
    elif VARIANT in V4_CFG:
        in_maps = _host_prep_v4(x, weight, VARIANT)
    elif VARIANT == "v5":
        in_maps = _host_prep_v5(x, weight)
    else:
        in_maps = _host_prep(x, weight)
    if "nc" not in _NC_CACHE:
        if VARIANT == "v13":
            _NC_CACHE["nc"] = _build_nc_v13(1)
        elif VARIANT == "v12":
            _NC_CACHE["nc"] = _build_nc_v12(1)
        elif VARIANT == "v11":
            _NC_CACHE["nc"] = _build_nc_v11(1)
        elif VARIANT == "v10":
            _NC_CACHE["nc"] = _build_nc_v10(1)
        elif VARIANT in ("v8", "v9", "v8h", "v9h"):
            _NC_CACHE["nc"] = _build_nc_v89(
                1, three_term=(VARIANT in ("v9", "v9h")),
                dt=(mybir.dt.float16 if VARIANT.endswith("h") else mybir.dt.bfloat16))
        elif VARIANT in V4_CFG:
            _NC_CACHE["nc"] = _build_nc_v4(1, VARIANT)
        elif VARIANT == "v5":
            _NC_CACHE["nc"] = _build_nc_v5()
        else:
            _NC_CACHE["nc"] = _build_nc(variant=VARIANT)
    nc = _NC_CACHE["nc"]
    res = run_bass_kernel_spmd(
        nc, in_maps, core_ids=list(range(N_CORES)),
        trace=_trace, trace_cores=_trace_cores,
    )
    if VARIANT == "v13":
        out = _assemble_v13(res.results)
    elif VARIANT == "v12":
        out = _assemble_v12(res.results)
    elif VARIANT == "v11":
        out = _assemble_v11(res.results)
    elif VARIANT in ("v8", "v9", "v8h", "v9h", "v10"):
        out = _assemble_v89(res.results)
    elif VARIANT in V4_CFG:
        out = _assemble_v4(res.results, VARIANT)
    elif VARIANT == "v5":
        out = _assemble_v5(res.results)
    else:
        out = _assemble(res.results, variant=VARIANT)
    if _trace:
        return out, res
    return out


if __name__ == "__main__":
    # quick self-check with random data against a numpy oracle
    rng = np.random.default_rng(0)
    x = rng.standard_normal((B, CI, H, W), dtype=np.float32)
    weight = rng.standard_normal((1, CO, CI, HO, WO, KH * KW), dtype=np.float32)
    wins = np.stack(
        [x[:, :, kh:kh + DH * HO:DH, kw:kw + DW * WO:DW]
         for kh in range(KH) for kw in range(KW)], axis=-1)
    expected = np.einsum("bipqk,oipqk->bopq", wins, weight[0], optimize=True)
    actual = kernel(x, weight)
    err = np.abs(actual - expected).max() / np.abs(expected).max()
    print("max out:", np.abs(expected).max(), "rel err:", err)
    tol = 1e-5 if VARIANT in ("v1", "v2", "v5") else (1e-2 if VARIANT in ("v8", "v4b", "v4b8") else 1e-3)
    assert err < tol, (err, tol)
    print("KERNEL OK")



# revision 54
# speedup vs baseline: 2.7931x; 1.0327x over previous
"""LocallyConnected2d Trainium2 kernel (8-core SPMD).

out[b,o,p,q] = sum_{i,kh,kw} x[b, i, 2p+kh, 2q+kw] * weight[0, o, i, p, q, kh*3+kw]

Shipped variant "v13" (~10.6us/iter vs the 24.3us v10 baseline):
- Shard the H' (=31) output-row dim across 8 cores (4 rows/core; core 7
  gets one duplicated padding row so the SPMD program is uniform). This
  splits the dominant traffic — the 35.4MB per-location weight — 8 ways.
- The kernel is a pure fp16 streaming problem (~3MB/core input). Measured
  HW DMA is per-SBUF-partition limited (~2.7GB/s/partition, ~345GB/s/core
  at 128 partitions), so the layout spreads bytes over ALL 128 partitions:
  contraction 288 = A(ik 0:128) + B(128:256) + C(256:288), with C's
  weights packed 4-blocks-per-32-partition-band and C's windows
  zero-padded to 128 rows so the zeros null the other blocks' band rows.
  This keeps every matmul on the SAME (128,32) PE tile config —
  alternating PE tile sizes measured >2x slower on HW.
- Per block of 4 locations: 3 accumulating matmuls (A/B/C),
  lhsT = windows [128, 32], rhs = weight [128, 128] -> psum [32, 128] in a
  rotating PSUM column band (tile_position (0, 32*band)); one [128, 512]
  fp32->fp16 copy per 4 groups (DVE / ACT); single fp16 out DMA.
  Diagonal extraction of the 4x-blocked output happens host-side.
- Exactly 8 DMAs per body so bass's 8 rotating DMA-completion semaphores
  reuse body-aligned (9+ DMAs made an input wait on the previous body's
  output DMA, serializing the stream).
- PE p-state warmup chain (~3us of dummy matmuls at startup) so real
  matmuls run at full clock.
- Timing builds unroll 16 bodies per For_i iteration: the all-engine
  barrier is per-For_i-iter, so consecutive bodies pipeline and the
  steady-state per-body time approaches the DMA stream floor (~9us).
"""

import os
import numpy as np
import ml_dtypes

import concourse.bacc as bacc
import concourse.mybir as mybir
import concourse.tile as tile
from concourse.bass_utils import run_bass_kernel_spmd

# Problem shapes (hardcoded per contract).
B, CI, H, W = 8, 32, 64, 64
CO = 32
KH = KW = 3
DH = DW = 2
HO = WO = 31
N_CORES = 8
RPC = 4                 # padded H'-rows per core
L = RPC * WO            # 124 locations per core
IK = CI * KH * KW       # 288 contraction
NCHUNK = 3
CK = IK // NCHUNK       # 96 partitions per chunk
GROUPS = RPC            # one compute/DMA group per H'-row
GL = L // GROUPS        # 31 locations per group

W_COLS = L * NCHUNK * CO     # 11904
WIN_COLS = L * NCHUNK * B    # 2976
OUT_COLS = L * B             # 992

_ROWS_PADDED = [[min(4 * c + j, HO - 1) for j in range(RPC)] for c in range(N_CORES)]

_NC_CACHE = {}


V2_GOUT = 256               # psum cols per group in v2: 8 col-blocks x 32 (o)
V2_OUT_COLS = V2_GOUT * GROUPS

# v4: blocked matmuls — BLK locations share one matmul (out is a BLK x BLK
# grid of [b, o] tiles; only the diagonal is useful, extracted host-side).
# fp32r needs moving free dim >= 256 for the 1 cycle/row fast path.
GLP = 32                    # padded locs per group (31 real + 1 dup)
V4_CFG = {
    "v4r": (mybir.dt.float32r, 8, np.float32),
    "v4b": (mybir.dt.bfloat16, 4, ml_dtypes.bfloat16),
    "v4b8": (mybir.dt.bfloat16, 8, ml_dtypes.bfloat16),
}


def _build_nc_v4(repeat, variant):
    dt, BLK, _ = V4_CFG[variant]
    NBLK = GLP // BLK
    gw = NCHUNK * GLP * CO   # 3072 weight cols per group
    gwin = NCHUNK * GLP * B  # 768 win cols per group
    bout = BLK * CO          # out cols per block
    orows = B * BLK          # out rows per block
    out_cols = GROUPS * NBLK * bout

    nc = bacc.Bacc("TRN2", target_bir_lowering=False)
    wT = nc.dram_tensor("wT", [GROUPS * CK, gw], dt, kind="ExternalInput")
    winT = nc.dram_tensor("winT", [GROUPS * CK, gwin], dt, kind="ExternalInput")
    out = nc.dram_tensor("out", [orows, out_cols], mybir.dt.float32, kind="ExternalOutput")

    with tile.TileContext(nc) as tc:
        with (
            tc.tile_pool(name="wp", bufs=3) as wp,
            tc.tile_pool(name="winp", bufs=3) as winp,
            tc.tile_pool(name="pp", bufs=4, space="PSUM") as pp,
            tc.tile_pool(name="op", bufs=4) as op,
        ):
            def body():
                for g in range(GROUPS):
                    wt = wp.tile([CK, gw], dt, tag="wt", name="wt")
                    nc.sync.dma_start(wt[:], wT.ap()[g * CK:(g + 1) * CK, :])
                    wint = winp.tile([CK, gwin], dt, tag="wint", name="wint")
                    nc.sync.dma_start(wint[:], winT.ap()[g * CK:(g + 1) * CK, :])

                    for bl in range(NBLK):
                        ps = pp.tile([orows, bout], mybir.dt.float32, tag="ps", name="ps")
                        for c in range(NCHUNK):
                            nc.tensor.matmul(
                                ps[:],
                                lhsT=wint[:, c * (GLP * B) + bl * (BLK * B):
                                          c * (GLP * B) + (bl + 1) * (BLK * B)],
                                rhs=wt[:, c * (GLP * CO) + bl * bout:
                                       c * (GLP * CO) + (bl + 1) * bout],
                                start=(c == 0),
                                stop=(c == NCHUNK - 1),
                            )
                        ot = op.tile([orows, bout], mybir.dt.float32, tag="ot", name="ot")
                        nc.vector.tensor_copy(ot[:], ps[:])
                        nc.sync.dma_start(
                            out.ap()[:, (g * NBLK + bl) * bout:(g * NBLK + bl + 1) * bout],
                            ot[:],
                        )

            if repeat == 1:
                body()
            else:
                with tc.For_i(0, repeat, 1):
                    body()
    nc.compile()
    return nc


def _build_nc_v5(repeat=1):
    """fp32 exact; all DMAs 128-partition; contraction 128+128+32 with the
    32-row remainder of all 4 groups packed into one 128-row tile."""
    gw = GL * CO     # 992 weight cols per (group, chunk)
    gwin = GL * B    # 248 win cols per (group, chunk)
    nc = bacc.Bacc("TRN2", target_bir_lowering=False)
    w01 = nc.dram_tensor("w01", [GROUPS * 2 * 128, gw], mybir.dt.float32, kind="ExternalInput")
    win01 = nc.dram_tensor("win01", [GROUPS * 2 * 128, gwin], mybir.dt.float32, kind="ExternalInput")
    w2 = nc.dram_tensor("w2", [GROUPS * 32, gw], mybir.dt.float32, kind="ExternalInput")
    win2 = nc.dram_tensor("win2", [GROUPS * 32, gwin], mybir.dt.float32, kind="ExternalInput")
    out = nc.dram_tensor("out", [GROUPS * 128, V2_GOUT], mybir.dt.float32, kind="ExternalOutput")

    with tile.TileContext(nc) as tc:
        with (
            tc.tile_pool(name="wp", bufs=3) as wp,
            tc.tile_pool(name="winp", bufs=3) as winp,
            tc.tile_pool(name="pp", bufs=2, space="PSUM") as pp,
            tc.tile_pool(name="op", bufs=2) as op,
        ):
            def body():
                for g in range(GROUPS):
                    wts, wints = [], []
                    for cc in range(2):
                        wt = wp.tile([128, gw], mybir.dt.float32, tag=f"wt{cc}", name=f"wt{cc}")
                        nc.sync.dma_start(
                            wt[:], w01.ap()[(g * 2 + cc) * 128:(g * 2 + cc + 1) * 128, :])
                        wint = winp.tile([128, gwin], mybir.dt.float32, tag=f"wint{cc}", name=f"wint{cc}")
                        nc.sync.dma_start(
                            wint[:], win01.ap()[(g * 2 + cc) * 128:(g * 2 + cc + 1) * 128, :])
                        wts.append(wt)
                        wints.append(wint)
                    w2t = wp.tile([32, gw], mybir.dt.float32, tag="w2t", name="w2t")
                    nc.sync.dma_start(w2t[:], w2.ap()[g * 32:(g + 1) * 32, :])
                    win2t = winp.tile([32, gwin], mybir.dt.float32, tag="win2t", name="win2t")
                    nc.sync.dma_start(win2t[:], win2.ap()[g * 32:(g + 1) * 32, :])

                    pss = [
                        pp.tile([128, V2_GOUT], mybir.dt.float32,
                                tag=f"ps{j}", name=f"ps{j}", bufs=2)
                        for j in range(4)
                    ]
                    for l in range(GL):
                        j = l % 4
                        blk = l // 4
                        dst = pss[j][32 * j:32 * j + B, blk * CO:(blk + 1) * CO]
                        for cc in range(2):
                            nc.tensor.matmul(
                                dst,
                                lhsT=wints[cc][:, l * B:(l + 1) * B],
                                rhs=wts[cc][:, l * CO:(l + 1) * CO],
                                start=(cc == 0),
                                stop=False,
                                tile_position=(0, 32 * j),
                            )
                        nc.tensor.matmul(
                            dst,
                            lhsT=win2t[:, l * B:(l + 1) * B],
                            rhs=w2t[:, l * CO:(l + 1) * CO],
                            start=False,
                            stop=True,
                            tile_position=(0, 32 * j),
                        )

                    ot = op.tile([128, V2_GOUT], mybir.dt.float32, tag="ot", name="ot")
                    for j in range(4):
                        nc.vector.tensor_copy(
                            ot[32 * j:32 * (j + 1), :],
                            pss[j][32 * j:32 * (j + 1), :],
                        )
                    nc.sync.dma_start(out.ap()[g * 128:(g + 1) * 128, :], ot[:])

            if repeat == 1:
                body()
            else:
                with tc.For_i(0, repeat, 1):
                    body()
    nc.compile()
    return nc


def _host_prep_v5(x, weight):
    x = np.ascontiguousarray(np.asarray(x, dtype=np.float32))
    weight = np.ascontiguousarray(np.asarray(weight, dtype=np.float32))
    wins = np.stack(
        [x[:, :, kh:kh + DH * HO:DH, kw:kw + DW * WO:DW]
         for kh in range(KH) for kw in range(KW)],
        axis=-1,
    )
    W2 = weight[0].transpose(1, 4, 2, 3, 0).reshape(IK, HO, WO, CO)
    W3 = wins.transpose(1, 4, 2, 3, 0).reshape(IK, HO, WO, B)
    in_maps = []
    for c in range(N_CORES):
        rows = _ROWS_PADDED[c]
        wsel = W2[:, rows]       # (288, 4, 31, CO)
        winsel = W3[:, rows]     # (288, 4, 31, B)
        # w01 rows: (g, c01, 128) ; cols (l, o)
        w01 = wsel[:256].reshape(2, 128, GROUPS, GL * CO).transpose(2, 0, 1, 3)
        win01 = winsel[:256].reshape(2, 128, GROUPS, GL * B).transpose(2, 0, 1, 3)
        w2 = wsel[256:].reshape(32, GROUPS, GL * CO).transpose(1, 0, 2)
        win2 = winsel[256:].reshape(32, GROUPS, GL * B).transpose(1, 0, 2)
        in_maps.append({
            "w01": np.ascontiguousarray(w01.reshape(GROUPS * 2 * 128, GL * CO)),
            "win01": np.ascontiguousarray(win01.reshape(GROUPS * 2 * 128, GL * B)),
            "w2": np.ascontiguousarray(w2.reshape(GROUPS * 32, GL * CO)),
            "win2": np.ascontiguousarray(win2.reshape(GROUPS * 32, GL * B)),
        })
    return in_maps


def _assemble_v5(results):
    out = np.empty((B, CO, HO, WO), np.float32)
    qs = np.arange(WO)
    for c in range(N_CORES):
        nreal = RPC if c < N_CORES - 1 else HO - 4 * (N_CORES - 1)
        buf = np.asarray(results[c]["out"])      # [GROUPS*128, 256]
        b5 = buf.reshape(GROUPS, 4, 32, 8, CO)   # (g, strip, 32row, blk, o)
        res = b5[:, qs % 4, :B, qs // 4, :]      # (g?, ...) advanced idx
        # advanced indices qs%4 (dim1) and qs//4 (dim3) -> (31, GROUPS, B, CO)
        out[:, :, 4 * c:4 * c + nreal, :] = res.transpose(2, 3, 1, 0)[:, :, :nreal, :]
    return out


V89_BLK = 4
V89_NBLK = GLP // V89_BLK            # 8 blocks of 4 locs per group
V89_GW = NCHUNK * GLP * CO           # 3072 weight cols per group
V89_GWIN = NCHUNK * GLP * B          # 768 win cols per group
V89_BOUT = V89_BLK * CO              # 128 out cols per block
V89_OROWS = B * V89_BLK              # 32 out rows
V89_OUTC = GROUPS * V89_NBLK * V89_BOUT  # 4096


def _build_nc_v89(repeat=1, three_term=False, dt=None):
    """16-bit blocked kernel, minimal DMA count, split across both HWDGE
    rings. three_term=True computes w≈wh+wl, win≈vh+vl and accumulates
    vh·wh + vh·wl + vl·wh (16-bit products are exact in fp32 -> ~1e-5 rel err).
    """
    if dt is None:
        dt = mybir.dt.bfloat16
    W = GROUPS * V89_GW
    WIN = GROUPS * V89_GWIN
    nc = bacc.Bacc("TRN2", target_bir_lowering=False)
    wh_d = nc.dram_tensor("wh", [CK, W], dt, kind="ExternalInput")
    winh_d = nc.dram_tensor("winh", [CK, WIN], dt, kind="ExternalInput")
    if three_term:
        wl_d = nc.dram_tensor("wl", [CK, W], dt, kind="ExternalInput")
        winl_d = nc.dram_tensor("winl", [CK, WIN], dt, kind="ExternalInput")
    out = nc.dram_tensor("out", [V89_OROWS, V89_OUTC], mybir.dt.float32, kind="ExternalOutput")

    half = W // 2  # 2 groups per ring half
    with tile.TileContext(nc) as tc:
        with (
            tc.tile_pool(name="wp", bufs=2) as wp,
            tc.tile_pool(name="winp", bufs=2) as winp,
            tc.tile_pool(name="pp", bufs=4, space="PSUM") as pp,
            tc.tile_pool(name="op", bufs=2) as op,
        ):
            def body():
                # weight: groups 0-1 via SP ring, groups 2-3 via ACT ring,
                # one piece per group -> compute starts after 1/4 of bytes
                wh = wp.tile([CK, W], dt, tag="wh", name="wh")
                for g in range(2):
                    nc.sync.dma_start(
                        wh[:, g * V89_GW:(g + 1) * V89_GW],
                        wh_d.ap()[:, g * V89_GW:(g + 1) * V89_GW])
                for g in range(2, 4):
                    nc.scalar.dma_start(
                        wh[:, g * V89_GW:(g + 1) * V89_GW],
                        wh_d.ap()[:, g * V89_GW:(g + 1) * V89_GW])
                winh = winp.tile([CK, WIN], dt, tag="winh", name="winh")
                nc.sync.dma_start(winh[:, :WIN // 2], winh_d.ap()[:, :WIN // 2])
                nc.scalar.dma_start(winh[:, WIN // 2:], winh_d.ap()[:, WIN // 2:])
                if three_term:
                    wl = wp.tile([CK, W], dt, tag="wl", name="wl")
                    for g in range(2):
                        nc.scalar.dma_start(
                            wl[:, g * V89_GW:(g + 1) * V89_GW],
                            wl_d.ap()[:, g * V89_GW:(g + 1) * V89_GW])
                    for g in range(2, 4):
                        nc.sync.dma_start(
                            wl[:, g * V89_GW:(g + 1) * V89_GW],
                            wl_d.ap()[:, g * V89_GW:(g + 1) * V89_GW])
                    winl = winp.tile([CK, WIN], dt, tag="winl", name="winl")
                    nc.scalar.dma_start(winl[:, :WIN // 2], winl_d.ap()[:, :WIN // 2])
                    nc.sync.dma_start(winl[:, WIN // 2:], winl_d.ap()[:, WIN // 2:])

                ot = op.tile([V89_OROWS, V89_OUTC], mybir.dt.float32, tag="ot", name="ot")
                for g in range(GROUPS):
                    for bl in range(V89_NBLK):
                        ps = pp.tile([V89_OROWS, V89_BOUT], mybir.dt.float32, tag="ps", name="ps")
                        first = True
                        for c in range(NCHUNK):
                            lo = g * V89_GWIN + c * (GLP * B) + bl * (V89_BLK * B)
                            ro = g * V89_GW + c * (GLP * CO) + bl * V89_BOUT
                            lhs_h = winh[:, lo:lo + V89_BLK * B]
                            rhs_h = wh[:, ro:ro + V89_BOUT]
                            terms = [(lhs_h, rhs_h)]
                            if three_term:
                                terms.append((lhs_h, wl[:, ro:ro + V89_BOUT]))
                                terms.append((winl[:, lo:lo + V89_BLK * B], rhs_h))
                            for ti, (lh, rh) in enumerate(terms):
                                last = (c == NCHUNK - 1) and (ti == len(terms) - 1)
                                nc.tensor.matmul(
                                    ps[:], lhsT=lh, rhs=rh,
                                    start=first, stop=last)
                                first = False
                        nc.vector.tensor_copy(
                            ot[:, (g * V89_NBLK + bl) * V89_BOUT:(g * V89_NBLK + bl + 1) * V89_BOUT],
                            ps[:])
                nc.gpsimd.dma_start(out.ap()[:, :], ot[:])

            if repeat == 1:
                body()
            else:
                with tc.For_i(0, repeat, 1):
                    body()
    nc.compile()
    return nc


def _host_prep_v89(x, weight, three_term=False, npdt=None):
    if npdt is None:
        npdt = ml_dtypes.bfloat16
    x = np.ascontiguousarray(np.asarray(x, dtype=np.float32))
    weight = np.ascontiguousarray(np.asarray(weight, dtype=np.float32))
    wins = np.stack(
        [x[:, :, kh:kh + DH * HO:DH, kw:kw + DW * WO:DW]
         for kh in range(KH) for kw in range(KW)],
        axis=-1,
    )
    W2 = weight[0].transpose(1, 4, 2, 3, 0).reshape(IK, HO, WO, CO)
    W3 = wins.transpose(1, 4, 2, 3, 0).reshape(IK, HO, WO, B)
    qpad = list(range(WO)) + [WO - 1]
    in_maps = []
    for c in range(N_CORES):
        rows = _ROWS_PADDED[c]
        wsel = W2[:, rows][:, :, qpad, :]       # (288, 4, 32, CO)
        winsel = W3[:, rows][:, :, qpad, :]     # (288, 4, 32, B)
        # -> [CK, (group, chunk, locp, {o|b})]
        wstk = np.stack([wsel[CK * cc:CK * (cc + 1)] for cc in range(NCHUNK)], axis=2)
        winstk = np.stack([winsel[CK * cc:CK * (cc + 1)] for cc in range(NCHUNK)], axis=2)
        # (CK, 4, chunk, 32, X) -> (CK, group*chunk*locp*X)
        wfull = wstk.reshape(CK, GROUPS * NCHUNK * GLP * CO)
        winfull = winstk.reshape(CK, GROUPS * NCHUNK * GLP * B)
        m = {}
        wh = wfull.astype(npdt)
        vh = winfull.astype(npdt)
        m["wh"] = np.ascontiguousarray(wh)
        m["winh"] = np.ascontiguousarray(vh)
        if three_term:
            m["wl"] = np.ascontiguousarray(
                (wfull - wh.astype(np.float32)).astype(npdt))
            m["winl"] = np.ascontiguousarray(
                (winfull - vh.astype(np.float32)).astype(npdt))
        in_maps.append(m)
    return in_maps


def _assemble_v89(results):
    BLK = V89_BLK
    NBLK = V89_NBLK
    out = np.empty((B, CO, HO, WO), np.float32)
    idx = np.arange(BLK)
    for c in range(N_CORES):
        nreal = RPC if c < N_CORES - 1 else HO - 4 * (N_CORES - 1)
        buf = np.asarray(results[c]["out"])          # [32, 4096]
        b6 = buf.reshape(BLK, B, GROUPS, NBLK, BLK, CO)
        d = b6[idx, :, :, :, idx, :]                 # (BLK, B, G, NBLK, CO)
        dd = d.transpose(1, 4, 2, 3, 0).reshape(B, CO, GROUPS, NBLK * BLK)
        out[:, :, 4 * c:4 * c + nreal, :] = dd[:, :, :nreal, :WO]
    return out


V11_NP = 8                      # pieces (half H'-rows) per core
V11_PL = 16                     # padded locations per piece
V11_NBLK = 4                    # blocks of BLK=4 locs per piece
V11_GW = NCHUNK * V11_PL * CO   # 1536 weight cols per piece
V11_GWIN = NCHUNK * V11_PL * B  # 384 win cols per piece
V11_GTOT = V11_GW + V11_GWIN    # 1920
V11_POUT = V11_NBLK * V89_BLK * CO  # 512 out cols per piece
V11_OUTC = V11_NP * V11_POUT    # 4096


def _build_nc_v11(repeat=1, dt=None):
    """8 self-contained pieces (16 locs each), one input DMA per piece on the
    SP queue; matmuls accumulate into a [32, 512] PSUM tile per piece (one
    bank); output DMA'd straight from PSUM on the Pool/SWDGE queue — no
    PSUM->SBUF copies at all."""
    if dt is None:
        dt = mybir.dt.float16
    BLK = V89_BLK
    orows = V89_OROWS           # 32 = BLK * B
    nc = bacc.Bacc("TRN2", target_bir_lowering=False)
    wx = nc.dram_tensor("wx", [CK, V11_NP * V11_GTOT], dt, kind="ExternalInput")
    out = nc.dram_tensor("out", [orows, V11_OUTC], mybir.dt.float32, kind="ExternalOutput")
    with tile.TileContext(nc) as tc:
        with (
            tc.tile_pool(name="wp", bufs=2) as wp,
            tc.tile_pool(name="pp", bufs=4, space="PSUM") as pp,
        ):
            def body():
                t = wp.tile([CK, V11_NP * V11_GTOT], dt, tag="t", name="t")
                for p in range(V11_NP):
                    nc.sync.dma_start(t[:, p * V11_GTOT:(p + 1) * V11_GTOT],
                                      wx.ap()[:, p * V11_GTOT:(p + 1) * V11_GTOT])
                for p in range(V11_NP):
                    base = p * V11_GTOT
                    ps = pp.tile([orows, V11_POUT], mybir.dt.float32, tag="ps", name="ps")
                    for bl in range(V11_NBLK):
                        for c in range(NCHUNK):
                            lo = base + V11_GW + c * (V11_PL * B) + bl * (BLK * B)
                            ro = base + c * (V11_PL * CO) + bl * (BLK * CO)
                            nc.tensor.matmul(
                                ps[:, bl * (BLK * CO):(bl + 1) * (BLK * CO)],
                                lhsT=t[:, lo:lo + BLK * B],
                                rhs=t[:, ro:ro + BLK * CO],
                                start=(c == 0), stop=(c == NCHUNK - 1))
                    nc.gpsimd.dma_start(
                        out.ap()[:, p * V11_POUT:(p + 1) * V11_POUT], ps[:])
            if repeat == 1:
                body()
            else:
                with tc.For_i(0, repeat, 1):
                    body()
    nc.compile()
    return nc


def _host_prep_v11(x, weight, npdt=None):
    if npdt is None:
        npdt = np.float16
    m = _host_prep_v89(x, weight, three_term=False, npdt=npdt)
    out_maps = []
    for mm in m:
        # wh: (CK, group4, chunk3, locp32, CO) ; winh: (..., B)
        wh = mm["wh"].reshape(CK, GROUPS, NCHUNK, GLP, CO)
        vh = mm["winh"].reshape(CK, GROUPS, NCHUNK, GLP, B)
        # -> (CK, row4, half2, chunk3, loc16, X)
        wp = wh.reshape(CK, GROUPS, NCHUNK, 2, V11_PL, CO).transpose(0, 1, 3, 2, 4, 5)
        vp = vh.reshape(CK, GROUPS, NCHUNK, 2, V11_PL, B).transpose(0, 1, 3, 2, 4, 5)
        wp = wp.reshape(CK, V11_NP, V11_GW)
        vp = vp.reshape(CK, V11_NP, V11_GWIN)
        wx = np.concatenate([wp, vp], axis=2).reshape(CK, V11_NP * V11_GTOT)
        out_maps.append({"wx": np.ascontiguousarray(wx)})
    return out_maps


def _assemble_v11(results):
    BLK = V89_BLK
    out = np.empty((B, CO, HO, WO), np.float32)
    idx = np.arange(BLK)
    for c in range(N_CORES):
        nreal = RPC if c < N_CORES - 1 else HO - 4 * (N_CORES - 1)
        buf = np.asarray(results[c]["out"])          # [32, 4096]
        b6 = buf.reshape(BLK, B, V11_NP, V11_NBLK, BLK, CO)
        d = b6[idx, :, :, :, idx, :]                 # (BLK, B, P, NBLK, CO)
        dd = d.transpose(1, 4, 2, 3, 0).reshape(B, CO, GROUPS, 2 * V11_NBLK * BLK)
        out[:, :, 4 * c:4 * c + nreal, :] = dd[:, :, :nreal, :WO]
    return out


V12_NBLKS = 32                  # blocks of BLK=4 locs per core (4 rows x 8)
V12_UNIT = V89_BLK * (B + CO)   # 160 cols per (block, chunk) unit: win | w
V12_BCOLS = NCHUNK * V12_UNIT   # 480 cols per block
V12_PIECES = (5, 5, 5, 5, 4, 4, 4)  # input DMA piece sizes in blocks (sum 32)
V12_OUTC = V12_NBLKS * V89_BLK * CO  # 4096 fp16 out cols (blocked, diag on host)
V12_NGRP = 8                         # compute groups of 4 blocks (1 PSUM bank)


def _build_nc_v12(repeat=1, dt=None, flat=False):
    """Block-major stream: 9 input pieces on SP (tiny last piece to shrink
    the tail), matmuls accumulate in per-piece PSUM banks (all 8), one
    contiguous fp32->fp16 PSUM->SBUF copy per piece round-robin over
    DVE/ACT/Pool, 2 output DMAs; diagonal extraction happens host-side.
    repeat>1 timing builds unroll 8 bodies per For_i iteration so
    consecutive bodies pipeline (the all-engine barrier is per-For_i-iter)."""
    if dt is None:
        dt = mybir.dt.float16
    BLK = V89_BLK
    nc = bacc.Bacc("TRN2", target_bir_lowering=False)
    wx = nc.dram_tensor("wx", [CK, V12_NBLKS * V12_BCOLS], dt, kind="ExternalInput")
    out = nc.dram_tensor("out", [32, V12_OUTC], mybir.dt.float16, kind="ExternalOutput")
    starts = []
    s = 0
    for n in V12_PIECES:
        starts.append(s)
        s += n
    assert s == V12_NBLKS
    BC = BLK * CO
    with tile.TileContext(nc) as tc:
        with (
            tc.tile_pool(name="wp", bufs=2) as wp,
            tc.tile_pool(name="pp", bufs=7, space="PSUM") as pp,
            tc.tile_pool(name="op", bufs=2) as op,
        ):
            def body():
                t = wp.tile([CK, V12_NBLKS * V12_BCOLS], dt, tag="t", name="t")
                for s, n in zip(starts, V12_PIECES):
                    nc.sync.dma_start(
                        t[:, s * V12_BCOLS:(s + n) * V12_BCOLS],
                        wx.ap()[:, s * V12_BCOLS:(s + n) * V12_BCOLS])
                # PE p-state warmup: ~3.4us of dummy matmuls on a memset tile
                # so real matmuls run at full clock once piece 0 lands.
                wu = wp.tile([CK, 256], dt, tag="wu", name="wu")
                nc.vector.memset(wu[:], 0.0)
                psw = pp.tile([32, 512], mybir.dt.float32, tag="psw", name="psw",
                              bufs=1)
                for i in range(12):
                    nc.tensor.matmul(psw[0:8, 0:256], lhsT=wu[:, :8], rhs=wu[:, :256],
                                     start=(i == 0), stop=(i == 11))
                for i in range(10):
                    nc.tensor.matmul(psw[0:8, 256:320], lhsT=wu[:, :8], rhs=wu[:, :64],
                                     start=(i == 0), stop=(i == 9))
                ot = op.tile([32, V12_OUTC], mybir.dt.float16, tag="ot", name="ot")
                for g in range(V12_NGRP):
                    ps = pp.tile([32, 4 * BC], mybir.dt.float32,
                                 tag="ps", name="ps")
                    for li in range(4):
                        u0 = (4 * g + li) * NCHUNK
                        for c in range(NCHUNK):
                            off = (u0 + c) * V12_UNIT
                            nc.tensor.matmul(
                                ps[:, li * BC:(li + 1) * BC],
                                lhsT=t[:, off:off + BLK * B],
                                rhs=t[:, off + BLK * B:off + V12_UNIT],
                                start=(c == 0), stop=(c == NCHUNK - 1))
                    dst = ot[:, g * 4 * BC:(g + 1) * 4 * BC]
                    if g % 2 == 0:
                        nc.vector.tensor_copy(dst, ps[:])
                    else:
                        nc.scalar.copy(dst, ps[:])
                nc.scalar.dma_start(out.ap()[:, :], ot[:, :])
            if repeat == 1:
                body()
            elif flat:
                for _ in range(repeat):
                    body()
            else:
                u = 8 if repeat % 8 == 0 else (4 if repeat % 4 == 0 else 1)
                with tc.For_i(0, repeat // u, 1):
                    for _ in range(u):
                        body()
    nc.compile()
    return nc


def _host_prep_v12(x, weight, npdt=None):
    if npdt is None:
        npdt = np.float16
    m = _host_prep_v89(x, weight, three_term=False, npdt=npdt)
    out_maps = []
    for mm in m:
        # wh: (CK, group4, chunk3, locp32, CO); winh: (..., B)
        wh = mm["wh"].reshape(CK, GROUPS, NCHUNK, GLP, CO)
        vh = mm["winh"].reshape(CK, GROUPS, NCHUNK, GLP, B)
        # -> (CK, block(row,bb)=32, chunk3, BLK locs, X) with loc = blk*4+j
        wb = wh.reshape(CK, GROUPS, NCHUNK, 8, V89_BLK, CO)
        vb = vh.reshape(CK, GROUPS, NCHUNK, 8, V89_BLK, B)
        wb = wb.transpose(0, 1, 3, 2, 4, 5).reshape(CK, V12_NBLKS, NCHUNK, V89_BLK * CO)
        vb = vb.transpose(0, 1, 3, 2, 4, 5).reshape(CK, V12_NBLKS, NCHUNK, V89_BLK * B)
        wx = np.concatenate([vb, wb], axis=3)  # (CK, blk, chunk, 32+128)
        out_maps.append({"wx": np.ascontiguousarray(
            wx.reshape(CK, V12_NBLKS * V12_BCOLS))})
    return out_maps


def _assemble_v12(results):
    BLK = V89_BLK
    out = np.empty((B, CO, HO, WO), np.float32)
    qs = np.arange(WO)
    idx = np.arange(BLK)
    for c in range(N_CORES):
        nreal = RPC if c < N_CORES - 1 else HO - 4 * (N_CORES - 1)
        buf = np.asarray(results[c]["out"]).astype(np.float32)  # [32, 4096]
        b6 = buf.reshape(BLK, B, GROUPS, 8, BLK, CO)  # (j, b, row, bb, j', o)
        d = b6[idx, :, :, :, idx, :]                  # (BLK, B, row, bb, o)
        res = d[qs % 4, :, :, qs // 4, :]             # (31, b, row, o)
        out[:, :, 4 * c:4 * c + nreal, :] = res.transpose(1, 3, 2, 0)[:, :, :nreal, :]
    return out


V13_UNIT = V89_BLK * (B + CO)       # 160 cols per (block, chunk) unit
V13_CSEC = 4 * 32 + 128             # C section: 4 zero-padded win blocks + packed w band
V13_GRP = 4 * 2 * V13_UNIT + V13_CSEC  # 1536 cols per 4-block group
V13_NGRP = 8
V13_OUTC = 1024                     # out [128, 1024] fp16 (blocked, diag on host)


def _build_nc_v13(repeat=1, dt=None, flat=False, dma_only=False, u=16, tbufs=2,
                  out_q="act", split_out=True):
    """128-partition DMA layout: contraction 288 = A(0:128) + B(128:256) on all
    partitions + C(256:288) packed 4-blocks-per-band; the C *window* operand is
    zero-padded to 128 rows so every matmul uses the same (128,32) PE tile
    config (alternating PE tile sizes measured ~2x slower on HW), and the
    zeros null out the other blocks' rows of the packed C weight band.
    Matmul outputs land in rotating PSUM col bands (32*(g%4)); one [128,512]
    copy per 4 groups (DVE then ACT); out [128,1024] fp16. 8 DMAs/body so the
    8 rotating DMA-completion semaphores stay body-aligned; repeat>1 unrolls
    8 bodies per For_i iteration."""
    if dt is None:
        dt = mybir.dt.float16
    BLK = V89_BLK
    nc = bacc.Bacc("TRN2", target_bir_lowering=False)
    wx = nc.dram_tensor("wx", [128, V13_NGRP * V13_GRP], dt, kind="ExternalInput")
    out = nc.dram_tensor("out", [128, V13_OUTC], mybir.dt.float16, kind="ExternalOutput")
    with tile.TileContext(nc) as tc:
        with (
            tc.tile_pool(name="wp", bufs=2) as wp,
            tc.tile_pool(name="pp", bufs=2, space="PSUM") as pp,
            tc.tile_pool(name="op", bufs=2) as op,
        ):
            def body():
                t = wp.tile([128, V13_NGRP * V13_GRP], dt, tag="t", name="t",
                            bufs=tbufs)
                # input DMAs: piece 0 covers groups 0-1, rest one group each
                # (6 coarser pieces when the output is split into 2 DMAs, so
                # the body stays at exactly 8 DMAs total)
                bounds = [0, 2, 4, 5, 6, 7, 8] if split_out else \
                    [0, 2, 3, 4, 5, 6, 7, 8]
                for i in range(len(bounds) - 1):
                    lo, hi = bounds[i] * V13_GRP, bounds[i + 1] * V13_GRP
                    nc.sync.dma_start(t[:, lo:hi], wx.ap()[:, lo:hi])
                if dma_only:
                    return
                wu = wp.tile([128, 256], dt, tag="wu", name="wu")
                nc.vector.memset(wu[:], 0.0)
                psw = pp.tile([32, 512], mybir.dt.float32, tag="psw", name="psw",
                              bufs=1)
                for i in range(12):
                    nc.tensor.matmul(psw[0:8, 0:256], lhsT=wu[:, :8], rhs=wu[:, :256],
                                     start=(i == 0), stop=(i == 11))
                for i in range(10):
                    nc.tensor.matmul(psw[0:8, 256:320], lhsT=wu[:, :8], rhs=wu[:, :64],
                                     start=(i == 0), stop=(i == 9))
                ot = op.tile([128, V13_OUTC], mybir.dt.float16, tag="ot", name="ot")
                for h in range(2):
                    psf = pp.tile([128, 512], mybir.dt.float32, tag="ps", name="ps",
                                  bufs=3)
                    for bi in range(4):
                        g = 4 * h + bi
                        base = g * V13_GRP
                        for li in range(4):
                            ab = base + li * (2 * V13_UNIT)
                            cs = base + 8 * V13_UNIT
                            dst = psf[32 * bi:32 * (bi + 1), li * 128:(li + 1) * 128]
                            nc.tensor.matmul(
                                dst, lhsT=t[:, ab:ab + 32],
                                rhs=t[:, ab + 32:ab + V13_UNIT],
                                start=True, stop=False,
                                tile_position=(0, 32 * bi))
                            nc.tensor.matmul(
                                dst, lhsT=t[:, ab + V13_UNIT:ab + V13_UNIT + 32],
                                rhs=t[:, ab + V13_UNIT + 32:ab + 2 * V13_UNIT],
                                start=False, stop=False,
                                tile_position=(0, 32 * bi))
                            nc.tensor.matmul(
                                dst,
                                lhsT=t[:, cs + 32 * li:cs + 32 * (li + 1)],
                                rhs=t[:, cs + 128:cs + 256],
                                start=False, stop=True,
                                tile_position=(0, 32 * bi))
                    dst = ot[:, h * 512:(h + 1) * 512]
                    if h == 0:
                        nc.vector.tensor_copy(dst, psf[:])
                    else:
                        nc.scalar.copy(dst, psf[:])
                    if split_out and h == 0:
                        nc.scalar.dma_start(out.ap()[:, :512], ot[:, :512])
                oq = nc.gpsimd if out_q == "pool" else nc.scalar
                if split_out:
                    oq.dma_start(out.ap()[:, 512:], ot[:, 512:])
                else:
                    oq.dma_start(out.ap()[:, :], ot[:, :])
            if repeat == 1:
                body()
            elif flat:
                for _ in range(repeat):
                    body()
            else:
                while repeat % u:
                    u //= 2
                with tc.For_i(0, repeat // u, 1):
                    for _ in range(u):
                        body()
    nc.compile()
    return nc


def _host_prep_v13(x, weight, npdt=None):
    if npdt is None:
        npdt = np.float16
    x = np.ascontiguousarray(np.asarray(x, dtype=np.float32))
    weight = np.ascontiguousarray(np.asarray(weight, dtype=np.float32))
    wins = np.stack(
        [x[:, :, kh:kh + DH * HO:DH, kw:kw + DW * WO:DW]
         for kh in range(KH) for kw in range(KW)],
        axis=-1,
    )
    W2 = weight[0].transpose(1, 4, 2, 3, 0).reshape(IK, HO, WO, CO)
    W3 = wins.transpose(1, 4, 2, 3, 0).reshape(IK, HO, WO, B)
    qpad = list(range(WO)) + [WO - 1]
    in_maps = []
    for c in range(N_CORES):
        rows = _ROWS_PADDED[c]
        wsel = W2[:, rows][:, :, qpad, :].astype(npdt)    # (288, 4, 32, CO)
        winsel = W3[:, rows][:, :, qpad, :].astype(npdt)  # (288, 4, 32, B)
        wx = np.zeros((128, V13_NGRP * V13_GRP), npdt)
        for g in range(V13_NGRP):
            row, half = g // 2, g % 2
            base = g * V13_GRP
            for li in range(4):
                bb = 4 * half + li
                ls = slice(bb * 4, (bb + 1) * 4)     # 4 locs of this block
                ab = base + li * (2 * V13_UNIT)
                # A unit: ik 0:128
                wx[:, ab:ab + 32] = winsel[0:128, row, ls, :].reshape(128, 32)
                wx[:, ab + 32:ab + V13_UNIT] = wsel[0:128, row, ls, :].reshape(128, 128)
                # B unit: ik 128:256
                wx[:, ab + V13_UNIT:ab + V13_UNIT + 32] = \
                    winsel[128:256, row, ls, :].reshape(128, 32)
                wx[:, ab + V13_UNIT + 32:ab + 2 * V13_UNIT] = \
                    wsel[128:256, row, ls, :].reshape(128, 128)
                # C section: ik 256:288. win zero-padded to 128 rows (band
                # 32*li holds block li's win); w packed 4-blocks-per-band.
                cs = base + 8 * V13_UNIT
                wx[32 * li:32 * (li + 1), cs + 32 * li:cs + 32 * (li + 1)] = \
                    winsel[256:288, row, ls, :].reshape(32, 32)
                wx[32 * li:32 * (li + 1), cs + 128:cs + 256] = \
                    wsel[256:288, row, ls, :].reshape(32, 128)
        in_maps.append({"wx": np.ascontiguousarray(wx)})
    return in_maps


def _assemble_v13(results):
    BLK = V89_BLK
    out = np.empty((B, CO, HO, WO), np.float32)
    for c in range(N_CORES):
        nreal = RPC if c < N_CORES - 1 else HO - 4 * (N_CORES - 1)
        buf = np.asarray(results[c]["out"]).astype(np.float32)  # [128, 1024]
        idx = np.arange(BLK)
        b6 = buf.reshape(4, BLK, B, 2, 4, BLK, CO)  # (band, j, b, h, li, j', o)
        d = b6[:, idx, :, :, :, idx, :]             # (j, band, b, h, li, o)
        for g in range(V13_NGRP):
            row, half = g // 2, g % 2
            if row >= nreal:
                continue
            blkq = d[:, g % 4, :, g // 4, :, :]     # (j, b, li, o)
            q0 = 16 * half
            arr = blkq.transpose(1, 3, 2, 0).reshape(B, CO, 16)  # (b, o, li*4+j)
            qs = np.arange(q0, q0 + 16)
            sel = qs < WO
            out[:, :, 4 * c + row, qs[sel]] = arr[:, :, sel]
    return out


V10_GTOT = NCHUNK * GLP * CO + NCHUNK * GLP * B   # 3840 cols/group: weight | windows


def _build_nc_v10(repeat=1, dt=None):
    """Like v8h but weight+windows interleaved per group in ONE DRAM tensor:
    one DMA per group (4 input DMAs total) — each dma_start costs ~1.5us of
    serialized ring time here, so DMA count is the dominant knob."""
    if dt is None:
        dt = mybir.dt.float16
    BLK = V89_BLK
    NBLK = V89_NBLK
    gw = V89_GW
    gtot = V10_GTOT
    bout = V89_BOUT
    orows = V89_OROWS
    nc = bacc.Bacc("TRN2", target_bir_lowering=False)
    wx = nc.dram_tensor("wx", [CK, GROUPS * gtot], dt, kind="ExternalInput")
    out = nc.dram_tensor("out", [orows, V89_OUTC], mybir.dt.float32, kind="ExternalOutput")
    with tile.TileContext(nc) as tc:
        with (
            tc.tile_pool(name="wp", bufs=2) as wp,
            tc.tile_pool(name="pp", bufs=4, space="PSUM") as pp,
            tc.tile_pool(name="op", bufs=2) as op,
        ):
            def body():
                t = wp.tile([CK, GROUPS * gtot], dt, tag="t", name="t")
                for g in range(GROUPS):
                    nc.sync.dma_start(t[:, g * gtot:(g + 1) * gtot],
                                      wx.ap()[:, g * gtot:(g + 1) * gtot])
                ot = op.tile([orows, V89_OUTC], mybir.dt.float32, tag="ot", name="ot")
                gout = NBLK * bout
                for g in range(GROUPS):
                    base = g * gtot
                    for bl in range(NBLK):
                        ps = pp.tile([orows, bout], mybir.dt.float32, tag="ps", name="ps")
                        for c in range(NCHUNK):
                            lo = base + gw + c * (GLP * B) + bl * (BLK * B)
                            ro = base + c * (GLP * CO) + bl * bout
                            nc.tensor.matmul(
                                ps[:],
                                lhsT=t[:, lo:lo + BLK * B],
                                rhs=t[:, ro:ro + bout],
                                start=(c == 0), stop=(c == NCHUNK - 1))
                        nc.vector.tensor_copy(
                            ot[:, (g * NBLK + bl) * bout:(g * NBLK + bl + 1) * bout], ps[:])
                    if g == GROUPS - 2:
                        # first 3/4 of the output leaves while group 3 computes
                        nc.gpsimd.dma_start(out.ap()[:, :3 * gout], ot[:, :3 * gout])
                nc.gpsimd.dma_start(out.ap()[:, 3 * gout:], ot[:, 3 * gout:])
            if repeat == 1:
                body()
            else:
                with tc.For_i(0, repeat, 1):
                    body()
    nc.compile()
    return nc


def _host_prep_v10(x, weight, npdt=None):
    if npdt is None:
        npdt = np.float16
    maps = _host_prep_v89(x, weight, three_term=False, npdt=npdt)
    gw = V89_GW
    gwin = V89_GWIN
    out_maps = []
    for m in maps:
        wh = m["wh"].reshape(CK, GROUPS, gw)
        vh = m["winh"].reshape(CK, GROUPS, gwin)
        wx = np.concatenate([wh, vh], axis=2).reshape(CK, GROUPS * V10_GTOT)
        out_maps.append({"wx": np.ascontiguousarray(wx)})
    return out_maps


def _host_prep_v4(x, weight, variant):
    dt, BLK, npdt = V4_CFG[variant]
    x = np.ascontiguousarray(np.asarray(x, dtype=np.float32))
    weight = np.ascontiguousarray(np.asarray(weight, dtype=np.float32))
    wins = np.stack(
        [x[:, :, kh:kh + DH * HO:DH, kw:kw + DW * WO:DW]
         for kh in range(KH) for kw in range(KW)],
        axis=-1,
    )
    W2 = weight[0].transpose(1, 4, 2, 3, 0).reshape(IK, HO, WO, CO)
    W3 = wins.transpose(1, 4, 2, 3, 0).reshape(IK, HO, WO, B)
    qpad = list(range(WO)) + [WO - 1]          # 31 real + 1 dup -> 32
    in_maps = []
    for c in range(N_CORES):
        rows = _ROWS_PADDED[c]
        # (ik, group, locp, {o|b})
        wsel = W2[:, rows][:, :, qpad, :]       # (288, 4, 32, CO)
        winsel = W3[:, rows][:, :, qpad, :]     # (288, 4, 32, B)
        # -> [group, CK, chunk, locp, {o|b}] -> [GROUPS*CK, chunk*locp*{o|b}]
        wstk = np.stack([wsel[CK * cc:CK * (cc + 1)] for cc in range(NCHUNK)], axis=2)
        winstk = np.stack([winsel[CK * cc:CK * (cc + 1)] for cc in range(NCHUNK)], axis=2)
        # wstk: (CK, 4, chunk, 32, CO) -> (4, CK, chunk, 32, CO)
        wstk = wstk.transpose(1, 0, 2, 3, 4).reshape(GROUPS * CK, NCHUNK * GLP * CO)
        winstk = winstk.transpose(1, 0, 2, 3, 4).reshape(GROUPS * CK, NCHUNK * GLP * B)
        in_maps.append({
            "wT": np.ascontiguousarray(wstk.astype(npdt)),
            "winT": np.ascontiguousarray(winstk.astype(npdt)),
        })
    return in_maps


def _assemble_v4(results, variant):
    dt, BLK, _ = V4_CFG[variant]
    NBLK = GLP // BLK
    out = np.empty((B, CO, HO, WO), np.float32)
    idx = np.arange(BLK)
    for c in range(N_CORES):
        nreal = RPC if c < N_CORES - 1 else HO - 4 * (N_CORES - 1)
        buf = np.asarray(results[c]["out"])
        b6 = buf.reshape(BLK, B, GROUPS, NBLK, BLK, CO)
        d = b6[idx, :, :, :, idx, :]            # (BLK, B, GROUPS, NBLK, CO)
        dd = d.transpose(1, 4, 2, 3, 0).reshape(B, CO, GROUPS, NBLK * BLK)
        out[:, :, 4 * c:4 * c + nreal, :] = dd[:, :, :nreal, :WO]
    return out


def _build_nc(repeat=1, variant="v2"):
    nc = bacc.Bacc("TRN2", target_bir_lowering=False)
    wT = nc.dram_tensor("wT", [CK, W_COLS], mybir.dt.float32, kind="ExternalInput")
    winT = nc.dram_tensor("winT", [CK, WIN_COLS], mybir.dt.float32, kind="ExternalInput")
    out_cols = OUT_COLS if variant == "v1" else V2_OUT_COLS
    out_rows = CO if variant == "v1" else 128
    out = nc.dram_tensor("out", [out_rows, out_cols], mybir.dt.float32, kind="ExternalOutput")

    gw = GL * NCHUNK * CO    # weight cols per group
    gwin = GL * NCHUNK * B   # window cols per group
    gout = GL * B            # v1 out cols per group

    with tile.TileContext(nc) as tc:
        with (
            tc.tile_pool(name="wp", bufs=3) as wp,
            tc.tile_pool(name="winp", bufs=3) as winp,
            tc.tile_pool(name="pp", bufs=2, space="PSUM") as pp,
            tc.tile_pool(name="op", bufs=2) as op,
        ):
            def body_v1():
                for g in range(GROUPS):
                    wt = wp.tile([CK, gw], mybir.dt.float32, tag="wt", name="wt")
                    nc.sync.dma_start(wt[:], wT.ap()[:, g * gw:(g + 1) * gw])
                    wint = winp.tile([CK, gwin], mybir.dt.float32, tag="wint", name="wint")
                    nc.sync.dma_start(wint[:], winT.ap()[:, g * gwin:(g + 1) * gwin])

                    ps = pp.tile([CO, gout], mybir.dt.float32, tag="ps", name="ps")
                    for l in range(GL):
                        for c in range(NCHUNK):
                            nc.tensor.matmul(
                                ps[:, l * B:(l + 1) * B],
                                lhsT=wt[:, (l * NCHUNK + c) * CO:(l * NCHUNK + c + 1) * CO],
                                rhs=wint[:, (l * NCHUNK + c) * B:(l * NCHUNK + c + 1) * B],
                                start=(c == 0),
                                stop=(c == NCHUNK - 1),
                            )

                    ot = op.tile([CO, gout], mybir.dt.float32, tag="ot", name="ot")
                    nc.vector.tensor_copy(ot[:], ps[:])
                    nc.sync.dma_start(out.ap()[:, g * gout:(g + 1) * gout], ot[:])

            def body_v2():
                # stationary = windows (8 cols, cheap fp32 self-load);
                # moving = weight (N=32); out[b, o] block at partition
                # offset 32*(l%4) via col-tiling -> 4 concurrent MM strips.
                for g in range(GROUPS):
                    wt = wp.tile([CK, gw], mybir.dt.float32, tag="wt", name="wt")
                    nc.sync.dma_start(wt[:], wT.ap()[:, g * gw:(g + 1) * gw])
                    wint = winp.tile([CK, gwin], mybir.dt.float32, tag="wint", name="wint")
                    nc.sync.dma_start(wint[:], winT.ap()[:, g * gwin:(g + 1) * gwin])

                    # one PSUM tile per col strip -> different banks, so the
                    # 4 strips' matmuls aren't serialized by bank tracking
                    pss = [
                        pp.tile([128, V2_GOUT], mybir.dt.float32,
                                tag=f"ps{j}", name=f"ps{j}", bufs=2)
                        for j in range(4)
                    ]
                    for l in range(GL):
                        j = l % 4
                        blk = l // 4
                        for c in range(NCHUNK):
                            nc.tensor.matmul(
                                pss[j][32 * j:32 * j + B, blk * CO:(blk + 1) * CO],
                                lhsT=wint[:, (l * NCHUNK + c) * B:(l * NCHUNK + c + 1) * B],
                                rhs=wt[:, (l * NCHUNK + c) * CO:(l * NCHUNK + c + 1) * CO],
                                start=(c == 0),
                                stop=(c == NCHUNK - 1),
                                tile_position=(0, 32 * j),
                            )

                    ot = op.tile([128, V2_GOUT], mybir.dt.float32, tag="ot", name="ot")
                    for j in range(4):
                        nc.vector.tensor_copy(
                            ot[32 * j:32 * (j + 1), :],
                            pss[j][32 * j:32 * (j + 1), :],
                        )
                    nc.sync.dma_start(out.ap()[:, g * V2_GOUT:(g + 1) * V2_GOUT], ot[:])

            body = body_v1 if variant == "v1" else body_v2
            if repeat == 1:
                body()
            else:
                with tc.For_i(0, repeat, 1):
                    body()
    nc.compile()
    return nc


def _host_prep(x, weight):
    """Build per-core DMA-ready layouts. Pure indexing/transpose, no math."""
    x = np.ascontiguousarray(np.asarray(x, dtype=np.float32))
    weight = np.ascontiguousarray(np.asarray(weight, dtype=np.float32))

    # windows[b, i, p, q, k] with k = kh*3+kw (matches torch unfold flatten)
    wins = np.stack(
        [x[:, :, kh:kh + DH * HO:DH, kw:kw + DW * WO:DW]
         for kh in range(KH) for kw in range(KW)],
        axis=-1,
    )  # (B, CI, HO, WO, 9)

    # (ik, p, q, o) and (ik, p, q, b)
    W2 = weight[0].transpose(1, 4, 2, 3, 0).reshape(IK, HO, WO, CO)
    W3 = wins.transpose(1, 4, 2, 3, 0).reshape(IK, HO, WO, B)

    in_maps = []
    for c in range(N_CORES):
        rows = _ROWS_PADDED[c]
        wsel = W2[:, rows].reshape(IK, L, CO)
        winsel = W3[:, rows].reshape(IK, L, B)
        # [CK, loc, chunk, {o|b}] — partition r of chunk-c col region holds ik=96c+r
        wT = np.stack([wsel[CK * cc:CK * (cc + 1)] for cc in range(NCHUNK)], axis=2)
        winT = np.stack([winsel[CK * cc:CK * (cc + 1)] for cc in range(NCHUNK)], axis=2)
        in_maps.append({
            "wT": np.ascontiguousarray(wT.reshape(CK, W_COLS)),
            "winT": np.ascontiguousarray(winT.reshape(CK, WIN_COLS)),
        })
    return in_maps


def _assemble(results, variant="v2"):
    out = np.empty((B, CO, HO, WO), np.float32)
    qs = np.arange(WO)
    for c in range(N_CORES):
        nreal = RPC if c < N_CORES - 1 else HO - 4 * (N_CORES - 1)
        buf = np.asarray(results[c]["out"])
        if variant == "v1":
            rr = buf.reshape(CO, RPC, WO, B)
            for j in range(nreal):
                out[:, :, 4 * c + j, :] = rr[:, j, :, :].transpose(2, 0, 1)
        else:
            # buf [128, GROUPS*256]: row = 32*(q%4)+b, col = g*256+(q//4)*32+o
            b4 = buf.reshape(4, 32, GROUPS, 8, CO)
            res = b4[qs % 4, :B, :, qs // 4, :]      # (31, b, g, o)
            out[:, :, 4 * c:4 * c + nreal, :] = res.transpose(1, 3, 2, 0)[:, :, :nreal, :]
    return out


VARIANT = os.environ.get("LC2D_VARIANT", "v13")


def timing_setup(x, weight):
    """(in_maps, build_fn) for test.py's slope timing."""
    if VARIANT == "v13":
        return _host_prep_v13(x, weight), (lambda n: _build_nc_v13(n))
    if VARIANT == "v12":
        return _host_prep_v12(x, weight), (lambda n: _build_nc_v12(n))
    if VARIANT == "v11":
        return _host_prep_v11(x, weight), (lambda n: _build_nc_v11(n))
    if VARIANT == "v10":
        return _host_prep_v10(x, weight), (lambda n: _build_nc_v10(n))
    raise NotImplementedError(VARIANT)


def kernel(x, weight, _trace=False, _trace_cores=None):
    if VARIANT == "v13":
        in_maps = _host_prep_v13(x, weight)
    elif VARIANT == "v12":
        in_maps = _host_prep_v12(x, weight)
    elif VARIANT == "v11":
        in_maps = _host_prep_v11(x, weight)
    elif VARIANT == "v10":
        in_maps = _host_prep_v10(x, weight)
    elif VARIANT in ("v8", "v9", "v8h", "v9h"):
        in_maps = _host_prep_v89(
            x, weight, three_term=(VARIANT in ("v9", "v9h")),
            npdt=(np.float16 if VARIANT.endswith("h") else ml_dtypes.bfloat16))
    elif VARIANT in V4_CFG:
        in_maps = _host_prep_v4(x, weight, VARIANT)
    elif VARIANT == "v5":
        in_maps = _host_prep_v5(x, weight)
    else:
        in_maps = _host_prep(x, weight)
    if "nc" not in _NC_CACHE:
        if VARIANT == "v13":
            _NC_CACHE["nc"] = _build_nc_v13(1)
        elif VARIANT == "v12":
            _NC_CACHE["nc"] = _build_nc_v12(1)
        elif VARIANT == "v11":
            _NC_CACHE["nc"] = _build_nc_v11(1)
        elif VARIANT == "v10":
            _NC_CACHE["nc"] = _build_nc_v10(1)
        elif VARIANT in ("v8", "v9", "v8h", "v9h"):
            _NC_CACHE["nc"] = _build_nc_v89(
                1, three_term=(VARIANT in ("v9", "v9h")),
                dt=(mybir.dt.float16 if VARIANT.endswith("h") else mybir.dt.bfloat16))
        elif VARIANT in V4_CFG:
            _NC_CACHE["nc"] = _build_nc_v4(1, VARIANT)
        elif VARIANT == "v5":
            _NC_CACHE["nc"] = _build_nc_v5()
        else:
            _NC_CACHE["nc"] = _build_nc(variant=VARIANT)
    nc = _NC_CACHE["nc"]
    res = run_bass_kernel_spmd(
        nc, in_maps, core_ids=list(range(N_CORES)),
        trace=_trace, trace_cores=_trace_cores,
    )
    if VARIANT == "v13":
        out = _assemble_v13(res.results)
    elif VARIANT == "v12":
        out = _assemble_v12(res.results)
    elif VARIANT == "v11":
        out = _assemble_v11(res.results)
    elif VARIANT in ("v8", "v9", "v8h", "v9h", "v10"):
        out = _assemble_v89(res.results)
    elif VARIANT in V4_CFG:
        out = _assemble_v4(res.results, VARIANT)
    elif VARIANT == "v5":
        out = _assemble_v5(res.results)
    else:
        out = _assemble(res.results, variant=VARIANT)
    if _trace:
        return out, res
    return out


if __name__ == "__main__":
    # quick self-check with random data against a numpy oracle
    rng = np.random.default_rng(0)
    x = rng.standard_normal((B, CI, H, W), dtype=np.float32)
    weight = rng.standard_normal((1, CO, CI, HO, WO, KH * KW), dtype=np.float32)
    wins = np.stack(
        [x[:, :, kh:kh + DH * HO:DH, kw:kw + DW * WO:DW]
         for kh in range(KH) for kw in range(KW)], axis=-1)
    expected = np.einsum("bipqk,oipqk->bopq", wins, weight[0], optimize=True)
    actual = kernel(x, weight)
    err = np.abs(actual - expected).max() / np.abs(expected).max()
    print("max out:", np.abs(expected).max(), "rel err:", err)
    tol = 1e-5 if VARIANT in ("v1", "v2", "v5") else (1e-2 if VARIANT in ("v8", "v4b", "v4b8") else 1e-3)
    assert err < tol, (err, tol)
    print("KERNEL OK")

